# revision 1
# baseline (speedup 1.0000x reference)
"""GNN message-passing (gated GCN style) on 8 Trainium2 NeuronCores.

Strategy (edge-parallel, dst-sorted shards):
- Host sorts edges by dst and splits into 8 shards snapped to node-run
  boundaries, so each device owns a contiguous node range and its complete
  incoming-edge runs. segment_max is fully local.
- Per device, nodes are sorted by in-degree and each node's run is padded to
  a per-tile power-of-2 slot count S, so segment_max becomes a fixed-window
  reduce_max over contiguous columns (feat-major).
- Per layer, each device computes per-node tables for its own node slice and
  AllGathers them (layer 0: [h@V | h@C] fp32; layer 1 / readout: bf16).
  Per-edge src-side gathers run as batched 512-row dma_gather calls
  (single_packet=False): layer 0 edge-major + PE transposes accumulated in
  PSUM, layer 1 / readout transposing gathers (feat-major, no PE transpose).
  int16 gather indices only span 32K rows, so the 8-shard table is split in
  two 4-shard halves gathered separately and summed; a zeros row absorbs the
  other half, a -1e30 guard row keeps empty-slot max semantics.
- h@B (dst side, sorted) is expanded with a constant kron-pattern matmul.
- BatchNorm statistics are masked sums reduced on-chip and combined with a
  small AllGather + on-chip reduction per layer (AllReduce is ~213us here).
  h-side loops run 4 node-tiles wide; inputs are packed into 6 tensors to
  amortize per-argument dispatch cost.
- The readout MLP runs feat-major per <=512-edge group; h@W0b / h@W0c are
  pre-folded into the final AllGather payload / local table.
"""

import numpy as np

NC = 8
D = 128
MEGA = 4  # max chunks per dma_gather call (HW limit: 512 indices)


# ---------------------------------------------------------------------------
# host-side planning
# ---------------------------------------------------------------------------


def _next_pow2(x):
    p = 1
    while p < x:
        p *= 2
    return p


def _pack_layout(plan):
    """Row layout for the packed fp32 [RW,384] and bf16 [RB,256] const
    tensors. Column-vector consts are stored as single rows and
    transposed by the load DMA."""
    S_vals = sorted(set(plan["S_list"]))
    F_N, F_E = plan["F_N"], plan["F_E"]
    w = {}
    r = 0

    def add(name, rows):
        nonlocal r
        w[name] = (r, rows)
        r += rows

    add("ones_row", 1)
    add("identb", 128)
    add("fill2", 2)
    add("emb_e_w", F_E)
    add("emb_n_w", F_N)
    add("emb_e_b", 1)
    add("emb_n_b", 1)
    add("A0", 128)
    add("A1", 128)
    add("VCB0", 128)
    add("V1", 128)
    add("U0", 128)
    add("U1", 128)
    add("WBC", 128)
    add("W0a", 128)
    add("W0b_col", 1)
    add("Wkb0", 1)
    add("Wkb1", 1)
    add("wfb", 1)
    for i in range(len(S_vals)):
        add(f"krons{i}", 128)
    RW = r
    b = {}
    r = 0

    def addb(name, rows):
        nonlocal r
        b[name] = (r, rows)
        r += rows

    addb("fill2b", 2)
    for i in range(len(S_vals)):
        addb(f"kronsb{i}", 128)
    addb("Wk0", 128)
    addb("Wk1", 128)
    addb("Wf", 1)
    RB = r
    return w, RW, b, RB


def _plan(src, dst, N):
    E = src.shape[0]
    order = np.argsort(dst, kind="stable")
    dsts = dst[order]
    srcs = src[order]

    # shard boundaries snapped to run starts
    bounds = [0]
    for r in range(1, NC):
        t = (E * r) // NC
        b = int(np.searchsorted(dsts, dsts[t], side="left"))
        bounds.append(max(b, bounds[-1]))
    bounds.append(E)

    lo = np.zeros(NC, np.int64)
    for d in range(1, NC):
        lo[d] = int(dsts[bounds[d]]) if bounds[d] < E else N
    hi = np.empty(NC, np.int64)
    hi[:-1] = lo[1:]
    hi[-1] = N

    n_r = [int(hi[d] - lo[d]) for d in range(NC)]
    NODE_CAP = 128 * int(np.ceil((max(n_r) + 2) / 128))
    T = NODE_CAP // 128

    lo = np.asarray(lo)
    shards = []
    for d in range(NC):
        sl = slice(bounds[d], bounds[d + 1])
        dl = dsts[sl] - lo[d]
        cnt = np.bincount(dl, minlength=n_r[d]) if n_r[d] > 0 else np.zeros(0, int)
        starts = np.concatenate([[0], np.cumsum(cnt)])
        perm = np.argsort(-cnt, kind="stable") if n_r[d] > 0 else np.zeros(0, int)
        ipos = np.empty(n_r[d], np.int64)
        ipos[perm] = np.arange(n_r[d])
        shards.append(
            dict(sl=sl, dl=dl, cnt=cnt, starts=starts, perm=perm, ipos=ipos, d=d)
        )

    # shared per-tile slot counts
    S_list = []
    for t in range(T):
        mx = 1
        for sh in shards:
            p = sh["perm"][t * 128 : (t + 1) * 128]
            if len(p):
                c = sh["cnt"][p]
                if len(c):
                    mx = max(mx, int(c.max()))
        S_list.append(min(_next_pow2(mx), 128))

    E_PAD = 128 * int(np.sum(S_list))
    C_E = E_PAD // 128
    chunks = []  # (tile, ci, S)
    for t in range(T):
        for ci in range(S_list[t]):
            chunks.append((t, ci, S_list[t]))
    groups = []  # (c0, gsz) — tile-aligned: all chunks in a group share a tile
    c = 0
    for t in range(T):
        S = S_list[t]
        ci = 0
        while ci < S:
            g = min(4, S - ci)
            groups.append((c + ci, g))
            ci += g
        c += S
    # megas: runs of whole groups, <= MEGA chunks per run
    megas = []  # (cm, nch, [group idx])
    cur = None
    for gi, (c0, gsz) in enumerate(groups):
        if cur is None or cur[1] + gsz > MEGA:
            cur = [c0, 0, []]
            megas.append(cur)
        cur[1] += gsz
        cur[2].append(gi)

    return dict(
        E=E,
        N=N,
        order=order,
        srcs=srcs,
        bounds=bounds,
        lo=np.array(lo),
        hi=np.array(hi),
        n_r=n_r,
        NODE_CAP=NODE_CAP,
        T=T,
        S_list=S_list,
        E_PAD=E_PAD,
        C_E=C_E,
        chunks=chunks,
        groups=groups,
        megas=[tuple(m) for m in megas],
        shards=shards,
    )


def _per_core_arrays(plan, d, h, e):
    """Build padded per-core host arrays for shard d."""
    sh = plan["shards"][d]
    NODE_CAP, T = plan["NODE_CAP"], plan["T"]
    S_list = plan["S_list"]
    E_PAD, C_E = plan["E_PAD"], plan["C_E"]
    n_r = plan["n_r"][d]
    guard_row = d * NODE_CAP + (NODE_CAP - 1)

    e_sh = e[plan["order"]][sh["sl"]]  # [E_s, F_E]
    src_sh = plan["srcs"][sh["sl"]]
    orig_sh = np.arange(plan["E"])[plan["order"]][sh["sl"]]

    F_E = e.shape[1]
    e0_pad = np.zeros((E_PAD, F_E), np.float32)
    srcrow = np.full(E_PAD, guard_row, np.int64)
    maskf = np.zeros(E_PAD, np.float32)
    origid = np.full(E_PAD, -1, np.int64)

    base = 0
    perm = sh["perm"]
    cnt = sh["cnt"]
    starts = sh["starts"]
    rank_of = lambda g: np.clip(
        np.searchsorted(plan["lo"], g, side="right") - 1, 0, NC - 1
    )
    for t in range(T):
        S = S_list[t]
        pn = perm[t * 128 : (t + 1) * 128]
        # index matrix [128, S] of local edge positions, -1 = dummy
        im = np.full((128, S), -1, np.int64)
        for i, n in enumerate(pn):
            dg = int(cnt[n])
            k = min(dg, S)
            if k:
                im[i, :k] = np.arange(starts[n], starts[n] + k)
        flat = im.reshape(-1)
        real = flat >= 0
        fr = flat[real]
        blk = slice(base, base + 128 * S)
        e0_blk = np.zeros((128 * S, F_E), np.float32)
        e0_blk[real] = e_sh[fr]
        e0_pad[blk] = e0_blk
        sr = np.full(128 * S, guard_row, np.int64)
        g = src_sh[fr]
        r = rank_of(g)
        loc = g - plan["lo"][r]
        pp_ = np.empty(len(g), np.int64)
        for rr in np.unique(r):
            m = r == rr
            pp_[m] = plan["shards"][rr]["ipos"][loc[m]]
        sr[real] = r * NODE_CAP + pp_
        srcrow[blk] = sr
        mk = np.zeros(128 * S, np.float32)
        mk[real] = 1.0
        maskf[blk] = mk
        oi = np.full(128 * S, -1, np.int64)
        oi[real] = orig_sh[fr]
        origid[blk] = oi
        base += 128 * S

    # edge slot i (= c*128+p) -> srcrow; the slot order the e-side pipeline
    # uses IS this flat order.  dma_gather consumes indices 16-wrapped:
    # idx[q, j] = slot j*16+q, replicated over the 8 16-partition blocks.
    HALF = 4 * NODE_CAP
    memberA = srcrow < HALF
    idxA = np.where(memberA, srcrow, NODE_CAP - 2).astype(np.int16)
    idxB = np.where(~memberA, srcrow - HALF, NODE_CAP - 2).astype(np.int16)

    def wrap16(a):
        w = a.reshape(E_PAD // 16, 16).T  # [16, E_PAD/16]
        return np.ascontiguousarray(np.tile(w, (8, 1)))

    ipack = np.concatenate([wrap16(idxA), wrap16(idxB)], axis=1)

    # mpack: mask_e rows [NG, 512] then nodemask rows [NW, 512] (4 tiles/row)
    NG = len(plan["groups"])
    NW = (T + 3) // 4
    mpack = np.zeros((NG + NW, 512), np.float32)
    for gi, (c0, gsz) in enumerate(plan["groups"]):
        mpack[gi, : gsz * 128] = maskf[c0 * 128 : (c0 + gsz) * 128]
    nm = np.zeros(NW * 512, np.float32)
    nm[:n_r] = 1.0
    mpack[NG:, :] = nm.reshape(NW, 512)
    # h0T [F_N, NODE_CAP] permuted
    F_N = h.shape[1]
    h0p = np.zeros((NODE_CAP, F_N), np.float32)
    hl = h[plan["lo"][d] : plan["hi"][d]]
    h0p[: len(perm)] = hl[perm]
    h0T = np.ascontiguousarray(h0p.T)

    return dict(
        h0T=h0T,
        e0T=np.ascontiguousarray(e0_pad.T),
        ipack=ipack,
        mpack=mpack,
        origid=origid,
    )


# ---------------------------------------------------------------------------
# device program
# ---------------------------------------------------------------------------


def _build_program(plan, stop_after="full", epochs=1):
    import concourse.bass as bass
    import concourse.mybir as mybir
    import concourse.tile as tile
    from concourse import bacc

    F32 = mybir.dt.float32
    BF16 = mybir.dt.bfloat16
    I16 = mybir.dt.int16
    AF = mybir.ActivationFunctionType
    OP = mybir.AluOpType
    AX = mybir.AxisListType

    NODE_CAP, T = plan["NODE_CAP"], plan["T"]
    E_PAD, C_E = plan["E_PAD"], plan["C_E"]
    chunks, groups, megas = plan["chunks"], plan["groups"], plan["megas"]
    NG = len(groups)
    N, E = plan["N"], plan["E"]
    S_vals = sorted(set(plan["S_list"]))
    kron_of = {s: i for i, s in enumerate(S_vals)}
    F_N, F_E = plan["F_N"], plan["F_E"]
    EPS = 1e-5
    HALF = 4 * NODE_CAP
    NI16 = E_PAD // 16

    _phases = ["embed", "bound0", "epass0", "layer0", "layer1", "full"]
    lvl = _phases.index(stop_after)

    nc = bacc.Bacc(
        "TRN2", target_bir_lowering=False, debug=False, num_devices=NC
    )

    def din(name, shape, dt=F32):
        return nc.dram_tensor(name, shape, dt, kind="ExternalInput")

    # per-core inputs
    WMAP, RW, BMAP, RB = _pack_layout(plan)
    h0T = din("h0T", [F_N, NODE_CAP])
    e0T = din("e0T", [F_E, E_PAD])
    ipack_d = din("ipack", [128, 2 * NI16], I16)
    NW = (T + 3) // 4
    mpack_d = din("mpack", [NG + NW, 512])
    wpack_d = din("wpack", [RW, 384])
    bpack_d = din("bpack", [RB, 256], BF16)

    y_out = nc.dram_tensor("y", [1, E_PAD], F32, kind="ExternalOutput")

    rg = [list(range(NC))]

    with tile.TileContext(nc) as tc:
        with (
            tc.tile_pool(name="const", bufs=1) as cp,
            tc.tile_pool(name="pers", bufs=1) as pp,
            tc.tile_pool(name="st", bufs=1) as stp,
            tc.tile_pool(name="s", bufs=2) as sp,
            tc.tile_pool(name="ps", bufs=2, space="PSUM") as ps,
            tc.tile_pool(name="dram", bufs=1, space="DRAM") as dp,
        ):
            # ---- load constants from packs
            def wload(name, width, dt=F32, pack=None, pmap=None):
                pk = pack if pack is not None else wpack_d
                mp = pmap if pmap is not None else WMAP
                r0, rows = mp[name]
                t = cp.tile([rows, width], dt, name=f"{name}_sb")
                nc.sync.dma_start(out=t[:], in_=pk[r0 : r0 + rows, :width])
                return t

            def wload_col(name, dt=F32, pack=None, pmap=None):
                pk = pack if pack is not None else wpack_d
                mp = pmap if pmap is not None else WMAP
                r0, rows = mp[name]
                t = cp.tile([128, 1], dt, name=f"{name}_sb")
                nc.sync.dma_start(
                    out=t[:], in_=pk[r0 : r0 + 1, :128].rearrange("a p -> p a")
                )
                return t

            def bload(name, width, dt=BF16):
                return wload(name, width, dt, pack=bpack_d, pmap=BMAP)

            ones_row = wload("ones_row", 128)
            identb = wload("identb", 128)
            embe_w = wload("emb_e_w", 128)
            embn_w = wload("emb_n_w", 128)
            embe_b = wload_col("emb_e_b")
            embn_b = wload_col("emb_n_b")
            A_sb = [wload(f"A{l}", 128) for l in range(2)]
            VCB0 = wload("VCB0", 384)
            V1 = wload("V1", 128)
            U_sb = [wload(f"U{l}", 128) for l in range(2)]
            WBC = wload("WBC", 256)
            W0a = wload("W0a", 128)
            W0bc = wload_col("W0b_col")
            Wk = [bload(f"Wk{k}", 128) for k in range(2)]
            Wkb = [wload_col(f"Wkb{k}") for k in range(2)]
            Wfr0, _ = BMAP["Wf"]
            Wf = cp.tile([128, 1], BF16, name="Wf_sb")
            nc.sync.dma_start(
                out=Wf[:], in_=bpack_d[Wfr0 : Wfr0 + 1, :128].rearrange("a p -> p a")
            )
            wfb = wload("wfb", 1)
            kron_sb = [wload(f"krons{i}", 128) for i in range(len(S_vals))]
            kron_bb = [bload(f"kronsb{i}", 128) for i in range(len(S_vals))]
            ipack = cp.tile([128, 2 * NI16], I16, name="ipack_sb")
            nc.sync.dma_start(out=ipack[:], in_=ipack_d[:])
            eps_col = cp.tile([128, 1], F32, name="eps_col")
            nc.gpsimd.memset(eps_col[:], EPS)

            # ---- dram buffers
            e_buf = dp.tile([128, E_PAD], F32, name="e_buf")
            z_buf = dp.tile([128, E_PAD], F32, name="z_buf")
            hb_buf = dp.tile([NODE_CAP, 128], F32, name="hb_buf")
            hfm_buf = [
                dp.tile([128, NODE_CAP], F32, name=f"hfm_buf{i}")
                for i in range(3)
            ]
            hlocal = dp.tile([NODE_CAP, 128], BF16, name="hlocal")
            cc_hin = [
                dp.tile(
                    [NODE_CAP, 256 if l == 0 else 128],
                    F32 if l == 0 else BF16,
                    name=f"cc_hin{l}",
                )
                for l in range(3)
            ]
            cc_hout_ep = [
                [
                    dp.tile(
                        [NC * NODE_CAP, 256 if l == 0 else 128],
                        F32 if l == 0 else BF16,
                        name=f"cc_hout{l}_e{e_}",
                        addr_space="Shared",
                    )
                    for l in range(3)
                ]
                for e_ in range(epochs)
            ]
            cc_st_in = [
                dp.tile([128, 4 if l == 0 else 2], F32, name=f"cc_st_in{l}")
                for l in range(2)
            ]
            cc_st_out_ep = [
                [
                    dp.tile(
                        [NC * 128, 4 if l == 0 else 2],
                        F32,
                        name=f"cc_st_out{l}_e{e_}",
                        addr_space="Shared",
                    )
                    for l in range(2)
                ]
                for e_ in range(epochs)
            ]
            cc_moy_in = dp.tile([128, 1], F32, name="cc_moy_in")
            cc_moy_out_ep = [
                dp.tile(
                    [NC * 128, 1], F32, name=f"cc_moy_out_e{e_}",
                    addr_space="Shared",
                )
                for e_ in range(epochs)
            ]

            def _epoch(ep):
                cc_hout = cc_hout_ep[ep]
                cc_st_out = cc_st_out_ep[ep]
                cc_moy_out = cc_moy_out_ep[ep]
                # ---- persistent sbuf tiles
                hU = [None] * NW
                agg = [None] * NW

                def group_gather_t(c0, gsz, l):
                    """Dual transposing gathers (bf16 feat-major) + merge:
                    gv[p, k*128 + q] = sum_half table[idx[(c0+k)*128+q], p]."""
                    n = gsz * 128
                    ga = sp.tile([128, 512], BF16, tag="gva", bufs=2)
                    gb = sp.tile([128, 512], BF16, tag="gvb", bufs=2)
                    gv = sp.tile([128, 512], BF16, tag="gvm", bufs=2)
                    for g, ioff, r0 in ((ga, 0, 0), (gb, NI16, HALF)):
                        nc.gpsimd.dma_gather(
                            g[:, :n].rearrange("p (j i) -> p j i", i=n),
                            cc_hout[l][r0 : r0 + HALF, :],
                            ipack[:, ioff + c0 * 8 : ioff + (c0 + gsz) * 8],
                            n,
                            n,
                            128,
                            transpose=True,
                            single_packet=False,
                        )
                    nc.vector.tensor_tensor(
                        out=gv[:, :n], in0=ga[:, :n], in1=gb[:, :n], op=OP.add
                    )
                    return gv

                def group_gather(c0, gsz, l):
                    """Dual half-table row gather for one group (edge-major):
                    gt?[p, k*W + f] = table_half[idx?[(c0+k)*128+p], f]."""
                    W = 256 if l == 0 else 128
                    n = gsz * 128
                    ga = sp.tile([128, 4 * 256], F32, tag="gta", bufs=2)
                    gb = sp.tile([128, 4 * 256], F32, tag="gtb", bufs=2)
                    for g, ioff, r0 in ((ga, 0, 0), (gb, NI16, HALF)):
                        nc.gpsimd.dma_gather(
                            g[:, : gsz * W].rearrange("p (k w) -> p k w", w=W),
                            cc_hout[l][r0 : r0 + HALF, :],
                            ipack[:, ioff + c0 * 8 : ioff + (c0 + gsz) * 8],
                            n,
                            n,
                            W,
                            transpose=False,
                            single_packet=False,
                        )
                    return ga, gb

                # ================= embed e =================
                for gi, (c0, gsz) in enumerate(groups):
                    w = gsz * 128
                    e0sl = sp.tile([F_E, 512], F32, tag="e0sl")
                    nc.sync.dma_start(
                        out=e0sl[:, :w], in_=e0T[:, c0 * 128 : c0 * 128 + w]
                    )
                    pe = ps.tile([128, 512], F32, tag="pa")
                    nc.tensor.matmul(
                        out=pe[:, :w], lhsT=embe_w[:], rhs=e0sl[:, :w],
                        start=True, stop=True,
                    )
                    esb = sp.tile([128, 512], F32, tag="esb", bufs=2)
                    nc.scalar.activation(
                        out=esb[:, :w], in_=pe[:, :w], func=AF.Identity,
                        bias=embe_b[:],
                    )
                    nc.sync.dma_start(
                        out=e_buf[:, c0 * 128 : c0 * 128 + w], in_=esb[:, :w]
                    )

                # ================= embed h =================
                for t in range(T):
                    h0sl = sp.tile([F_N, 128], F32, tag="h0sl", bufs=2, name="h0sl")
                    nc.sync.dma_start(
                        out=h0sl[:], in_=h0T[:, t * 128 : (t + 1) * 128]
                    )
                    ph = ps.tile([128, 128], F32, tag="pc")
                    nc.tensor.matmul(
                        out=ph[:], lhsT=embn_w[:], rhs=h0sl[:],
                        start=True, stop=True,
                    )
                    hf = sp.tile([128, 128], F32, tag="hnew", bufs=4, name="hemb")
                    nc.scalar.activation(
                        out=hf[:], in_=ph[:], func=AF.Identity, bias=embn_b[:]
                    )
                    nc.sync.dma_start(
                        out=hfm_buf[0][:, t * 128 : (t + 1) * 128], in_=hf[:]
                    )

                # ================= boundary =================
                def boundary(l):
                    """Build tables for layer l (or readout if l==2) from hfm."""
                    if l == 0:
                        rhs, wdt, U = VCB0, 384, U_sb[0]
                    elif l == 1:
                        rhs, wdt, U = V1, 128, U_sb[1]
                    else:
                        rhs, wdt, U = WBC, 256, None
                    scat_w = 256 if l == 0 else 128
                    bdt = F32 if l == 0 else BF16
                    for t in range(T):
                        hfl = sp.tile([128, 128], F32, tag="hfl", bufs=4, name="hfl")
                        nc.sync.dma_start(
                            out=hfl[:], in_=hfm_buf[l][:, t * 128 : (t + 1) * 128]
                        )
                        pb = ps.tile([128, 512], F32, tag="pa")
                        nc.tensor.matmul(
                            out=pb[:, :wdt], lhsT=hfl[:], rhs=rhs[:],
                            start=True, stop=True,
                        )
                        bsb = sp.tile([128, 512], bdt, tag="bsb" if l == 0 else "bsbb", bufs=2)
                        nc.scalar.activation(
                            out=bsb[:, :wdt], in_=pb[:, :wdt], func=AF.Copy
                        )
                        nc.sync.dma_start(
                            out=cc_hin[l][t * 128 : (t + 1) * 128, :scat_w],
                            in_=bsb[:, :scat_w],
                        )
                        if l == 0:
                            nc.sync.dma_start(
                                out=hb_buf[t * 128 : (t + 1) * 128, :],
                                in_=bsb[:, 256:384],
                            )
                        if l == 2:
                            nc.sync.dma_start(
                                out=hlocal[t * 128 : (t + 1) * 128, :],
                                in_=bsb[:, 128:256],
                            )
                        if U is not None:
                            pu = ps.tile([128, 128], F32, tag="pc")
                            nc.tensor.matmul(
                                out=pu[:], lhsT=U[:], rhs=hfl[:],
                                start=True, stop=True,
                            )
                            if t % 4 == 0:
                                hU[t // 4] = pp.tile(
                                    [128, 512], F32, tag=f"hUw{t // 4}",
                                    name=f"hUw_{t // 4}_{l}_{ep}",
                                )
                            nc.scalar.activation(
                                out=hU[t // 4][
                                    :, (t % 4) * 128 : (t % 4) * 128 + 128
                                ],
                                in_=pu[:], func=AF.Copy,
                            )
                    # zero row (NODE_CAP-2) + guard row (NODE_CAP-1)
                    if l == 0:
                        fr0, _ = WMAP["fill2"]
                        nc.sync.dma_start(
                            out=cc_hin[l][NODE_CAP - 2 : NODE_CAP, :scat_w],
                            in_=wpack_d[fr0 : fr0 + 2, :scat_w],
                        )
                    else:
                        fr0, _ = BMAP["fill2b"]
                        nc.sync.dma_start(
                            out=cc_hin[l][NODE_CAP - 2 : NODE_CAP, :scat_w],
                            in_=bpack_d[fr0 : fr0 + 2, :scat_w],
                        )
                    nc.gpsimd.collective_compute(
                        "AllGather",
                        OP.bypass,
                        replica_groups=rg,
                        ins=[cc_hin[l][:]],
                        outs=[cc_hout[l][:]],
                    )

                if lvl >= 1:
                    boundary(0)

                # ================= layers =================
                for l in range(2):
                    if l == 0 and lvl < 2:
                        break
                    if l == 1 and lvl < 4:
                        break
                    # stats accumulators
                    if l == 0:
                        ssum_e = stp.tile([128, NG], F32, name=f"ssum_e{ep}")
                        ssq_e = stp.tile([128, NG], F32, name=f"ssq_e{ep}")
                    hsum = stp.tile([128, NW], F32, name=f"hsum{l}_{ep}")
                    hssq = stp.tile([128, NW], F32, name=f"hssq{l}_{ep}")

                    # ---- e-pass (layer 1's is fused into the l==0 e-update)
                    for gi, (c0, gsz) in enumerate(groups if l == 0 else []):
                        if True:
                            w = gsz * 128
                            t, ci0, S = chunks[c0]
                            G = 128 // S
                            gta, gtb = group_gather(c0, gsz, 0)
                            esb = sp.tile([128, 512], F32, tag="esb", bufs=2)
                            nc.sync.dma_start(
                                out=esb[:, :w],
                                in_=e_buf[:, c0 * 128 : c0 * 128 + w],
                            )
                            wsb = sp.tile([128, 512], F32, tag="wsb", bufs=2)
                            nc.scalar.activation(
                                out=wsb[:, :w], in_=esb[:, :w], func=AF.Sigmoid
                            )
                            mrow = sp.tile([1, 512], F32, tag="mrow", bufs=1)
                            nc.sync.dma_start(out=mrow[:], in_=mpack_d[gi : gi + 1, :])
                            pm = ps.tile([128, 512], F32, tag="pb")
                            nc.tensor.matmul(
                                out=pm[:, :w], lhsT=ones_row[:], rhs=mrow[:, :w],
                                start=True, stop=True,
                            )
                            pz = ps.tile([128, 512], F32, tag="pa")
                            nc.tensor.matmul(
                                out=pz[:, :w], lhsT=A_sb[l][:], rhs=esb[:, :w],
                                start=True, stop=False, skip_group_check=True,
                            )
                            band = sp.tile([128, 512], F32, tag="hbt", bufs=2, name="band")
                            nc.sync.dma_start(
                                out=band[:G, : gsz * 128].rearrange(
                                    "p (k c) -> p k c", c=128
                                ),
                                in_=hb_buf[
                                    t * 128 + ci0 * G : t * 128 + (ci0 + gsz) * G, :
                                ].rearrange("(k p) c -> p k c", p=G),
                            )
                            for k in range(gsz):
                                nc.tensor.matmul(
                                    out=pz[:, k * 128 : (k + 1) * 128],
                                    lhsT=band[:G, k * 128 : (k + 1) * 128],
                                    rhs=kron_sb[kron_of[S]][:G, :],
                                    start=False, stop=(k == gsz - 1),
                                    skip_group_check=True,
                                )
                            # message path: msg = (hV0)[src] * w, windowed max
                            pd = ps.tile([128, 512], F32, tag="pdy", bufs=2, name="pd")
                            for k in range(gsz):
                                ci = ci0 + k
                                phv = ps.tile([128, 128], F32, tag="pc")
                                nc.tensor.matmul(
                                    out=phv[:], lhsT=gta[:, k * 256 : k * 256 + 128],
                                    rhs=identb[:], is_transpose=True,
                                    start=True, stop=False, skip_group_check=True,
                                )
                                nc.tensor.matmul(
                                    out=phv[:], lhsT=gtb[:, k * 256 : k * 256 + 128],
                                    rhs=identb[:], is_transpose=True,
                                    start=False, stop=True, skip_group_check=True,
                                )
                                msg = sp.tile([128, 128], F32, tag="msg", bufs=3)
                                nc.vector.tensor_tensor(
                                    out=msg[:], in0=phv[:],
                                    in1=wsb[:, k * 128 : (k + 1) * 128], op=OP.mult,
                                )
                                if ci == 0 and t % 4 == 0:
                                    agg[t // 4] = pp.tile(
                                        [128, 512], F32, tag=f"aggw{t // 4}",
                                        name=f"aggw_{t // 4}_0_{ep}",
                                    )
                                ao = (t % 4) * 128
                                nc.vector.tensor_reduce(
                                    out=agg[t // 4][
                                        :, ao + ci * G : ao + (ci + 1) * G
                                    ],
                                    in_=msg[:].rearrange("p (g s) -> p g s", s=S),
                                    op=OP.max,
                                    axis=AX.X,
                                )
                                nc.tensor.matmul(
                                    out=pd[:, k * 128 : (k + 1) * 128],
                                    lhsT=gta[:, k * 256 + 128 : k * 256 + 256],
                                    rhs=identb[:], is_transpose=True,
                                    start=True, stop=False, skip_group_check=True,
                                )
                                nc.tensor.matmul(
                                    out=pd[:, k * 128 : (k + 1) * 128],
                                    lhsT=gtb[:, k * 256 + 128 : k * 256 + 256],
                                    rhs=identb[:], is_transpose=True,
                                    start=False, stop=True, skip_group_check=True,
                                )
                            # z path: z = A e + (C h)[src] + kron(B h local)
                            zraw = sp.tile([128, 512], F32, tag="zraw", bufs=2)
                            nc.scalar.activation(
                                out=zraw[:, :w], in_=pz[:, :w], func=AF.Copy
                            )
                            zsum = sp.tile([128, 512], F32, tag="zsum", bufs=2)
                            nc.vector.tensor_tensor(
                                out=zsum[:, :w], in0=zraw[:, :w],
                                in1=pd[:, :w], op=OP.add,
                            )
                            zm = sp.tile([128, 512], F32, tag="zm", bufs=2)
                            nc.vector.tensor_tensor(
                                out=zm[:, :w], in0=zsum[:, :w], in1=pm[:, :w],
                                op=OP.mult,
                            )
                            nc.vector.tensor_reduce(
                                out=ssum_e[:, gi : gi + 1], in_=zm[:, :w],
                                op=OP.add, axis=AX.X,
                            )
                            sq = sp.tile([128, 512], F32, tag="sq", bufs=2)
                            nc.scalar.activation(
                                out=sq[:, :w], in_=zm[:, :w], func=AF.Square
                            )
                            nc.vector.tensor_reduce(
                                out=ssq_e[:, gi : gi + 1], in_=sq[:, :w],
                                op=OP.add, axis=AX.X,
                            )
                            nc.sync.dma_start(
                                out=z_buf[:, c0 * 128 : c0 * 128 + w],
                                in_=zm[:, :w],
                            )

                    if l == 0 and lvl < 3:
                        break

                    # ---- h side: z_h = hU + select(agg); masked stats (wide)
                    for j in range(NW):
                        wj = min(512, (T - 4 * j) * 128)
                        m01 = sp.tile([128, 512], F32, tag="zhm", bufs=3)
                        nc.vector.tensor_scalar(
                            out=m01[:, :wj], in0=agg[j][:, :wj], scalar1=-1e20,
                            scalar2=None, op0=OP.is_gt,
                        )
                        nc.vector.tensor_tensor(
                            out=agg[j][:, :wj], in0=agg[j][:, :wj],
                            in1=m01[:, :wj], op=OP.mult,
                        )
                        nc.vector.tensor_tensor(
                            out=agg[j][:, :wj], in0=agg[j][:, :wj],
                            in1=hU[j][:, :wj], op=OP.add,
                        )
                        nmr = sp.tile([1, 512], F32, tag="nmr", bufs=1)
                        nc.sync.dma_start(
                            out=nmr[:], in_=mpack_d[NG + j : NG + j + 1, :]
                        )
                        pnm = ps.tile([128, 512], F32, tag="pc")
                        nc.tensor.matmul(
                            out=pnm[:, :wj], lhsT=ones_row[:], rhs=nmr[:, :wj],
                            start=True, stop=True, skip_group_check=True,
                        )
                        zhm = sp.tile([128, 512], F32, tag="zhm", bufs=3)
                        nc.vector.tensor_tensor(
                            out=zhm[:, :wj], in0=agg[j][:, :wj],
                            in1=pnm[:, :wj], op=OP.mult,
                        )
                        nc.vector.tensor_reduce(
                            out=hsum[:, j : j + 1], in_=zhm[:, :wj], op=OP.add,
                            axis=AX.X,
                        )
                        sqh = sp.tile([128, 512], F32, tag="zhm", bufs=3)
                        nc.scalar.activation(
                            out=sqh[:, :wj], in_=zhm[:, :wj], func=AF.Square
                        )
                        nc.vector.tensor_reduce(
                            out=hssq[:, j : j + 1], in_=sqh[:, :wj], op=OP.add,
                            axis=AX.X,
                        )

                    # ---- pack + allreduce stats
                    ncols = 4 if l == 0 else 2
                    pack = stp.tile([128, 4], F32, name=f"pack{l}_{ep}")
                    nc.vector.tensor_reduce(
                        out=pack[:, 0:1], in_=hsum[:], op=OP.add, axis=AX.X
                    )
                    nc.vector.tensor_reduce(
                        out=pack[:, 1:2], in_=hssq[:], op=OP.add, axis=AX.X
                    )
                    if l == 0:
                        nc.vector.tensor_reduce(
                            out=pack[:, 2:3], in_=ssum_e[:], op=OP.add, axis=AX.X
                        )
                        nc.vector.tensor_reduce(
                            out=pack[:, 3:4], in_=ssq_e[:], op=OP.add, axis=AX.X
                        )
                    nc.sync.dma_start(out=cc_st_in[l][:], in_=pack[:, :ncols])
                    nc.gpsimd.collective_compute(
                        "AllGather", OP.bypass, replica_groups=rg,
                        ins=[cc_st_in[l][:]], outs=[cc_st_out[l][:]],
                    )
                    gat = stp.tile([128, 4 * NC], F32, name=f"gat{l}_{ep}")
                    nc.sync.dma_start(
                        out=gat[:, : ncols * NC].rearrange("p (f c) -> p f c", c=NC),
                        in_=cc_st_out[l][:].rearrange("(c p) f -> p f c", p=128),
                    )
                    stt = stp.tile([128, 4], F32, name=f"stt{l}_{ep}")
                    nc.vector.tensor_reduce(
                        out=stt[:, :ncols].rearrange("p (f x) -> p f x", x=1),
                        in_=gat[:, : ncols * NC].rearrange("p (f c) -> p f c", c=NC),
                        op=OP.add, axis=AX.X,
                    )

                    # ---- bn coefficients
                    def bn_cols(sum_c, ssq_c, count, pref):
                        mean = stp.tile([128, 1], F32, name=f"{pref}mean{l}_{ep}")
                        nc.vector.tensor_scalar(
                            out=mean[:], in0=sum_c, scalar1=1.0 / count,
                            scalar2=None, op0=OP.mult,
                        )
                        msq = stp.tile([128, 1], F32, name=f"{pref}msq{l}_{ep}")
                        nc.vector.tensor_scalar(
                            out=msq[:], in0=ssq_c, scalar1=1.0 / count,
                            scalar2=None, op0=OP.mult,
                        )
                        m2 = stp.tile([128, 1], F32, name=f"{pref}m2{l}_{ep}")
                        nc.scalar.activation(out=m2[:], in_=mean[:], func=AF.Square)
                        var = stp.tile([128, 1], F32, name=f"{pref}var{l}_{ep}")
                        nc.vector.tensor_tensor(
                            out=var[:], in0=msq[:], in1=m2[:], op=OP.subtract
                        )
                        sd = stp.tile([128, 1], F32, name=f"{pref}sd{l}_{ep}")
                        nc.scalar.activation(
                            out=sd[:], in_=var[:], func=AF.Sqrt, bias=eps_col[:]
                        )
                        rs = stp.tile([128, 1], F32, name=f"{pref}rs{l}_{ep}")
                        nc.vector.reciprocal(out=rs[:], in_=sd[:])
                        bb = stp.tile([128, 1], F32, name=f"{pref}bb{l}_{ep}")
                        nc.vector.tensor_tensor(
                            out=bb[:], in0=mean[:], in1=rs[:], op=OP.mult
                        )
                        nc.vector.tensor_scalar(
                            out=bb[:], in0=bb[:], scalar1=-1.0, scalar2=None,
                            op0=OP.mult,
                        )
                        return rs, bb

                    rs_h, bb_h = bn_cols(stt[:, 0:1], stt[:, 1:2], N, "h")
                    if l == 0:
                        rs_e, bb_e = bn_cols(stt[:, 2:3], stt[:, 3:4], E, "e")

                    # ---- h update (wide)
                    for j in range(NW):
                        wj = min(512, (T - 4 * j) * 128)
                        r = sp.tile([128, 512], F32, tag="rh", bufs=2)
                        nc.scalar.activation(
                            out=r[:, :wj], in_=agg[j][:, :wj], func=AF.Relu,
                            bias=bb_h[:], scale=rs_h[:],
                        )
                        hfl = sp.tile([128, 512], F32, tag="hflw", bufs=2, name="hflu")
                        nc.sync.dma_start(
                            out=hfl[:, :wj],
                            in_=hfm_buf[l][:, j * 512 : j * 512 + wj],
                        )
                        hf2 = sp.tile([128, 512], F32, tag="hneww", bufs=2, name="hupd")
                        nc.vector.tensor_tensor(
                            out=hf2[:, :wj], in0=hfl[:, :wj], in1=r[:, :wj],
                            op=OP.add,
                        )
                        nc.sync.dma_start(
                            out=hfm_buf[l + 1][:, j * 512 : j * 512 + wj],
                            in_=hf2[:, :wj],
                        )

                    boundary(l + 1)

                    # ---- e update + fused layer-1 message pass
                    if l == 0:
                        for gi, (c0, gsz) in enumerate(groups):
                            if True:
                                w = gsz * 128
                                t, ci0, S = chunks[c0]
                                G = 128 // S
                                gv1 = group_gather_t(c0, gsz, 1)
                                zsb = sp.tile([128, 512], F32, tag="zsb", bufs=2)
                                nc.sync.dma_start(
                                    out=zsb[:, :w],
                                    in_=z_buf[:, c0 * 128 : c0 * 128 + w],
                                )
                                r = sp.tile([128, 512], F32, tag="re", bufs=2)
                                nc.scalar.activation(
                                    out=r[:, :w], in_=zsb[:, :w], func=AF.Relu,
                                    bias=bb_e[:], scale=rs_e[:],
                                )
                                eold = sp.tile([128, 512], F32, tag="esb", bufs=2)
                                nc.sync.dma_start(
                                    out=eold[:, :w],
                                    in_=e_buf[:, c0 * 128 : c0 * 128 + w],
                                )
                                enew = sp.tile([128, 512], F32, tag="enew", bufs=2)
                                nc.vector.tensor_tensor(
                                    out=enew[:, :w], in0=eold[:, :w], in1=r[:, :w],
                                    op=OP.add,
                                )
                                w1 = sp.tile([128, 512], F32, tag="wsb", bufs=2)
                                nc.scalar.activation(
                                    out=w1[:, :w], in_=enew[:, :w], func=AF.Sigmoid
                                )
                                msgg = sp.tile([128, 512], F32, tag="msgg", bufs=2)
                                nc.vector.tensor_tensor(
                                    out=msgg[:, :w], in0=gv1[:, :w],
                                    in1=w1[:, :w], op=OP.mult,
                                )
                                for k in range(gsz):
                                    ci = ci0 + k
                                    if ci == 0 and t % 4 == 0:
                                        agg[t // 4] = pp.tile(
                                            [128, 512], F32, tag=f"aggw{t // 4}",
                                            name=f"aggw_{t // 4}_1_{ep}",
                                        )
                                    ao = (t % 4) * 128
                                    nc.vector.tensor_reduce(
                                        out=agg[t // 4][
                                            :, ao + ci * G : ao + (ci + 1) * G
                                        ],
                                        in_=msgg[:, k * 128 : (k + 1) * 128].rearrange(
                                            "p (g s) -> p g s", s=S
                                        ),
                                        op=OP.max,
                                        axis=AX.X,
                                    )

            # ================= moy + base =================
                if lvl < 5:
                    ydummy = sp.tile([1, 4096], F32, tag="ydummy", bufs=1)
                    nc.gpsimd.memset(ydummy[:], 0.0)
                    for c0 in range(0, E_PAD, 4096):
                        w = min(4096, E_PAD - c0)
                        nc.sync.dma_start(
                            out=y_out[0:1, c0 : c0 + w], in_=ydummy[:, :w]
                        )
                else:
                    moysum = stp.tile([128, NW], F32, name=f"moysum{ep}")
                    for j in range(NW):
                        wj = min(512, (T - 4 * j) * 128)
                        nmr = sp.tile([1, 512], F32, tag="nmr", bufs=1)
                        nc.sync.dma_start(
                            out=nmr[:], in_=mpack_d[NG + j : NG + j + 1, :]
                        )
                        pnm = ps.tile([128, 512], F32, tag="pc")
                        nc.tensor.matmul(
                            out=pnm[:, :wj], lhsT=ones_row[:], rhs=nmr[:, :wj],
                            start=True, stop=True, skip_group_check=True,
                        )
                        hfl = sp.tile([128, 512], F32, tag="hflw", bufs=2, name="hflm")
                        nc.sync.dma_start(
                            out=hfl[:, :wj],
                            in_=hfm_buf[2][:, j * 512 : j * 512 + wj],
                        )
                        hm = sp.tile([128, 512], F32, tag="zhm", bufs=3)
                        nc.vector.tensor_tensor(
                            out=hm[:, :wj], in0=hfl[:, :wj], in1=pnm[:, :wj],
                            op=OP.mult,
                        )
                        nc.vector.tensor_reduce(
                            out=moysum[:, j : j + 1], in_=hm[:, :wj], op=OP.add,
                            axis=AX.X,
                        )
                    moyp = stp.tile([128, 1], F32, name=f"moyp{ep}")
                    nc.vector.tensor_reduce(
                        out=moyp[:], in_=moysum[:], op=OP.add, axis=AX.X
                    )
                    nc.sync.dma_start(out=cc_moy_in[:], in_=moyp[:])
                    nc.gpsimd.collective_compute(
                        "AllGather", OP.bypass, replica_groups=rg,
                        ins=[cc_moy_in[:]], outs=[cc_moy_out[:]],
                    )
                    gatm = stp.tile([128, NC], F32, name=f"gatm{ep}")
                    nc.sync.dma_start(
                        out=gatm[:].rearrange("p (f c) -> p f c", c=NC),
                        in_=cc_moy_out[:].rearrange("(c p) f -> p f c", p=128),
                    )
                    moyc = stp.tile([128, 1], F32, name=f"moyc{ep}")
                    nc.vector.tensor_reduce(
                        out=moyc[:].rearrange("p (f x) -> p f x", x=1),
                        in_=gatm[:].rearrange("p (f c) -> p f c", c=NC),
                        op=OP.add, axis=AX.X,
                    )
                    nc.vector.tensor_scalar(
                        out=moyc[:], in0=moyc[:], scalar1=1.0 / N, scalar2=None,
                        op0=OP.mult,
                    )
                    pbase = ps.tile([128, 128], F32, tag="pc")
                    nc.tensor.matmul(
                        out=pbase[:, 0:1], lhsT=W0a[:], rhs=moyc[:],
                        start=True, stop=True, skip_group_check=True,
                    )
                    base_col = stp.tile([128, 1], F32, name=f"base_col{ep}")
                    nc.vector.tensor_tensor(
                        out=base_col[:], in0=pbase[:, 0:1], in1=W0bc[:], op=OP.add
                    )

                    # ================= readout =================
                    for gi, (c0, gsz) in enumerate(groups):
                        if True:
                            w = gsz * 128
                            t, ci0, S = chunks[c0]
                            G = 128 // S
                            gvr = group_gather_t(c0, gsz, 2)
                            pm1 = ps.tile([128, 512], F32, tag="pa")
                            band = sp.tile(
                                [128, 512], BF16, tag="hbtb", bufs=2, name="bandb"
                            )
                            nc.sync.dma_start(
                                out=band[:G, : gsz * 128].rearrange(
                                    "p (k c) -> p k c", c=128
                                ),
                                in_=hlocal[
                                    t * 128 + ci0 * G : t * 128 + (ci0 + gsz) * G, :
                                ].rearrange("(k p) c -> p k c", p=G),
                            )
                            for k in range(gsz):
                                nc.tensor.matmul(
                                    out=pm1[:, k * 128 : (k + 1) * 128],
                                    lhsT=band[:G, k * 128 : (k + 1) * 128],
                                    rhs=kron_bb[kron_of[S]][:G, :],
                                    start=True, stop=True,
                                    skip_group_check=True,
                                )
                            zs = sp.tile([128, 512], F32, tag="zs", bufs=2)
                            nc.vector.tensor_tensor(
                                out=zs[:, :w], in0=pm1[:, :w],
                                in1=gvr[:, :w], op=OP.add,
                            )
                            t1 = sp.tile([128, 512], BF16, tag="t1", bufs=2)
                            nc.scalar.activation(
                                out=t1[:, :w], in_=zs[:, :w], func=AF.Relu,
                                bias=base_col[:],
                            )
                            pt2 = ps.tile([128, 512], F32, tag="pb")
                            nc.tensor.matmul(
                                out=pt2[:, :w], lhsT=Wk[0][:], rhs=t1[:, :w],
                                start=True, stop=True, skip_group_check=True,
                            )
                            t2 = sp.tile([128, 512], BF16, tag="t2", bufs=2)
                            nc.scalar.activation(
                                out=t2[:, :w], in_=pt2[:, :w], func=AF.Relu,
                                bias=Wkb[0][:],
                            )
                            pt3 = ps.tile([128, 512], F32, tag="pc")
                            nc.tensor.matmul(
                                out=pt3[:, :w], lhsT=Wk[1][:], rhs=t2[:, :w],
                                start=True, stop=True, skip_group_check=True,
                            )
                            t3 = sp.tile([128, 512], BF16, tag="t3", bufs=2)
                            nc.scalar.activation(
                                out=t3[:, :w], in_=pt3[:, :w], func=AF.Relu,
                                bias=Wkb[1][:],
                            )
                            py = ps.tile([1, 512], F32, tag="pdy", bufs=2, name="py")
                            nc.tensor.matmul(
                                out=py[:, :w], lhsT=Wf[:], rhs=t3[:, :w],
                                start=True, stop=True, skip_group_check=True,
                            )
                            ysb = sp.tile([1, 512], F32, tag="ysb", bufs=2)
                            nc.scalar.activation(
                                out=ysb[:, :w], in_=py[:, :w], func=AF.Sigmoid,
                                bias=wfb[:],
                            )
                            nc.sync.dma_start(
                                out=y_out[0:1, c0 * 128 : c0 * 128 + w],
                                in_=ysb[:, :w],
                            )


            for _ep in range(epochs):
                _epoch(_ep)

    nc.compile()
    return nc


# ---------------------------------------------------------------------------
# top level
# ---------------------------------------------------------------------------


def _make_kron(S):
    G = 128 // S
    k = np.zeros((128, 128), np.float32)
    for p in range(128):
        g = p % G
        k[p, g * S : (g + 1) * S] = 1.0
    return k


def _prep(inputs):
    """plan + per-core input maps + origids (host-side prep)."""
    import ml_dtypes

    BF = ml_dtypes.bfloat16
    h = np.asarray(inputs["h"], np.float32)
    e = np.asarray(inputs["e"], np.float32)
    src = np.asarray(inputs["src"]).astype(np.int64)
    dst = np.asarray(inputs["dst"]).astype(np.int64)
    N = h.shape[0]

    plan = _plan(src, dst, N)
    plan["F_N"] = h.shape[1]
    plan["F_E"] = e.shape[1]

    U = np.asarray(inputs["U"], np.float32)
    V = np.asarray(inputs["V"], np.float32)
    A = np.asarray(inputs["A"], np.float32)
    B = np.asarray(inputs["B"], np.float32)
    C = np.asarray(inputs["C"], np.float32)
    W0_w = np.asarray(inputs["W0_w"], np.float32)
    Wk_w = np.asarray(inputs["Wk_w"], np.float32)
    Wk_b = np.asarray(inputs["Wk_b"], np.float32)
    Wf_w = np.asarray(inputs["Wf_w"], np.float32)
    Wf_b = np.asarray(inputs["Wf_b"], np.float32)

    S_vals = sorted(set(plan["S_list"]))
    krons = np.stack([_make_kron(s) for s in S_vals])
    fill2 = np.zeros((2, 256), np.float32)
    fill2[1, :] = -1e30

    WMAP, RW, BMAP, RB = _pack_layout(plan)
    wpack = np.zeros((RW, 384), np.float32)
    bpack = np.zeros((RB, 256), np.float32)

    def wput(name, arr):
        arr = np.atleast_2d(np.asarray(arr, np.float32))
        r0, rows = WMAP[name]
        assert arr.shape[0] == rows, (name, arr.shape)
        wpack[r0 : r0 + rows, : arr.shape[1]] = arr

    def bput(name, arr):
        arr = np.atleast_2d(np.asarray(arr, np.float32))
        r0, rows = BMAP[name]
        assert arr.shape[0] == rows, (name, arr.shape)
        bpack[r0 : r0 + rows, : arr.shape[1]] = arr

    wput("ones_row", np.ones((1, 128), np.float32))
    wput("identb", np.eye(128, dtype=np.float32))
    wput("fill2", fill2)
    wput("emb_e_w", np.asarray(inputs["emb_e_w"], np.float32))
    wput("emb_n_w", np.asarray(inputs["emb_n_w"], np.float32))
    wput("emb_e_b", np.asarray(inputs["emb_e_b"], np.float32).reshape(1, 128))
    wput("emb_n_b", np.asarray(inputs["emb_n_b"], np.float32).reshape(1, 128))
    wput("A0", A[0])
    wput("A1", A[1])
    wput("VCB0", np.concatenate([V[0], C[0], B[0]], axis=1))
    wput("V1", V[1])
    wput("U0", U[0])
    wput("U1", U[1])
    wput("WBC", np.concatenate([W0_w[128:256], W0_w[256:384]], axis=1))
    wput("W0a", W0_w[:128])
    wput("W0b_col", np.asarray(inputs["W0_b"], np.float32).reshape(1, 128))
    wput("Wkb0", Wk_b[0].reshape(1, 128))
    wput("Wkb1", Wk_b[1].reshape(1, 128))
    wput("wfb", np.full((1, 1), float(Wf_b), np.float32))
    for i, s in enumerate(S_vals):
        wput(f"krons{i}", krons[i])
    bput("fill2b", fill2)
    for i, s in enumerate(S_vals):
        bput(f"kronsb{i}", krons[i])
    bput("Wk0", Wk_w[0])
    bput("Wk1", Wk_w[1])
    bput("Wf", Wf_w.reshape(1, 128))

    shared = dict(wpack=wpack, bpack=bpack.astype(BF))

    in_maps = []
    origids = []
    for d in range(NC):
        pc = _per_core_arrays(plan, d, h, e)
        origids.append(pc.pop("origid"))
        m = dict(pc)
        m.update(shared)
        in_maps.append(m)
    return plan, in_maps, origids


def kernel(**inputs):
    import sys

    if "/opt/trn_rl_repo" not in sys.path:
        sys.path.insert(0, "/opt/trn_rl_repo")
    from concourse.bass_utils import run_bass_kernel_spmd

    plan, in_maps, origids = _prep(inputs)
    nc = _build_program(plan)
    res = run_bass_kernel_spmd(nc, in_maps, list(range(NC)))

    E = plan["E"]
    out = np.zeros(E, np.float32)
    for d in range(NC):
        y = np.asarray(res.results[d]["y"]).reshape(-1)
        oid = origids[d]
        valid = oid >= 0
        out[oid[valid]] = y[valid]
    return out



# revision 17
# speedup vs baseline: 1.3270x; 1.3270x over previous
"""GNN message-passing (gated GCN style) on 8 Trainium2 NeuronCores.

Strategy (edge-parallel, dst-sorted shards):
- Host sorts edges by dst and splits into 8 shards snapped to node-run
  boundaries, so each device owns a contiguous node range and its complete
  incoming-edge runs. segment_max is fully local.
- Per device, nodes are sorted by in-degree and each node's run is padded to
  a per-tile power-of-2 slot count S, so segment_max becomes a fixed-window
  reduce_max over contiguous columns (feat-major).
- Per layer, each device computes per-node tables for its own node slice and
  AllGathers them (layer 0: [h@V | h@C] fp32; layer 1 / readout: bf16).
  Per-edge src-side gathers run as batched 512-row dma_gather calls
  (single_packet=False): layer 0 edge-major + PE transposes accumulated in
  PSUM, layer 1 / readout transposing gathers (feat-major, no PE transpose).
  int16 gather indices only span 32K rows, so the 8-shard table is split in
  two 4-shard halves gathered separately and summed; a zeros row absorbs the
  other half, a -1e30 guard row keeps empty-slot max semantics.
- h@B (dst side, sorted) is expanded with a constant kron-pattern matmul.
- BatchNorm statistics are masked sums reduced on-chip and combined with a
  small AllGather + on-chip reduction per layer (AllReduce is ~213us here).
  h-side loops run 4 node-tiles wide; inputs are packed into 6 tensors to
  amortize per-argument dispatch cost.
- The readout MLP runs feat-major per <=512-edge group; h@W0b / h@W0c are
  pre-folded into the final AllGather payload / local table.
"""

import numpy as np

NC = 8
D = 128
MEGA = 4  # max chunks per dma_gather call (HW limit: 512 indices)


# ---------------------------------------------------------------------------
# host-side planning
# ---------------------------------------------------------------------------


def _next_pow2(x):
    p = 1
    while p < x:
        p *= 2
    return p


def _pack_layout(plan):
    """Row layout for the packed fp32 [RW,384] and bf16 [RB,256] const
    tensors. Column-vector consts are stored as single rows and
    transposed by the load DMA."""
    S_vals = sorted(set(plan["S_list"]))
    F_N, F_E = plan["F_N"], plan["F_E"]
    w = {}
    r = 0

    def add(name, rows):
        nonlocal r
        w[name] = (r, rows)
        r += rows

    add("ones_row", 1)
    add("emb_e_b", 1)
    add("emb_n_b", 1)
    add("VCB0", 128)
    add("V1", 128)
    add("U0", 128)
    add("U1", 128)
    add("WBC", 128)
    add("W0a", 128)
    add("W0b_col", 1)
    add("Wkb0", 1)
    add("Wkb1", 1)
    add("wfb", 1)
    RW = r
    b = {}
    r = 0

    def addb(name, rows):
        nonlocal r
        b[name] = (r, rows)
        r += rows

    addb("fill2b", 2)
    addb("emb_e_w", F_E)
    addb("emb_n_w", F_N)
    addb("A0", 128)
    addb("A1", 128)
    for i in range(len(S_vals)):
        addb(f"kronsb{i}", 128)
    addb("Wk0", 128)
    addb("Wk1", 128)
    addb("Wf", 1)
    RB = r
    return w, RW, b, RB


def _plan(src, dst, N):
    E = src.shape[0]
    order = np.argsort(dst, kind="stable")
    dsts = dst[order]
    srcs = src[order]

    # shard boundaries snapped to run starts
    bounds = [0]
    for r in range(1, NC):
        t = (E * r) // NC
        b = int(np.searchsorted(dsts, dsts[t], side="left"))
        bounds.append(max(b, bounds[-1]))
    bounds.append(E)

    lo = np.zeros(NC, np.int64)
    for d in range(1, NC):
        lo[d] = int(dsts[bounds[d]]) if bounds[d] < E else N
    hi = np.empty(NC, np.int64)
    hi[:-1] = lo[1:]
    hi[-1] = N

    n_r = [int(hi[d] - lo[d]) for d in range(NC)]
    NODE_CAP = 128 * int(np.ceil((max(n_r) + 2) / 128))
    T = NODE_CAP // 128

    lo = np.asarray(lo)
    shards = []
    for d in range(NC):
        sl = slice(bounds[d], bounds[d + 1])
        dl = dsts[sl] - lo[d]
        cnt = np.bincount(dl, minlength=n_r[d]) if n_r[d] > 0 else np.zeros(0, int)
        starts = np.concatenate([[0], np.cumsum(cnt)])
        perm = np.argsort(-cnt, kind="stable") if n_r[d] > 0 else np.zeros(0, int)
        ipos = np.empty(n_r[d], np.int64)
        ipos[perm] = np.arange(n_r[d])
        shards.append(
            dict(sl=sl, dl=dl, cnt=cnt, starts=starts, perm=perm, ipos=ipos, d=d)
        )

    # shared per-tile slot counts
    S_list = []
    for t in range(T):
        mx = 1
        for sh in shards:
            p = sh["perm"][t * 128 : (t + 1) * 128]
            if len(p):
                c = sh["cnt"][p]
                if len(c):
                    mx = max(mx, int(c.max()))
        S_list.append(min(_next_pow2(mx), 128))

    E_PAD = 128 * int(np.sum(S_list))
    C_E = E_PAD // 128
    chunks = []  # (tile, ci, S)
    for t in range(T):
        for ci in range(S_list[t]):
            chunks.append((t, ci, S_list[t]))
    groups = []  # (c0, gsz) — tile-aligned: all chunks in a group share a tile
    c = 0
    for t in range(T):
        S = S_list[t]
        ci = 0
        while ci < S:
            g = min(4, S - ci)
            groups.append((c + ci, g))
            ci += g
        c += S
    # megas: runs of whole groups, <= MEGA chunks per run
    megas = []  # (cm, nch, [group idx])
    cur = None
    for gi, (c0, gsz) in enumerate(groups):
        if cur is None or cur[1] + gsz > MEGA:
            cur = [c0, 0, []]
            megas.append(cur)
        cur[1] += gsz
        cur[2].append(gi)

    return dict(
        E=E,
        N=N,
        order=order,
        srcs=srcs,
        bounds=bounds,
        lo=np.array(lo),
        hi=np.array(hi),
        n_r=n_r,
        NODE_CAP=NODE_CAP,
        T=T,
        S_list=S_list,
        E_PAD=E_PAD,
        C_E=C_E,
        chunks=chunks,
        groups=groups,
        megas=[tuple(m) for m in megas],
        shards=shards,
    )


def _per_core_arrays(plan, d, h, e):
    """Build padded per-core host arrays for shard d."""
    import ml_dtypes

    BF = ml_dtypes.bfloat16
    sh = plan["shards"][d]
    NODE_CAP, T = plan["NODE_CAP"], plan["T"]
    S_list = plan["S_list"]
    E_PAD, C_E = plan["E_PAD"], plan["C_E"]
    n_r = plan["n_r"][d]
    guard_row = d * NODE_CAP + (NODE_CAP - 1)

    e_sh = e[plan["order"]][sh["sl"]]  # [E_s, F_E]
    src_sh = plan["srcs"][sh["sl"]]
    orig_sh = np.arange(plan["E"])[plan["order"]][sh["sl"]]

    F_E = e.shape[1]
    e0_pad = np.zeros((E_PAD, F_E), np.float32)
    srcrow = np.full(E_PAD, guard_row, np.int64)
    maskf = np.zeros(E_PAD, np.float32)
    origid = np.full(E_PAD, -1, np.int64)

    base = 0
    perm = sh["perm"]
    cnt = sh["cnt"]
    starts = sh["starts"]
    rank_of = lambda g: np.clip(
        np.searchsorted(plan["lo"], g, side="right") - 1, 0, NC - 1
    )
    for t in range(T):
        S = S_list[t]
        pn = perm[t * 128 : (t + 1) * 128]
        # index matrix [128, S] of local edge positions, -1 = dummy
        im = np.full((128, S), -1, np.int64)
        for i, n in enumerate(pn):
            dg = int(cnt[n])
            k = min(dg, S)
            if k:
                im[i, :k] = np.arange(starts[n], starts[n] + k)
        flat = im.reshape(-1)
        real = flat >= 0
        fr = flat[real]
        blk = slice(base, base + 128 * S)
        e0_blk = np.zeros((128 * S, F_E), np.float32)
        e0_blk[real] = e_sh[fr]
        e0_pad[blk] = e0_blk
        sr = np.full(128 * S, guard_row, np.int64)
        g = src_sh[fr]
        r = rank_of(g)
        loc = g - plan["lo"][r]
        pp_ = np.empty(len(g), np.int64)
        for rr in np.unique(r):
            m = r == rr
            pp_[m] = plan["shards"][rr]["ipos"][loc[m]]
        sr[real] = r * NODE_CAP + pp_
        srcrow[blk] = sr
        mk = np.zeros(128 * S, np.float32)
        mk[real] = 1.0
        maskf[blk] = mk
        oi = np.full(128 * S, -1, np.int64)
        oi[real] = orig_sh[fr]
        origid[blk] = oi
        base += 128 * S

    # edge slot i (= c*128+p) -> srcrow; the slot order the e-side pipeline
    # uses IS this flat order.  dma_gather consumes indices 16-wrapped:
    # idx[q, j] = slot j*16+q, replicated over the 8 16-partition blocks.
    HALF = 4 * NODE_CAP
    memberA = srcrow < HALF
    idxA = np.where(memberA, srcrow, NODE_CAP - 2).astype(np.int16)
    idxB = np.where(~memberA, srcrow - HALF, NODE_CAP - 2).astype(np.int16)

    def wrap16(a):
        w = a.reshape(E_PAD // 16, 16).T  # [16, E_PAD/16]
        return np.ascontiguousarray(np.tile(w, (8, 1)))

    ipack = np.concatenate([wrap16(idxA), wrap16(idxB)], axis=1)

    # mpack: mask_e rows [NG, 512] then nodemask rows [NW, 512] (4 tiles/row)
    NG = len(plan["groups"])
    NW = (T + 3) // 4
    mpack = np.zeros((NG + NW, 512), np.float32)
    for gi, (c0, gsz) in enumerate(plan["groups"]):
        mpack[gi, : gsz * 128] = maskf[c0 * 128 : (c0 + gsz) * 128]
    nm = np.zeros(NW * 512, np.float32)
    nm[:n_r] = 1.0
    mpack[NG:, :] = nm.reshape(NW, 512)
    # h0T [F_N, NODE_CAP] permuted
    F_N = h.shape[1]
    h0p = np.zeros((NODE_CAP, F_N), np.float32)
    hl = h[plan["lo"][d] : plan["hi"][d]]
    h0p[: len(perm)] = hl[perm]
    h0T = np.ascontiguousarray(h0p.T)

    return dict(
        h0T=h0T.astype(BF),
        e0T=np.ascontiguousarray(e0_pad.T).astype(BF),
        ipack=ipack,
        mpack=mpack,
        origid=origid,
    )


# ---------------------------------------------------------------------------
# device program
# ---------------------------------------------------------------------------


def _build_program(plan, stop_after="full", epochs=1):
    import concourse.bass as bass
    import concourse.mybir as mybir
    import concourse.tile as tile
    from concourse import bacc

    F32 = mybir.dt.float32
    BF16 = mybir.dt.bfloat16
    I16 = mybir.dt.int16
    AF = mybir.ActivationFunctionType
    OP = mybir.AluOpType
    AX = mybir.AxisListType

    NODE_CAP, T = plan["NODE_CAP"], plan["T"]
    E_PAD, C_E = plan["E_PAD"], plan["C_E"]
    chunks, groups, megas = plan["chunks"], plan["groups"], plan["megas"]
    NG = len(groups)
    N, E = plan["N"], plan["E"]
    S_vals = sorted(set(plan["S_list"]))
    kron_of = {s: i for i, s in enumerate(S_vals)}
    F_N, F_E = plan["F_N"], plan["F_E"]
    EPS = 1e-5
    HALF = 4 * NODE_CAP
    NI16 = E_PAD // 16

    _phases = ["embed", "bound0", "epass0", "layer0", "layer1", "full"]
    lvl = _phases.index(stop_after)

    nc = bacc.Bacc(
        "TRN2", target_bir_lowering=False, debug=False, num_devices=NC
    )

    def din(name, shape, dt=F32):
        return nc.dram_tensor(name, shape, dt, kind="ExternalInput")

    # per-core inputs
    WMAP, RW, BMAP, RB = _pack_layout(plan)
    h0T = din("h0T", [F_N, NODE_CAP], BF16)
    e0T = din("e0T", [F_E, E_PAD], BF16)
    ipack_d = din("ipack", [128, 2 * NI16], I16)
    NW = (T + 3) // 4
    mpack_d = din("mpack", [NG + NW, 512])
    wpack_d = din("wpack", [RW, 384])
    bpack_d = din("bpack", [RB, 256], BF16)

    y_out = nc.dram_tensor("y", [1, E_PAD], F32, kind="ExternalOutput")

    rg = [list(range(NC))]

    with tile.TileContext(nc) as tc:
        with (
            tc.tile_pool(name="const", bufs=1) as cp,
            tc.tile_pool(name="pers", bufs=1) as pp,
            tc.tile_pool(name="st", bufs=1) as stp,
            tc.tile_pool(name="s", bufs=2) as sp,
            tc.tile_pool(name="ps", bufs=2, space="PSUM") as ps,
            tc.tile_pool(name="dram", bufs=1, space="DRAM") as dp,
        ):
            # ---- load constants from packs
            def wload(name, width, dt=F32, pack=None, pmap=None):
                pk = pack if pack is not None else wpack_d
                mp = pmap if pmap is not None else WMAP
                r0, rows = mp[name]
                t = cp.tile([rows, width], dt, name=f"{name}_sb")
                nc.sync.dma_start(out=t[:], in_=pk[r0 : r0 + rows, :width])
                return t

            def wload_col(name, dt=F32, pack=None, pmap=None):
                pk = pack if pack is not None else wpack_d
                mp = pmap if pmap is not None else WMAP
                r0, rows = mp[name]
                t = cp.tile([128, 1], dt, name=f"{name}_sb")
                nc.sync.dma_start(
                    out=t[:], in_=pk[r0 : r0 + 1, :128].rearrange("a p -> p a")
                )
                return t

            def bload(name, width, dt=BF16):
                return wload(name, width, dt, pack=bpack_d, pmap=BMAP)

            ones_row = wload("ones_row", 128)
            embe_w = bload("emb_e_w", 128)
            embn_w = bload("emb_n_w", 128)
            embe_b = wload_col("emb_e_b")
            embn_b = wload_col("emb_n_b")
            A_sb = [bload(f"A{l}", 128) for l in range(2)]
            VCB0 = wload("VCB0", 384)
            V1 = wload("V1", 128)
            U_sb = [wload(f"U{l}", 128) for l in range(2)]
            WBC = wload("WBC", 256)
            W0a = wload("W0a", 128)
            W0bc = wload_col("W0b_col")
            Wk = [bload(f"Wk{k}", 128) for k in range(2)]
            Wkb = [wload_col(f"Wkb{k}") for k in range(2)]
            Wfr0, _ = BMAP["Wf"]
            Wf = cp.tile([128, 1], BF16, name="Wf_sb")
            nc.sync.dma_start(
                out=Wf[:], in_=bpack_d[Wfr0 : Wfr0 + 1, :128].rearrange("a p -> p a")
            )
            wfb = wload("wfb", 1)
            kron_bb = [bload(f"kronsb{i}", 128) for i in range(len(S_vals))]
            ipack = cp.tile([128, 2 * NI16], I16, name="ipack_sb")
            nc.sync.dma_start(out=ipack[:], in_=ipack_d[:])
            eps_col = cp.tile([128, 1], F32, name="eps_col")
            nc.gpsimd.memset(eps_col[:], EPS)

            # ---- dram buffers
            e_buf = dp.tile([128, E_PAD], BF16, name="e_buf")
            z_buf = dp.tile([128, E_PAD], BF16, name="z_buf")
            hb_buf = dp.tile([NODE_CAP, 128], BF16, name="hb_buf")
            hfm_buf = [
                dp.tile([128, NODE_CAP], F32, name=f"hfm_buf{i}")
                for i in range(3)
            ]
            hlocal = dp.tile([NODE_CAP, 128], BF16, name="hlocal")
            cc_hin = [
                dp.tile(
                    [NODE_CAP, 256 if l == 0 else 128],
                    BF16,
                    name=f"cc_hin{l}",
                )
                for l in range(3)
            ]
            cc_hout_ep = [
                [
                    dp.tile(
                        [NC * NODE_CAP, 256 if l == 0 else 128],
                        BF16,
                        name=f"cc_hout{l}_e{e_}",
                        addr_space="Shared",
                    )
                    for l in range(3)
                ]
                for e_ in range(epochs)
            ]
            cc_st_in = [
                dp.tile([128, 4 if l == 0 else 2], F32, name=f"cc_st_in{l}")
                for l in range(2)
            ]
            cc_st_out_ep = [
                [
                    dp.tile(
                        [NC * 128, 4 if l == 0 else 2],
                        F32,
                        name=f"cc_st_out{l}_e{e_}",
                        addr_space="Shared",
                    )
                    for l in range(2)
                ]
                for e_ in range(epochs)
            ]
            cc_moy_in = dp.tile([128, 1], F32, name="cc_moy_in")
            cc_moy_out_ep = [
                dp.tile(
                    [NC * 128, 1], F32, name=f"cc_moy_out_e{e_}",
                    addr_space="Shared",
                )
                for e_ in range(epochs)
            ]

            def _epoch(ep):
                cc_hout = cc_hout_ep[ep]
                cc_st_out = cc_st_out_ep[ep]
                cc_moy_out = cc_moy_out_ep[ep]
                # ---- persistent sbuf tiles
                hU = [None] * NW
                agg = [None] * NW

                def group_gather_t(c0, gsz, l):
                    """Dual transposing gathers (bf16 feat-major) + merge:
                    gv[p, k*128 + q] = sum_half table[idx[(c0+k)*128+q], p]."""
                    n = gsz * 128
                    ga = sp.tile([128, 512], BF16, tag="gva", bufs=2)
                    gb = sp.tile([128, 512], BF16, tag="gvb", bufs=2)
                    gv = sp.tile([128, 512], BF16, tag="gvm", bufs=2)
                    for g, ioff, r0 in ((ga, 0, 0), (gb, NI16, HALF)):
                        nc.gpsimd.dma_gather(
                            g[:, :n].rearrange("p (j i) -> p j i", i=n),
                            cc_hout[l][r0 : r0 + HALF, :],
                            ipack[:, ioff + c0 * 8 : ioff + (c0 + gsz) * 8],
                            n,
                            n,
                            128,
                            transpose=True,
                            single_packet=False,
                        )
                    nc.vector.tensor_tensor(
                        out=gv[:, :n], in0=ga[:, :n], in1=gb[:, :n], op=OP.add
                    )
                    return gv

                def group_gather_t2(c0, gsz):
                    """Dual transposing gathers (bf16 feat-major, 256-wide
                    table) + merge: gv[p, 0*n+i] = V-part feat p of slot i,
                    gv[p, 1*n+i] = C-part feat p of slot i."""
                    n = gsz * 128
                    ga = sp.tile([128, 1024], BF16, tag="gta", bufs=2)
                    gb = sp.tile([128, 1024], BF16, tag="gtb", bufs=2)
                    gv = sp.tile([128, 1024], BF16, tag="gtm", bufs=2)
                    for g, ioff, r0 in ((ga, 0, 0), (gb, NI16, HALF)):
                        nc.gpsimd.dma_gather(
                            g[:, : 2 * n].rearrange("p (j i) -> p j i", i=n),
                            cc_hout[0][r0 : r0 + HALF, :],
                            ipack[:, ioff + c0 * 8 : ioff + (c0 + gsz) * 8],
                            n,
                            n,
                            256,
                            transpose=True,
                            single_packet=False,
                        )
                    nc.vector.tensor_tensor(
                        out=gv[:, : 2 * n], in0=ga[:, : 2 * n],
                        in1=gb[:, : 2 * n], op=OP.add,
                    )
                    return gv

                # ================= embed e =================
                for gi, (c0, gsz) in enumerate(groups):
                    w = gsz * 128
                    e0sl = sp.tile([F_E, 512], BF16, tag="e0sl")
                    nc.sync.dma_start(
                        out=e0sl[:, :w], in_=e0T[:, c0 * 128 : c0 * 128 + w]
                    )
                    pe = ps.tile([128, 512], F32, tag="pa")
                    nc.tensor.matmul(
                        out=pe[:, :w], lhsT=embe_w[:], rhs=e0sl[:, :w],
                        start=True, stop=True,
                    )
                    esb = sp.tile([128, 512], BF16, tag="esb", bufs=2)
                    nc.scalar.activation(
                        out=esb[:, :w], in_=pe[:, :w], func=AF.Identity,
                        bias=embe_b[:],
                    )
                    nc.sync.dma_start(
                        out=e_buf[:, c0 * 128 : c0 * 128 + w], in_=esb[:, :w]
                    )

                # ================= embed h =================
                for t in range(T):
                    h0sl = sp.tile([F_N, 128], BF16, tag="h0sl", bufs=2, name="h0sl")
                    nc.sync.dma_start(
                        out=h0sl[:], in_=h0T[:, t * 128 : (t + 1) * 128]
                    )
                    ph = ps.tile([128, 128], F32, tag="pc")
                    nc.tensor.matmul(
                        out=ph[:], lhsT=embn_w[:], rhs=h0sl[:],
                        start=True, stop=True,
                    )
                    hf = sp.tile([128, 128], F32, tag="hnew", bufs=4, name="hemb")
                    nc.scalar.activation(
                        out=hf[:], in_=ph[:], func=AF.Identity, bias=embn_b[:]
                    )
                    nc.sync.dma_start(
                        out=hfm_buf[0][:, t * 128 : (t + 1) * 128], in_=hf[:]
                    )

                # ================= boundary =================
                def boundary(l):
                    """Build tables for layer l (or readout if l==2) from hfm."""
                    if l == 0:
                        rhs, wdt, U = VCB0, 384, U_sb[0]
                    elif l == 1:
                        rhs, wdt, U = V1, 128, U_sb[1]
                    else:
                        rhs, wdt, U = WBC, 256, None
                    scat_w = 256 if l == 0 else 128
                    bdt = BF16
                    for t in range(T):
                        hfl = sp.tile([128, 128], F32, tag="hfl", bufs=4, name="hfl")
                        nc.sync.dma_start(
                            out=hfl[:], in_=hfm_buf[l][:, t * 128 : (t + 1) * 128]
                        )
                        pb = ps.tile([128, 512], F32, tag="pa")
                        nc.tensor.matmul(
                            out=pb[:, :wdt], lhsT=hfl[:], rhs=rhs[:],
                            start=True, stop=True,
                        )
                        bsb = sp.tile([128, 512], bdt, tag="bsbb", bufs=2)
                        nc.scalar.activation(
                            out=bsb[:, :wdt], in_=pb[:, :wdt], func=AF.Copy
                        )
                        nc.sync.dma_start(
                            out=cc_hin[l][t * 128 : (t + 1) * 128, :scat_w],
                            in_=bsb[:, :scat_w],
                        )
                        if l == 0:
                            nc.sync.dma_start(
                                out=hb_buf[t * 128 : (t + 1) * 128, :],
                                in_=bsb[:, 256:384],
                            )
                        if l == 2:
                            nc.sync.dma_start(
                                out=hlocal[t * 128 : (t + 1) * 128, :],
                                in_=bsb[:, 128:256],
                            )
                        if U is not None:
                            pu = ps.tile([128, 128], F32, tag="pc")
                            nc.tensor.matmul(
                                out=pu[:], lhsT=U[:], rhs=hfl[:],
                                start=True, stop=True,
                            )
                            if t % 4 == 0:
                                hU[t // 4] = pp.tile(
                                    [128, 512], F32, tag=f"hUw{t // 4}",
                                    name=f"hUw_{t // 4}_{l}_{ep}",
                                )
                            nc.scalar.activation(
                                out=hU[t // 4][
                                    :, (t % 4) * 128 : (t % 4) * 128 + 128
                                ],
                                in_=pu[:], func=AF.Copy,
                            )
                    # zero row (NODE_CAP-2) + guard row (NODE_CAP-1)
                    fr0, _ = BMAP["fill2b"]
                    nc.sync.dma_start(
                        out=cc_hin[l][NODE_CAP - 2 : NODE_CAP, :scat_w],
                        in_=bpack_d[fr0 : fr0 + 2, :scat_w],
                    )
                    nc.gpsimd.collective_compute(
                        "AllGather",
                        OP.bypass,
                        replica_groups=rg,
                        ins=[cc_hin[l][:]],
                        outs=[cc_hout[l][:]],
                    )

                if lvl >= 1:
                    boundary(0)

                # ================= layers =================
                for l in range(2):
                    if l == 0 and lvl < 2:
                        break
                    if l == 1 and lvl < 4:
                        break
                    # stats accumulators
                    if l == 0:
                        ssum_e = stp.tile([128, NG], F32, name=f"ssum_e{ep}")
                        ssq_e = stp.tile([128, NG], F32, name=f"ssq_e{ep}")
                    hsum = stp.tile([128, NW], F32, name=f"hsum{l}_{ep}")
                    hssq = stp.tile([128, NW], F32, name=f"hssq{l}_{ep}")

                    # ---- e-pass (layer 1's is fused into the l==0 e-update)
                    for gi, (c0, gsz) in enumerate(groups if l == 0 else []):
                        if True:
                            w = gsz * 128
                            t, ci0, S = chunks[c0]
                            G = 128 // S
                            gv = group_gather_t2(c0, gsz)
                            esb = sp.tile([128, 512], BF16, tag="esb", bufs=2)
                            nc.sync.dma_start(
                                out=esb[:, :w],
                                in_=e_buf[:, c0 * 128 : c0 * 128 + w],
                            )
                            wsb = sp.tile([128, 512], F32, tag="wsb", bufs=2)
                            nc.scalar.activation(
                                out=wsb[:, :w], in_=esb[:, :w], func=AF.Sigmoid
                            )
                            mrow = sp.tile([1, 512], F32, tag="mrow", bufs=1)
                            nc.sync.dma_start(out=mrow[:], in_=mpack_d[gi : gi + 1, :])
                            pm = ps.tile([128, 512], F32, tag="pb")
                            nc.tensor.matmul(
                                out=pm[:, :w], lhsT=ones_row[:], rhs=mrow[:, :w],
                                start=True, stop=True,
                            )
                            pz = ps.tile([128, 512], F32, tag="pa")
                            nc.tensor.matmul(
                                out=pz[:, :w], lhsT=A_sb[l][:], rhs=esb[:, :w],
                                start=True, stop=False, skip_group_check=True,
                            )
                            band = sp.tile([128, 512], BF16, tag="hbt", bufs=2, name="band")
                            nc.sync.dma_start(
                                out=band[:G, : gsz * 128].rearrange(
                                    "p (k c) -> p k c", c=128
                                ),
                                in_=hb_buf[
                                    t * 128 + ci0 * G : t * 128 + (ci0 + gsz) * G, :
                                ].rearrange("(k p) c -> p k c", p=G),
                            )
                            for k in range(gsz):
                                nc.tensor.matmul(
                                    out=pz[:, k * 128 : (k + 1) * 128],
                                    lhsT=band[:G, k * 128 : (k + 1) * 128],
                                    rhs=kron_bb[kron_of[S]][:G, :],
                                    start=False, stop=(k == gsz - 1),
                                    skip_group_check=True,
                                )
                            # message path: msg = (hV0)[src] * w, windowed max
                            msg = sp.tile([128, 512], F32, tag="msg", bufs=2)
                            nc.vector.tensor_tensor(
                                out=msg[:, :w], in0=gv[:, :w],
                                in1=wsb[:, :w], op=OP.mult,
                            )
                            for k in range(gsz):
                                ci = ci0 + k
                                if ci == 0 and t % 4 == 0:
                                    agg[t // 4] = pp.tile(
                                        [128, 512], F32, tag=f"aggw{t // 4}",
                                        name=f"aggw_{t // 4}_0_{ep}",
                                    )
                                ao = (t % 4) * 128
                                nc.vector.tensor_reduce(
                                    out=agg[t // 4][
                                        :, ao + ci * G : ao + (ci + 1) * G
                                    ],
                                    in_=msg[:, k * 128 : (k + 1) * 128].rearrange(
                                        "p (g s) -> p g s", s=S
                                    ),
                                    op=OP.max,
                                    axis=AX.X,
                                )
                            # z path: z = A e + (C h)[src] + kron(B h local)
                            zsum = sp.tile([128, 512], F32, tag="zsum", bufs=2)
                            nc.vector.tensor_tensor(
                                out=zsum[:, :w], in0=pz[:, :w],
                                in1=gv[:, w : 2 * w], op=OP.add,
                            )
                            zm = sp.tile([128, 512], BF16, tag="zm", bufs=2)
                            nc.vector.tensor_tensor(
                                out=zm[:, :w], in0=zsum[:, :w], in1=pm[:, :w],
                                op=OP.mult,
                            )
                            nc.vector.tensor_reduce(
                                out=ssum_e[:, gi : gi + 1], in_=zm[:, :w],
                                op=OP.add, axis=AX.X,
                            )
                            sq = sp.tile([128, 512], F32, tag="sq", bufs=2)
                            nc.scalar.activation(
                                out=sq[:, :w], in_=zm[:, :w], func=AF.Square
                            )
                            nc.vector.tensor_reduce(
                                out=ssq_e[:, gi : gi + 1], in_=sq[:, :w],
                                op=OP.add, axis=AX.X,
                            )
                            nc.sync.dma_start(
                                out=z_buf[:, c0 * 128 : c0 * 128 + w],
                                in_=zm[:, :w],
                            )

                    if l == 0 and lvl < 3:
                        break

                    # ---- h side: z_h = hU + select(agg); masked stats (wide)
                    for j in range(NW):
                        wj = min(512, (T - 4 * j) * 128)
                        m01 = sp.tile([128, 512], F32, tag="zhm", bufs=3)
                        nc.vector.tensor_scalar(
                            out=m01[:, :wj], in0=agg[j][:, :wj], scalar1=-1e20,
                            scalar2=None, op0=OP.is_gt,
                        )
                        nc.vector.tensor_tensor(
                            out=agg[j][:, :wj], in0=agg[j][:, :wj],
                            in1=m01[:, :wj], op=OP.mult,
                        )
                        nc.vector.tensor_tensor(
                            out=agg[j][:, :wj], in0=agg[j][:, :wj],
                            in1=hU[j][:, :wj], op=OP.add,
                        )
                        nmr = sp.tile([1, 512], F32, tag="nmr", bufs=1)
                        nc.sync.dma_start(
                            out=nmr[:], in_=mpack_d[NG + j : NG + j + 1, :]
                        )
                        pnm = ps.tile([128, 512], F32, tag="pc")
                        nc.tensor.matmul(
                            out=pnm[:, :wj], lhsT=ones_row[:], rhs=nmr[:, :wj],
                            start=True, stop=True, skip_group_check=True,
                        )
                        zhm = sp.tile([128, 512], F32, tag="zhm", bufs=3)
                        nc.vector.tensor_tensor(
                            out=zhm[:, :wj], in0=agg[j][:, :wj],
                            in1=pnm[:, :wj], op=OP.mult,
                        )
                        nc.vector.tensor_reduce(
                            out=hsum[:, j : j + 1], in_=zhm[:, :wj], op=OP.add,
                            axis=AX.X,
                        )
                        sqh = sp.tile([128, 512], F32, tag="zhm", bufs=3)
                        nc.scalar.activation(
                            out=sqh[:, :wj], in_=zhm[:, :wj], func=AF.Square
                        )
                        nc.vector.tensor_reduce(
                            out=hssq[:, j : j + 1], in_=sqh[:, :wj], op=OP.add,
                            axis=AX.X,
                        )

                    # ---- pack + allreduce stats
                    ncols = 4 if l == 0 else 2
                    pack = stp.tile([128, 4], F32, name=f"pack{l}_{ep}")
                    nc.vector.tensor_reduce(
                        out=pack[:, 0:1], in_=hsum[:], op=OP.add, axis=AX.X
                    )
                    nc.vector.tensor_reduce(
                        out=pack[:, 1:2], in_=hssq[:], op=OP.add, axis=AX.X
                    )
                    if l == 0:
                        nc.vector.tensor_reduce(
                            out=pack[:, 2:3], in_=ssum_e[:], op=OP.add, axis=AX.X
                        )
                        nc.vector.tensor_reduce(
                            out=pack[:, 3:4], in_=ssq_e[:], op=OP.add, axis=AX.X
                        )
                    nc.sync.dma_start(out=cc_st_in[l][:], in_=pack[:, :ncols])
                    nc.gpsimd.collective_compute(
                        "AllGather", OP.bypass, replica_groups=rg,
                        ins=[cc_st_in[l][:]], outs=[cc_st_out[l][:]],
                    )
                    gat = stp.tile([128, 4 * NC], F32, name=f"gat{l}_{ep}")
                    nc.sync.dma_start(
                        out=gat[:, : ncols * NC].rearrange("p (f c) -> p f c", c=NC),
                        in_=cc_st_out[l][:].rearrange("(c p) f -> p f c", p=128),
                    )
                    stt = stp.tile([128, 4], F32, name=f"stt{l}_{ep}")
                    nc.vector.tensor_reduce(
                        out=stt[:, :ncols].rearrange("p (f x) -> p f x", x=1),
                        in_=gat[:, : ncols * NC].rearrange("p (f c) -> p f c", c=NC),
                        op=OP.add, axis=AX.X,
                    )

                    # ---- bn coefficients
                    def bn_cols(sum_c, ssq_c, count, pref):
                        mean = stp.tile([128, 1], F32, name=f"{pref}mean{l}_{ep}")
                        nc.vector.tensor_scalar(
                            out=mean[:], in0=sum_c, scalar1=1.0 / count,
                            scalar2=None, op0=OP.mult,
                        )
                        msq = stp.tile([128, 1], F32, name=f"{pref}msq{l}_{ep}")
                        nc.vector.tensor_scalar(
                            out=msq[:], in0=ssq_c, scalar1=1.0 / count,
                            scalar2=None, op0=OP.mult,
                        )
                        m2 = stp.tile([128, 1], F32, name=f"{pref}m2{l}_{ep}")
                        nc.scalar.activation(out=m2[:], in_=mean[:], func=AF.Square)
                        var = stp.tile([128, 1], F32, name=f"{pref}var{l}_{ep}")
                        nc.vector.tensor_tensor(
                            out=var[:], in0=msq[:], in1=m2[:], op=OP.subtract
                        )
                        sd = stp.tile([128, 1], F32, name=f"{pref}sd{l}_{ep}")
                        nc.scalar.activation(
                            out=sd[:], in_=var[:], func=AF.Sqrt, bias=eps_col[:]
                        )
                        rs = stp.tile([128, 1], F32, name=f"{pref}rs{l}_{ep}")
                        nc.vector.reciprocal(out=rs[:], in_=sd[:])
                        bb = stp.tile([128, 1], F32, name=f"{pref}bb{l}_{ep}")
                        nc.vector.tensor_tensor(
                            out=bb[:], in0=mean[:], in1=rs[:], op=OP.mult
                        )
                        nc.vector.tensor_scalar(
                            out=bb[:], in0=bb[:], scalar1=-1.0, scalar2=None,
                            op0=OP.mult,
                        )
                        return rs, bb

                    rs_h, bb_h = bn_cols(stt[:, 0:1], stt[:, 1:2], N, "h")
                    if l == 0:
                        rs_e, bb_e = bn_cols(stt[:, 2:3], stt[:, 3:4], E, "e")

                    # ---- h update (wide)
                    for j in range(NW):
                        wj = min(512, (T - 4 * j) * 128)
                        r = sp.tile([128, 512], F32, tag="rh", bufs=2)
                        nc.scalar.activation(
                            out=r[:, :wj], in_=agg[j][:, :wj], func=AF.Relu,
                            bias=bb_h[:], scale=rs_h[:],
                        )
                        hfl = sp.tile([128, 512], F32, tag="hflw", bufs=2, name="hflu")
                        nc.sync.dma_start(
                            out=hfl[:, :wj],
                            in_=hfm_buf[l][:, j * 512 : j * 512 + wj],
                        )
                        hf2 = sp.tile([128, 512], F32, tag="hneww", bufs=2, name="hupd")
                        nc.vector.tensor_tensor(
                            out=hf2[:, :wj], in0=hfl[:, :wj], in1=r[:, :wj],
                            op=OP.add,
                        )
                        nc.sync.dma_start(
                            out=hfm_buf[l + 1][:, j * 512 : j * 512 + wj],
                            in_=hf2[:, :wj],
                        )

                    boundary(l + 1)

                    # ---- e update + fused layer-1 message pass
                    if l == 0:
                        for gi, (c0, gsz) in enumerate(groups):
                            if True:
                                w = gsz * 128
                                t, ci0, S = chunks[c0]
                                G = 128 // S
                                gv1 = group_gather_t(c0, gsz, 1)
                                zsb = sp.tile([128, 512], BF16, tag="zsb", bufs=2)
                                nc.sync.dma_start(
                                    out=zsb[:, :w],
                                    in_=z_buf[:, c0 * 128 : c0 * 128 + w],
                                )
                                r = sp.tile([128, 512], F32, tag="re", bufs=2)
                                nc.scalar.activation(
                                    out=r[:, :w], in_=zsb[:, :w], func=AF.Relu,
                                    bias=bb_e[:], scale=rs_e[:],
                                )
                                eold = sp.tile([128, 512], BF16, tag="esb", bufs=2)
                                nc.sync.dma_start(
                                    out=eold[:, :w],
                                    in_=e_buf[:, c0 * 128 : c0 * 128 + w],
                                )
                                enew = sp.tile([128, 512], F32, tag="enew", bufs=2)
                                nc.vector.tensor_tensor(
                                    out=enew[:, :w], in0=eold[:, :w], in1=r[:, :w],
                                    op=OP.add,
                                )
                                w1 = sp.tile([128, 512], F32, tag="wsb", bufs=2)
                                nc.scalar.activation(
                                    out=w1[:, :w], in_=enew[:, :w], func=AF.Sigmoid
                                )
                                msgg = sp.tile([128, 512], F32, tag="msgg", bufs=2)
                                nc.vector.tensor_tensor(
                                    out=msgg[:, :w], in0=gv1[:, :w],
                                    in1=w1[:, :w], op=OP.mult,
                                )
                                for k in range(gsz):
                                    ci = ci0 + k
                                    if ci == 0 and t % 4 == 0:
                                        agg[t // 4] = pp.tile(
                                            [128, 512], F32, tag=f"aggw{t // 4}",
                                            name=f"aggw_{t // 4}_1_{ep}",
                                        )
                                    ao = (t % 4) * 128
                                    nc.vector.tensor_reduce(
                                        out=agg[t // 4][
                                            :, ao + ci * G : ao + (ci + 1) * G
                                        ],
                                        in_=msgg[:, k * 128 : (k + 1) * 128].rearrange(
                                            "p (g s) -> p g s", s=S
                                        ),
                                        op=OP.max,
                                        axis=AX.X,
                                    )

            # ================= moy + base =================
                if lvl < 5:
                    ydummy = sp.tile([1, 4096], F32, tag="ydummy", bufs=1)
                    nc.gpsimd.memset(ydummy[:], 0.0)
                    for c0 in range(0, E_PAD, 4096):
                        w = min(4096, E_PAD - c0)
                        nc.sync.dma_start(
                            out=y_out[0:1, c0 : c0 + w], in_=ydummy[:, :w]
                        )
                else:
                    moysum = stp.tile([128, NW], F32, name=f"moysum{ep}")
                    for j in range(NW):
                        wj = min(512, (T - 4 * j) * 128)
                        nmr = sp.tile([1, 512], F32, tag="nmr", bufs=1)
                        nc.sync.dma_start(
                            out=nmr[:], in_=mpack_d[NG + j : NG + j + 1, :]
                        )
                        pnm = ps.tile([128, 512], F32, tag="pc")
                        nc.tensor.matmul(
                            out=pnm[:, :wj], lhsT=ones_row[:], rhs=nmr[:, :wj],
                            start=True, stop=True, skip_group_check=True,
                        )
                        hfl = sp.tile([128, 512], F32, tag="hflw", bufs=2, name="hflm")
                        nc.sync.dma_start(
                            out=hfl[:, :wj],
                            in_=hfm_buf[2][:, j * 512 : j * 512 + wj],
                        )
                        hm = sp.tile([128, 512], F32, tag="zhm", bufs=3)
                        nc.vector.tensor_tensor(
                            out=hm[:, :wj], in0=hfl[:, :wj], in1=pnm[:, :wj],
                            op=OP.mult,
                        )
                        nc.vector.tensor_reduce(
                            out=moysum[:, j : j + 1], in_=hm[:, :wj], op=OP.add,
                            axis=AX.X,
                        )
                    moyp = stp.tile([128, 1], F32, name=f"moyp{ep}")
                    nc.vector.tensor_reduce(
                        out=moyp[:], in_=moysum[:], op=OP.add, axis=AX.X
                    )
                    nc.sync.dma_start(out=cc_moy_in[:], in_=moyp[:])
                    nc.gpsimd.collective_compute(
                        "AllGather", OP.bypass, replica_groups=rg,
                        ins=[cc_moy_in[:]], outs=[cc_moy_out[:]],
                    )
                    gatm = stp.tile([128, NC], F32, name=f"gatm{ep}")
                    nc.sync.dma_start(
                        out=gatm[:].rearrange("p (f c) -> p f c", c=NC),
                        in_=cc_moy_out[:].rearrange("(c p) f -> p f c", p=128),
                    )
                    moyc = stp.tile([128, 1], F32, name=f"moyc{ep}")
                    nc.vector.tensor_reduce(
                        out=moyc[:].rearrange("p (f x) -> p f x", x=1),
                        in_=gatm[:].rearrange("p (f c) -> p f c", c=NC),
                        op=OP.add, axis=AX.X,
                    )
                    nc.vector.tensor_scalar(
                        out=moyc[:], in0=moyc[:], scalar1=1.0 / N, scalar2=None,
                        op0=OP.mult,
                    )
                    pbase = ps.tile([128, 128], F32, tag="pc")
                    nc.tensor.matmul(
                        out=pbase[:, 0:1], lhsT=W0a[:], rhs=moyc[:],
                        start=True, stop=True, skip_group_check=True,
                    )
                    base_col = stp.tile([128, 1], F32, name=f"base_col{ep}")
                    nc.vector.tensor_tensor(
                        out=base_col[:], in0=pbase[:, 0:1], in1=W0bc[:], op=OP.add
                    )

                    # ================= readout =================
                    for gi, (c0, gsz) in enumerate(groups):
                        if True:
                            w = gsz * 128
                            t, ci0, S = chunks[c0]
                            G = 128 // S
                            gvr = group_gather_t(c0, gsz, 2)
                            pm1 = ps.tile([128, 512], F32, tag="pa")
                            band = sp.tile(
                                [128, 512], BF16, tag="hbtb", bufs=2, name="bandb"
                            )
                            nc.sync.dma_start(
                                out=band[:G, : gsz * 128].rearrange(
                                    "p (k c) -> p k c", c=128
                                ),
                                in_=hlocal[
                                    t * 128 + ci0 * G : t * 128 + (ci0 + gsz) * G, :
                                ].rearrange("(k p) c -> p k c", p=G),
                            )
                            for k in range(gsz):
                                nc.tensor.matmul(
                                    out=pm1[:, k * 128 : (k + 1) * 128],
                                    lhsT=band[:G, k * 128 : (k + 1) * 128],
                                    rhs=kron_bb[kron_of[S]][:G, :],
                                    start=True, stop=True,
                                    skip_group_check=True,
                                )
                            zs = sp.tile([128, 512], F32, tag="zs", bufs=2)
                            nc.vector.tensor_tensor(
                                out=zs[:, :w], in0=pm1[:, :w],
                                in1=gvr[:, :w], op=OP.add,
                            )
                            t1 = sp.tile([128, 512], BF16, tag="t1", bufs=2)
                            nc.scalar.activation(
                                out=t1[:, :w], in_=zs[:, :w], func=AF.Relu,
                                bias=base_col[:],
                            )
                            pt2 = ps.tile([128, 512], F32, tag="pb")
                            nc.tensor.matmul(
                                out=pt2[:, :w], lhsT=Wk[0][:], rhs=t1[:, :w],
                                start=True, stop=True, skip_group_check=True,
                            )
                            t2 = sp.tile([128, 512], BF16, tag="t2", bufs=2)
                            nc.scalar.activation(
                                out=t2[:, :w], in_=pt2[:, :w], func=AF.Relu,
                                bias=Wkb[0][:],
                            )
                            pt3 = ps.tile([128, 512], F32, tag="pc")
                            nc.tensor.matmul(
                                out=pt3[:, :w], lhsT=Wk[1][:], rhs=t2[:, :w],
                                start=True, stop=True, skip_group_check=True,
                            )
                            t3 = sp.tile([128, 512], BF16, tag="t3", bufs=2)
                            nc.scalar.activation(
                                out=t3[:, :w], in_=pt3[:, :w], func=AF.Relu,
                                bias=Wkb[1][:],
                            )
                            py = ps.tile([1, 512], F32, tag="pdy", bufs=2, name="py")
                            nc.tensor.matmul(
                                out=py[:, :w], lhsT=Wf[:], rhs=t3[:, :w],
                                start=True, stop=True, skip_group_check=True,
                            )
                            ysb = sp.tile([1, 512], F32, tag="ysb", bufs=2)
                            nc.scalar.activation(
                                out=ysb[:, :w], in_=py[:, :w], func=AF.Sigmoid,
                                bias=wfb[:],
                            )
                            nc.sync.dma_start(
                                out=y_out[0:1, c0 * 128 : c0 * 128 + w],
                                in_=ysb[:, :w],
                            )


            for _ep in range(epochs):
                _epoch(_ep)

    nc.compile()
    return nc


# ---------------------------------------------------------------------------
# top level
# ---------------------------------------------------------------------------


def _make_kron(S):
    G = 128 // S
    k = np.zeros((128, 128), np.float32)
    for p in range(128):
        g = p % G
        k[p, g * S : (g + 1) * S] = 1.0
    return k


def _prep(inputs):
    """plan + per-core input maps + origids (host-side prep)."""
    import ml_dtypes

    BF = ml_dtypes.bfloat16
    h = np.asarray(inputs["h"], np.float32)
    e = np.asarray(inputs["e"], np.float32)
    src = np.asarray(inputs["src"]).astype(np.int64)
    dst = np.asarray(inputs["dst"]).astype(np.int64)
    N = h.shape[0]

    plan = _plan(src, dst, N)
    plan["F_N"] = h.shape[1]
    plan["F_E"] = e.shape[1]

    U = np.asarray(inputs["U"], np.float32)
    V = np.asarray(inputs["V"], np.float32)
    A = np.asarray(inputs["A"], np.float32)
    B = np.asarray(inputs["B"], np.float32)
    C = np.asarray(inputs["C"], np.float32)
    W0_w = np.asarray(inputs["W0_w"], np.float32)
    Wk_w = np.asarray(inputs["Wk_w"], np.float32)
    Wk_b = np.asarray(inputs["Wk_b"], np.float32)
    Wf_w = np.asarray(inputs["Wf_w"], np.float32)
    Wf_b = np.asarray(inputs["Wf_b"], np.float32)

    S_vals = sorted(set(plan["S_list"]))
    krons = np.stack([_make_kron(s) for s in S_vals])
    fill2 = np.zeros((2, 256), np.float32)
    fill2[1, :] = -1e30

    WMAP, RW, BMAP, RB = _pack_layout(plan)
    wpack = np.zeros((RW, 384), np.float32)
    bpack = np.zeros((RB, 256), np.float32)

    def wput(name, arr):
        arr = np.atleast_2d(np.asarray(arr, np.float32))
        r0, rows = WMAP[name]
        assert arr.shape[0] == rows, (name, arr.shape)
        wpack[r0 : r0 + rows, : arr.shape[1]] = arr

    def bput(name, arr):
        arr = np.atleast_2d(np.asarray(arr, np.float32))
        r0, rows = BMAP[name]
        assert arr.shape[0] == rows, (name, arr.shape)
        bpack[r0 : r0 + rows, : arr.shape[1]] = arr

    wput("ones_row", np.ones((1, 128), np.float32))
    wput("emb_e_b", np.asarray(inputs["emb_e_b"], np.float32).reshape(1, 128))
    wput("emb_n_b", np.asarray(inputs["emb_n_b"], np.float32).reshape(1, 128))
    bput("emb_e_w", np.asarray(inputs["emb_e_w"], np.float32))
    bput("emb_n_w", np.asarray(inputs["emb_n_w"], np.float32))
    bput("A0", A[0])
    bput("A1", A[1])
    wput("VCB0", np.concatenate([V[0], C[0], B[0]], axis=1))
    wput("V1", V[1])
    wput("U0", U[0])
    wput("U1", U[1])
    wput("WBC", np.concatenate([W0_w[128:256], W0_w[256:384]], axis=1))
    wput("W0a", W0_w[:128])
    wput("W0b_col", np.asarray(inputs["W0_b"], np.float32).reshape(1, 128))
    wput("Wkb0", Wk_b[0].reshape(1, 128))
    wput("Wkb1", Wk_b[1].reshape(1, 128))
    wput("wfb", np.full((1, 1), float(Wf_b), np.float32))
    bput("fill2b", fill2)
    for i, s in enumerate(S_vals):
        bput(f"kronsb{i}", krons[i])
    bput("Wk0", Wk_w[0])
    bput("Wk1", Wk_w[1])
    bput("Wf", Wf_w.reshape(1, 128))

    shared = dict(wpack=wpack, bpack=bpack.astype(BF))

    in_maps = []
    origids = []
    for d in range(NC):
        pc = _per_core_arrays(plan, d, h, e)
        origids.append(pc.pop("origid"))
        m = dict(pc)
        m.update(shared)
        in_maps.append(m)
    return plan, in_maps, origids


def kernel(**inputs):
    import sys

    if "/opt/trn_rl_repo" not in sys.path:
        sys.path.insert(0, "/opt/trn_rl_repo")
    from concourse.bass_utils import run_bass_kernel_spmd

    plan, in_maps, origids = _prep(inputs)
    nc = _build_program(plan)
    res = run_bass_kernel_spmd(nc, in_maps, list(range(NC)))

    E = plan["E"]
    out = np.zeros(E, np.float32)
    for d in range(NC):
        y = np.asarray(res.results[d]["y"]).reshape(-1)
        oid = origids[d]
        valid = oid >= 0
        out[oid[valid]] = y[valid]
    return out



# revision 30
# speedup vs baseline: 1.4177x; 1.0684x over previous
"""GNN message-passing (gated GCN style) on 8 Trainium2 NeuronCores.

Strategy (edge-parallel, dst-sorted shards):
- Host sorts edges by dst and splits into 8 shards snapped to node-run
  boundaries, so each device owns a contiguous node range and its complete
  incoming-edge runs. segment_max is fully local.
- Per device, nodes are sorted by in-degree and each node's run is padded to
  a per-tile power-of-2 slot count S, so segment_max becomes a fixed-window
  reduce_max over contiguous columns (feat-major).
- Per layer, each device computes per-node tables for its own node slice and
  AllGathers them (layer 0: [h@V | h@C] fp32; layer 1 / readout: bf16).
  Per-edge src-side gathers run as batched 512-row dma_gather calls
  (single_packet=False): layer 0 edge-major + PE transposes accumulated in
  PSUM, layer 1 / readout transposing gathers (feat-major, no PE transpose).
  int16 gather indices only span 32K rows, so the 8-shard table is split in
  two 4-shard halves gathered separately and summed; a zeros row absorbs the
  other half, a -1e30 guard row keeps empty-slot max semantics.
- h@B (dst side, sorted) is expanded with a constant kron-pattern matmul.
- BatchNorm statistics are masked sums reduced on-chip and combined with a
  small AllGather + on-chip reduction per layer (AllReduce is ~213us here).
  h-side loops run 4 node-tiles wide; inputs are packed into 6 tensors to
  amortize per-argument dispatch cost.
- The readout MLP runs feat-major per <=512-edge group; h@W0b / h@W0c are
  pre-folded into the final AllGather payload / local table.
"""

import numpy as np

NC = 8
D = 128
MEGA = 4  # max chunks per dma_gather call (HW limit: 512 indices)

USE_STT = True  # scalar_tensor_tensor select in h-side
USE_TTR = False  # tensor_tensor_reduce fused stats — CRASHES HW, keep off
USE_TS = True  # tensor_scalar bias+relu in readout


# ---------------------------------------------------------------------------
# host-side planning
# ---------------------------------------------------------------------------


def _next_pow2(x):
    p = 1
    while p < x:
        p *= 2
    return p


def _pack_layout(plan):
    """Row layout for the packed fp32 [RW,384] and bf16 [RB,256] const
    tensors. Column-vector consts are stored as single rows and
    transposed by the load DMA."""
    S_vals = sorted(set(plan["S_list"]))
    F_N, F_E = plan["F_N"], plan["F_E"]
    w = {}
    r = 0

    def add(name, rows):
        nonlocal r
        w[name] = (r, rows)
        r += rows

    add("ones_row", 1)
    add("emb_e_b", 1)
    add("emb_n_b", 1)
    add("VCB0", 128)
    add("V1", 128)
    add("U0", 128)
    add("U1", 128)
    add("WBC", 128)
    add("W0a", 128)
    add("W0b_col", 1)
    add("Wkb0", 1)
    add("Wkb1", 1)
    add("wfb", 1)
    RW = r
    b = {}
    r = 0

    def addb(name, rows):
        nonlocal r
        b[name] = (r, rows)
        r += rows

    addb("fill2b", 2)
    addb("emb_e_w", F_E)
    addb("emb_n_w", F_N)
    addb("A0", 128)
    addb("A1", 128)
    for i in range(len(S_vals)):
        addb(f"kronsb{i}", 128)
    addb("Wk0", 128)
    addb("Wk1", 128)
    addb("Wf", 1)
    RB = r
    return w, RW, b, RB


def _plan(src, dst, N):
    E = src.shape[0]
    order = np.argsort(dst, kind="stable")
    dsts = dst[order]
    srcs = src[order]

    # shard boundaries snapped to run starts
    bounds = [0]
    for r in range(1, NC):
        t = (E * r) // NC
        b = int(np.searchsorted(dsts, dsts[t], side="left"))
        bounds.append(max(b, bounds[-1]))
    bounds.append(E)

    lo = np.zeros(NC, np.int64)
    for d in range(1, NC):
        lo[d] = int(dsts[bounds[d]]) if bounds[d] < E else N
    hi = np.empty(NC, np.int64)
    hi[:-1] = lo[1:]
    hi[-1] = N

    n_r = [int(hi[d] - lo[d]) for d in range(NC)]
    NODE_CAP = 128 * int(np.ceil((max(n_r) + 2) / 128))
    T = NODE_CAP // 128

    lo = np.asarray(lo)
    shards = []
    for d in range(NC):
        sl = slice(bounds[d], bounds[d + 1])
        dl = dsts[sl] - lo[d]
        cnt = np.bincount(dl, minlength=n_r[d]) if n_r[d] > 0 else np.zeros(0, int)
        starts = np.concatenate([[0], np.cumsum(cnt)])
        perm = np.argsort(-cnt, kind="stable") if n_r[d] > 0 else np.zeros(0, int)
        ipos = np.empty(n_r[d], np.int64)
        ipos[perm] = np.arange(n_r[d])
        shards.append(
            dict(sl=sl, dl=dl, cnt=cnt, starts=starts, perm=perm, ipos=ipos, d=d)
        )

    # shared per-tile slot counts
    S_list = []
    for t in range(T):
        mx = 1
        for sh in shards:
            p = sh["perm"][t * 128 : (t + 1) * 128]
            if len(p):
                c = sh["cnt"][p]
                if len(c):
                    mx = max(mx, int(c.max()))
        S_list.append(min(_next_pow2(mx), 128))

    E_PAD = 128 * int(np.sum(S_list))
    C_E = E_PAD // 128
    chunks = []  # (tile, ci, S)
    for t in range(T):
        for ci in range(S_list[t]):
            chunks.append((t, ci, S_list[t]))
    groups = []  # (c0, gsz) — tile-aligned: all chunks in a group share a tile
    c = 0
    for t in range(T):
        S = S_list[t]
        ci = 0
        while ci < S:
            g = min(4, S - ci)
            groups.append((c + ci, g))
            ci += g
        c += S
    # megas: runs of whole groups, <= MEGA chunks per run
    megas = []  # (cm, nch, [group idx])
    cur = None
    for gi, (c0, gsz) in enumerate(groups):
        if cur is None or cur[1] + gsz > MEGA:
            cur = [c0, 0, []]
            megas.append(cur)
        cur[1] += gsz
        cur[2].append(gi)

    return dict(
        E=E,
        N=N,
        order=order,
        srcs=srcs,
        bounds=bounds,
        lo=np.array(lo),
        hi=np.array(hi),
        n_r=n_r,
        NODE_CAP=NODE_CAP,
        T=T,
        S_list=S_list,
        E_PAD=E_PAD,
        C_E=C_E,
        chunks=chunks,
        groups=groups,
        megas=[tuple(m) for m in megas],
        shards=shards,
    )


def _per_core_arrays(plan, d, h, e):
    """Build padded per-core host arrays for shard d."""
    import ml_dtypes

    BF = ml_dtypes.bfloat16
    sh = plan["shards"][d]
    NODE_CAP, T = plan["NODE_CAP"], plan["T"]
    S_list = plan["S_list"]
    E_PAD, C_E = plan["E_PAD"], plan["C_E"]
    n_r = plan["n_r"][d]
    guard_row = d * NODE_CAP + (NODE_CAP - 1)

    e_sh = e[plan["order"]][sh["sl"]]  # [E_s, F_E]
    src_sh = plan["srcs"][sh["sl"]]
    orig_sh = np.arange(plan["E"])[plan["order"]][sh["sl"]]

    F_E = e.shape[1]
    e0_pad = np.zeros((E_PAD, F_E), np.float32)
    srcrow = np.full(E_PAD, guard_row, np.int64)
    maskf = np.zeros(E_PAD, np.float32)
    origid = np.full(E_PAD, -1, np.int64)

    base = 0
    perm = sh["perm"]
    cnt = sh["cnt"]
    starts = sh["starts"]
    rank_of = lambda g: np.clip(
        np.searchsorted(plan["lo"], g, side="right") - 1, 0, NC - 1
    )
    for t in range(T):
        S = S_list[t]
        pn = perm[t * 128 : (t + 1) * 128]
        # index matrix [128, S] of local edge positions, -1 = dummy
        im = np.full((128, S), -1, np.int64)
        for i, n in enumerate(pn):
            dg = int(cnt[n])
            k = min(dg, S)
            if k:
                im[i, :k] = np.arange(starts[n], starts[n] + k)
        flat = im.reshape(-1)
        real = flat >= 0
        fr = flat[real]
        blk = slice(base, base + 128 * S)
        e0_blk = np.zeros((128 * S, F_E), np.float32)
        e0_blk[real] = e_sh[fr]
        e0_pad[blk] = e0_blk
        sr = np.full(128 * S, guard_row, np.int64)
        g = src_sh[fr]
        r = rank_of(g)
        loc = g - plan["lo"][r]
        pp_ = np.empty(len(g), np.int64)
        for rr in np.unique(r):
            m = r == rr
            pp_[m] = plan["shards"][rr]["ipos"][loc[m]]
        sr[real] = r * NODE_CAP + pp_
        srcrow[blk] = sr
        mk = np.zeros(128 * S, np.float32)
        mk[real] = 1.0
        maskf[blk] = mk
        oi = np.full(128 * S, -1, np.int64)
        oi[real] = orig_sh[fr]
        origid[blk] = oi
        base += 128 * S

    # edge slot i (= c*128+p) -> srcrow; the slot order the e-side pipeline
    # uses IS this flat order.  dma_gather consumes indices 16-wrapped:
    # idx[q, j] = slot j*16+q, replicated over the 8 16-partition blocks.
    HALF = 4 * NODE_CAP
    memberA = srcrow < HALF
    idxA = np.where(memberA, srcrow, NODE_CAP - 2).astype(np.int16)
    idxB = np.where(~memberA, srcrow - HALF, NODE_CAP - 2).astype(np.int16)

    def wrap16(a):
        w = a.reshape(E_PAD // 16, 16).T  # [16, E_PAD/16]
        return np.ascontiguousarray(np.tile(w, (8, 1)))

    ipack = np.concatenate([wrap16(idxA), wrap16(idxB)], axis=1)

    # mpack: mask_e rows [NG, 512] then nodemask rows [NW, 512] (4 tiles/row)
    NG = len(plan["groups"])
    NW = (T + 3) // 4
    mpack = np.zeros((NG + NW, 512), np.float32)
    for gi, (c0, gsz) in enumerate(plan["groups"]):
        mpack[gi, : gsz * 128] = maskf[c0 * 128 : (c0 + gsz) * 128]
    nm = np.zeros(NW * 512, np.float32)
    nm[:n_r] = 1.0
    mpack[NG:, :] = nm.reshape(NW, 512)
    # h0T [F_N, NODE_CAP] permuted
    F_N = h.shape[1]
    h0p = np.zeros((NODE_CAP, F_N), np.float32)
    hl = h[plan["lo"][d] : plan["hi"][d]]
    h0p[: len(perm)] = hl[perm]
    h0T = np.ascontiguousarray(h0p.T)

    return dict(
        h0T=h0T.astype(BF),
        e0T=np.ascontiguousarray(e0_pad.T).astype(BF),
        ipack=ipack,
        mpack=mpack,
        origid=origid,
    )


# ---------------------------------------------------------------------------
# device program
# ---------------------------------------------------------------------------


def _build_program(plan, stop_after="full", epochs=1):
    import concourse.bass as bass
    import concourse.mybir as mybir
    import concourse.tile as tile
    from concourse import bacc

    F32 = mybir.dt.float32
    BF16 = mybir.dt.bfloat16
    I16 = mybir.dt.int16
    AF = mybir.ActivationFunctionType
    OP = mybir.AluOpType
    AX = mybir.AxisListType

    NODE_CAP, T = plan["NODE_CAP"], plan["T"]
    E_PAD, C_E = plan["E_PAD"], plan["C_E"]
    chunks, groups, megas = plan["chunks"], plan["groups"], plan["megas"]
    NG = len(groups)
    N, E = plan["N"], plan["E"]
    S_vals = sorted(set(plan["S_list"]))
    kron_of = {s: i for i, s in enumerate(S_vals)}
    F_N, F_E = plan["F_N"], plan["F_E"]
    EPS = 1e-5
    HALF = 4 * NODE_CAP
    NI16 = E_PAD // 16

    _phases = ["embed", "bound0", "epass0", "layer0", "layer1", "full"]
    lvl = _phases.index(stop_after)

    nc = bacc.Bacc(
        "TRN2", target_bir_lowering=False, debug=False, num_devices=NC
    )

    def din(name, shape, dt=F32):
        return nc.dram_tensor(name, shape, dt, kind="ExternalInput")

    # per-core inputs
    WMAP, RW, BMAP, RB = _pack_layout(plan)
    h0T = din("h0T", [F_N, NODE_CAP], BF16)
    e0T = din("e0T", [F_E, E_PAD], BF16)
    ipack_d = din("ipack", [128, 2 * NI16], I16)
    NW = (T + 3) // 4
    mpack_d = din("mpack", [NG + NW, 512])
    wpack_d = din("wpack", [RW, 384])
    bpack_d = din("bpack", [RB, 256], BF16)

    y_out = nc.dram_tensor("y", [1, E_PAD], F32, kind="ExternalOutput")

    rg = [list(range(NC))]

    with tile.TileContext(nc) as tc:
        with (
            tc.tile_pool(name="const", bufs=1) as cp,
            tc.tile_pool(name="pers", bufs=1) as pp,
            tc.tile_pool(name="st", bufs=1) as stp,
            tc.tile_pool(name="s", bufs=2) as sp,
            tc.tile_pool(name="ps", bufs=2, space="PSUM") as ps,
            tc.tile_pool(name="dram", bufs=1, space="DRAM") as dp,
        ):
            # ---- load constants from packs
            def wload(name, width, dt=F32, pack=None, pmap=None):
                pk = pack if pack is not None else wpack_d
                mp = pmap if pmap is not None else WMAP
                r0, rows = mp[name]
                t = cp.tile([rows, width], dt, name=f"{name}_sb")
                nc.sync.dma_start(out=t[:], in_=pk[r0 : r0 + rows, :width])
                return t

            def wload_col(name, dt=F32, pack=None, pmap=None):
                pk = pack if pack is not None else wpack_d
                mp = pmap if pmap is not None else WMAP
                r0, rows = mp[name]
                t = cp.tile([128, 1], dt, name=f"{name}_sb")
                nc.sync.dma_start(
                    out=t[:], in_=pk[r0 : r0 + 1, :128].rearrange("a p -> p a")
                )
                return t

            def bload(name, width, dt=BF16):
                return wload(name, width, dt, pack=bpack_d, pmap=BMAP)

            ones_row = wload("ones_row", 128)
            embe_w = bload("emb_e_w", 128)
            embn_w = bload("emb_n_w", 128)
            embe_b = wload_col("emb_e_b")
            embn_b = wload_col("emb_n_b")
            A_sb = [bload(f"A{l}", 128) for l in range(2)]
            VCB0 = wload("VCB0", 384)
            V1 = wload("V1", 128)
            U_sb = [wload(f"U{l}", 128) for l in range(2)]
            WBC = wload("WBC", 256)
            W0a = wload("W0a", 128)
            W0bc = wload_col("W0b_col")
            Wk = [bload(f"Wk{k}", 128) for k in range(2)]
            Wkb = [wload_col(f"Wkb{k}") for k in range(2)]
            Wfr0, _ = BMAP["Wf"]
            Wf = cp.tile([128, 1], BF16, name="Wf_sb")
            nc.sync.dma_start(
                out=Wf[:], in_=bpack_d[Wfr0 : Wfr0 + 1, :128].rearrange("a p -> p a")
            )
            wfb = wload("wfb", 1)
            kron_bb = [bload(f"kronsb{i}", 128) for i in range(len(S_vals))]
            ipack = cp.tile([128, 2 * NI16], I16, name="ipack_sb")
            nc.sync.dma_start(out=ipack[:], in_=ipack_d[:])
            eps_col = cp.tile([128, 1], F32, name="eps_col")
            nc.gpsimd.memset(eps_col[:], EPS)

            # ---- dram buffers
            e_buf = dp.tile([128, E_PAD], BF16, name="e_buf")
            z_buf = dp.tile([128, E_PAD], BF16, name="z_buf")
            w_buf = dp.tile([128, E_PAD], BF16, name="w_buf")
            hb_buf = dp.tile([NODE_CAP, 128], BF16, name="hb_buf")
            hfm_buf = [
                dp.tile([128, NODE_CAP], F32, name=f"hfm_buf{i}")
                for i in range(3)
            ]
            hlocal = dp.tile([NODE_CAP, 128], BF16, name="hlocal")
            cc_hin = [
                dp.tile(
                    [NODE_CAP, 256 if l == 0 else 128],
                    BF16,
                    name=f"cc_hin{l}",
                )
                for l in range(3)
            ]
            cc_hout_ep = [
                [
                    dp.tile(
                        [NC * NODE_CAP, 256 if l == 0 else 128],
                        BF16,
                        name=f"cc_hout{l}_e{e_}",
                        addr_space="Shared",
                    )
                    for l in range(3)
                ]
                for e_ in range(epochs)
            ]
            cc_st_in = [
                dp.tile([128, 4 if l == 0 else 2], F32, name=f"cc_st_in{l}")
                for l in range(2)
            ]
            cc_st_out_ep = [
                [
                    dp.tile(
                        [NC * 128, 4 if l == 0 else 2],
                        F32,
                        name=f"cc_st_out{l}_e{e_}",
                        addr_space="Shared",
                    )
                    for l in range(2)
                ]
                for e_ in range(epochs)
            ]
            cc_moy_in = dp.tile([128, 1], F32, name="cc_moy_in")
            cc_moy_out_ep = [
                dp.tile(
                    [NC * 128, 1], F32, name=f"cc_moy_out_e{e_}",
                    addr_space="Shared",
                )
                for e_ in range(epochs)
            ]

            def _epoch(ep):
                cc_hout = cc_hout_ep[ep]
                cc_st_out = cc_st_out_ep[ep]
                cc_moy_out = cc_moy_out_ep[ep]
                # ---- persistent sbuf tiles
                hU = [None] * NW
                agg = [None] * NW

                def group_gather_t(c0, gsz, l):
                    """Dual transposing gathers (bf16 feat-major) + merge:
                    gv[p, k*128 + q] = sum_half table[idx[(c0+k)*128+q], p]."""
                    n = gsz * 128
                    ga = sp.tile([128, 512], BF16, tag="gva", bufs=2)
                    gb = sp.tile([128, 512], BF16, tag="gvb", bufs=2)
                    gv = sp.tile([128, 512], BF16, tag="gvm", bufs=2)
                    for g, ioff, r0 in ((ga, 0, 0), (gb, NI16, HALF)):
                        nc.gpsimd.dma_gather(
                            g[:, :n].rearrange("p (j i) -> p j i", i=n),
                            cc_hout[l][r0 : r0 + HALF, :],
                            ipack[:, ioff + c0 * 8 : ioff + (c0 + gsz) * 8],
                            n,
                            n,
                            128,
                            transpose=True,
                            single_packet=False,
                        )
                    nc.vector.tensor_tensor(
                        out=gv[:, :n], in0=ga[:, :n], in1=gb[:, :n], op=OP.add
                    )
                    return gv

                def group_gather_t2(c0, gsz):
                    """Dual transposing gathers (bf16 feat-major, 256-wide
                    table) + merge: gv[p, 0*n+i] = V-part feat p of slot i,
                    gv[p, 1*n+i] = C-part feat p of slot i."""
                    n = gsz * 128
                    ga = sp.tile([128, 1024], BF16, tag="gta", bufs=2)
                    gb = sp.tile([128, 1024], BF16, tag="gtb", bufs=2)
                    gv = sp.tile([128, 1024], BF16, tag="gtm", bufs=2)
                    for g, ioff, r0 in ((ga, 0, 0), (gb, NI16, HALF)):
                        nc.gpsimd.dma_gather(
                            g[:, : 2 * n].rearrange("p (j i) -> p j i", i=n),
                            cc_hout[0][r0 : r0 + HALF, :],
                            ipack[:, ioff + c0 * 8 : ioff + (c0 + gsz) * 8],
                            n,
                            n,
                            256,
                            transpose=True,
                            single_packet=False,
                        )
                    nc.vector.tensor_tensor(
                        out=gv[:, : 2 * n], in0=ga[:, : 2 * n],
                        in1=gb[:, : 2 * n], op=OP.add,
                    )
                    return gv

                # ================= embed h =================
                for t in range(T):
                    h0sl = sp.tile([F_N, 128], BF16, tag="h0sl", bufs=2, name="h0sl")
                    nc.sync.dma_start(
                        out=h0sl[:], in_=h0T[:, t * 128 : (t + 1) * 128]
                    )
                    ph = ps.tile([128, 128], F32, tag="pc")
                    nc.tensor.matmul(
                        out=ph[:], lhsT=embn_w[:], rhs=h0sl[:],
                        start=True, stop=True,
                    )
                    hf = sp.tile([128, 128], F32, tag="hnew", bufs=4, name="hemb")
                    nc.scalar.activation(
                        out=hf[:], in_=ph[:], func=AF.Identity, bias=embn_b[:]
                    )
                    nc.sync.dma_start(
                        out=hfm_buf[0][:, t * 128 : (t + 1) * 128], in_=hf[:]
                    )

                # ================= boundary =================
                def boundary(l):
                    """Build tables for layer l (or readout if l==2) from hfm."""
                    if l == 0:
                        rhs, wdt, U = VCB0, 384, U_sb[0]
                    elif l == 1:
                        rhs, wdt, U = V1, 128, U_sb[1]
                    else:
                        rhs, wdt, U = WBC, 256, None
                    scat_w = 256 if l == 0 else 128
                    bdt = BF16
                    for t in range(T):
                        hfl = sp.tile([128, 128], F32, tag="hfl", bufs=4, name="hfl")
                        nc.sync.dma_start(
                            out=hfl[:], in_=hfm_buf[l][:, t * 128 : (t + 1) * 128]
                        )
                        pb = ps.tile([128, 512], F32, tag="pa")
                        nc.tensor.matmul(
                            out=pb[:, :wdt], lhsT=hfl[:], rhs=rhs[:],
                            start=True, stop=True,
                        )
                        bsb = sp.tile([128, 512], bdt, tag="bsbb", bufs=2)
                        nc.scalar.activation(
                            out=bsb[:, :wdt], in_=pb[:, :wdt], func=AF.Copy
                        )
                        nc.sync.dma_start(
                            out=cc_hin[l][t * 128 : (t + 1) * 128, :scat_w],
                            in_=bsb[:, :scat_w],
                        )
                        if l == 0:
                            nc.sync.dma_start(
                                out=hb_buf[t * 128 : (t + 1) * 128, :],
                                in_=bsb[:, 256:384],
                            )
                        if l == 2:
                            nc.sync.dma_start(
                                out=hlocal[t * 128 : (t + 1) * 128, :],
                                in_=bsb[:, 128:256],
                            )
                        if U is not None:
                            pu = ps.tile([128, 128], F32, tag="pc")
                            nc.tensor.matmul(
                                out=pu[:], lhsT=U[:], rhs=hfl[:],
                                start=True, stop=True,
                            )
                            if t % 4 == 0:
                                hU[t // 4] = pp.tile(
                                    [128, 512], F32, tag=f"hUw{t // 4}",
                                    name=f"hUw_{t // 4}_{l}_{ep}",
                                )
                            nc.scalar.activation(
                                out=hU[t // 4][
                                    :, (t % 4) * 128 : (t % 4) * 128 + 128
                                ],
                                in_=pu[:], func=AF.Copy,
                            )
                    # zero row (NODE_CAP-2) + guard row (NODE_CAP-1)
                    fr0, _ = BMAP["fill2b"]
                    nc.sync.dma_start(
                        out=cc_hin[l][NODE_CAP - 2 : NODE_CAP, :scat_w],
                        in_=bpack_d[fr0 : fr0 + 2, :scat_w],
                    )
                    nc.gpsimd.collective_compute(
                        "AllGather",
                        OP.bypass,
                        replica_groups=rg,
                        ins=[cc_hin[l][:]],
                        outs=[cc_hout[l][:]],
                    )

                if lvl >= 1:
                    boundary(0)

                # ================= embed e =================
                for gi, (c0, gsz) in enumerate(groups):
                    w = gsz * 128
                    e0sl = sp.tile([F_E, 512], BF16, tag="e0sl")
                    nc.sync.dma_start(
                        out=e0sl[:, :w], in_=e0T[:, c0 * 128 : c0 * 128 + w]
                    )
                    pe = ps.tile([128, 512], F32, tag="pa")
                    nc.tensor.matmul(
                        out=pe[:, :w], lhsT=embe_w[:], rhs=e0sl[:, :w],
                        start=True, stop=True,
                    )
                    esb = sp.tile([128, 512], BF16, tag="esb", bufs=2)
                    nc.scalar.activation(
                        out=esb[:, :w], in_=pe[:, :w], func=AF.Identity,
                        bias=embe_b[:],
                    )
                    nc.sync.dma_start(
                        out=e_buf[:, c0 * 128 : c0 * 128 + w], in_=esb[:, :w]
                    )


                # ================= layers =================
                for l in range(2):
                    if l == 0 and lvl < 2:
                        break
                    if l == 1 and lvl < 4:
                        break
                    # stats accumulators
                    if l == 0:
                        ssum_e = stp.tile([128, NG], F32, name=f"ssum_e{ep}")
                        ssq_e = stp.tile([128, NG], F32, name=f"ssq_e{ep}")
                    hsum = stp.tile([128, NW], F32, name=f"hsum{l}_{ep}")
                    hssq = stp.tile([128, NW], F32, name=f"hssq{l}_{ep}")

                    # ---- e-pass (layer 1's is fused into the l==0 e-update)
                    for gi, (c0, gsz) in enumerate(groups if l == 0 else []):
                        if True:
                            w = gsz * 128
                            t, ci0, S = chunks[c0]
                            G = 128 // S
                            gv = group_gather_t2(c0, gsz)
                            esb = sp.tile([128, 512], BF16, tag="esb", bufs=2)
                            nc.sync.dma_start(
                                out=esb[:, :w],
                                in_=e_buf[:, c0 * 128 : c0 * 128 + w],
                            )
                            wsb = sp.tile([128, 512], BF16, tag="wsb", bufs=2)
                            nc.scalar.activation(
                                out=wsb[:, :w], in_=esb[:, :w], func=AF.Sigmoid
                            )
                            mrow = sp.tile([1, 512], F32, tag="mrow", bufs=1)
                            nc.sync.dma_start(out=mrow[:], in_=mpack_d[gi : gi + 1, :])
                            pm = ps.tile([128, 512], F32, tag="pb")
                            nc.tensor.matmul(
                                out=pm[:, :w], lhsT=ones_row[:], rhs=mrow[:, :w],
                                start=True, stop=True,
                            )
                            pz = ps.tile([128, 512], F32, tag="pa")
                            nc.tensor.matmul(
                                out=pz[:, :w], lhsT=A_sb[l][:], rhs=esb[:, :w],
                                start=True, stop=False, skip_group_check=True,
                            )
                            band = sp.tile([128, 512], BF16, tag="hbt", bufs=2, name="band")
                            nc.sync.dma_start(
                                out=band[:G, : gsz * 128].rearrange(
                                    "p (k c) -> p k c", c=128
                                ),
                                in_=hb_buf[
                                    t * 128 + ci0 * G : t * 128 + (ci0 + gsz) * G, :
                                ].rearrange("(k p) c -> p k c", p=G),
                            )
                            for k in range(gsz):
                                nc.tensor.matmul(
                                    out=pz[:, k * 128 : (k + 1) * 128],
                                    lhsT=band[:G, k * 128 : (k + 1) * 128],
                                    rhs=kron_bb[kron_of[S]][:G, :],
                                    start=False, stop=(k == gsz - 1),
                                    skip_group_check=True,
                                )
                            # message path: msg = (hV0)[src] * w, windowed max
                            msg = sp.tile([128, 512], BF16, tag="msg", bufs=2)
                            nc.vector.tensor_tensor(
                                out=msg[:, :w], in0=gv[:, :w],
                                in1=wsb[:, :w], op=OP.mult,
                            )
                            for k in range(gsz):
                                ci = ci0 + k
                                if ci == 0 and t % 4 == 0:
                                    agg[t // 4] = pp.tile(
                                        [128, 512], F32, tag=f"aggw{t // 4}",
                                        name=f"aggw_{t // 4}_0_{ep}",
                                    )
                                ao = (t % 4) * 128
                                nc.vector.tensor_reduce(
                                    out=agg[t // 4][
                                        :, ao + ci * G : ao + (ci + 1) * G
                                    ],
                                    in_=msg[:, k * 128 : (k + 1) * 128].rearrange(
                                        "p (g s) -> p g s", s=S
                                    ),
                                    op=OP.max,
                                    axis=AX.X,
                                )
                            # z path: z = A e + (C h)[src] + kron(B h local)
                            zsum = sp.tile([128, 512], F32, tag="zsum", bufs=2)
                            nc.vector.tensor_tensor(
                                out=zsum[:, :w], in0=pz[:, :w],
                                in1=gv[:, w : 2 * w], op=OP.add,
                            )
                            zm = sp.tile([128, 512], BF16, tag="zm", bufs=2)
                            if USE_TTR:
                                nc.vector.tensor_tensor_reduce(
                                    out=zm[:, :w], in0=zsum[:, :w], in1=pm[:, :w],
                                    scale=1.0, scalar=0.0, op0=OP.mult, op1=OP.add,
                                    accum_out=ssum_e[:, gi : gi + 1],
                                )
                                sq = sp.tile([128, 512], BF16, tag="sq", bufs=2)
                                nc.vector.tensor_tensor_reduce(
                                    out=sq[:, :w], in0=zm[:, :w], in1=zm[:, :w],
                                    scale=1.0, scalar=0.0, op0=OP.mult, op1=OP.add,
                                    accum_out=ssq_e[:, gi : gi + 1],
                                )
                            else:
                                nc.vector.tensor_tensor(
                                    out=zm[:, :w], in0=zsum[:, :w], in1=pm[:, :w],
                                    op=OP.mult,
                                )
                                nc.vector.tensor_reduce(
                                    out=ssum_e[:, gi : gi + 1], in_=zm[:, :w],
                                    op=OP.add, axis=AX.X,
                                )
                                sq = sp.tile([128, 512], F32, tag="sq", bufs=2)
                                nc.scalar.activation(
                                    out=sq[:, :w], in_=zm[:, :w], func=AF.Square
                                )
                                nc.vector.tensor_reduce(
                                    out=ssq_e[:, gi : gi + 1], in_=sq[:, :w],
                                    op=OP.add, axis=AX.X,
                                )
                            nc.sync.dma_start(
                                out=z_buf[:, c0 * 128 : c0 * 128 + w],
                                in_=zm[:, :w],
                            )

                    if l == 0 and lvl < 3:
                        break

                    # ---- h side: z_h = hU + select(agg); masked stats (wide)
                    for j in range(NW):
                        wj = min(512, (T - 4 * j) * 128)
                        if USE_STT:
                            nc.vector.scalar_tensor_tensor(
                                out=agg[j][:, :wj], in0=agg[j][:, :wj],
                                scalar=-1e20, in1=agg[j][:, :wj],
                                op0=OP.is_gt, op1=OP.mult,
                            )
                        else:
                            m01 = sp.tile([128, 512], F32, tag="zhm", bufs=3)
                            nc.vector.tensor_scalar(
                                out=m01[:, :wj], in0=agg[j][:, :wj],
                                scalar1=-1e20, scalar2=None, op0=OP.is_gt,
                            )
                            nc.vector.tensor_tensor(
                                out=agg[j][:, :wj], in0=agg[j][:, :wj],
                                in1=m01[:, :wj], op=OP.mult,
                            )
                        nc.vector.tensor_tensor(
                            out=agg[j][:, :wj], in0=agg[j][:, :wj],
                            in1=hU[j][:, :wj], op=OP.add,
                        )
                        nmr = sp.tile([1, 512], F32, tag="nmr", bufs=1)
                        nc.sync.dma_start(
                            out=nmr[:], in_=mpack_d[NG + j : NG + j + 1, :]
                        )
                        pnm = ps.tile([128, 512], F32, tag="pc")
                        nc.tensor.matmul(
                            out=pnm[:, :wj], lhsT=ones_row[:], rhs=nmr[:, :wj],
                            start=True, stop=True, skip_group_check=True,
                        )
                        zhm = sp.tile([128, 512], F32, tag="zhm", bufs=3)
                        if USE_TTR:
                            nc.vector.tensor_tensor_reduce(
                                out=zhm[:, :wj], in0=agg[j][:, :wj],
                                in1=pnm[:, :wj], scale=1.0, scalar=0.0,
                                op0=OP.mult, op1=OP.add,
                                accum_out=hsum[:, j : j + 1],
                            )
                            sqh = sp.tile([128, 512], F32, tag="zhm", bufs=3)
                            nc.vector.tensor_tensor_reduce(
                                out=sqh[:, :wj], in0=zhm[:, :wj], in1=zhm[:, :wj],
                                scale=1.0, scalar=0.0, op0=OP.mult, op1=OP.add,
                                accum_out=hssq[:, j : j + 1],
                            )
                        else:
                            nc.vector.tensor_tensor(
                                out=zhm[:, :wj], in0=agg[j][:, :wj],
                                in1=pnm[:, :wj], op=OP.mult,
                            )
                            nc.vector.tensor_reduce(
                                out=hsum[:, j : j + 1], in_=zhm[:, :wj],
                                op=OP.add, axis=AX.X,
                            )
                            sqh = sp.tile([128, 512], F32, tag="zhm", bufs=3)
                            nc.scalar.activation(
                                out=sqh[:, :wj], in_=zhm[:, :wj], func=AF.Square
                            )
                            nc.vector.tensor_reduce(
                                out=hssq[:, j : j + 1], in_=sqh[:, :wj],
                                op=OP.add, axis=AX.X,
                            )

                    # ---- pack + allreduce stats
                    ncols = 4 if l == 0 else 2
                    pack = stp.tile([128, 4], F32, name=f"pack{l}_{ep}")
                    nc.vector.tensor_reduce(
                        out=pack[:, 0:1], in_=hsum[:], op=OP.add, axis=AX.X
                    )
                    nc.vector.tensor_reduce(
                        out=pack[:, 1:2], in_=hssq[:], op=OP.add, axis=AX.X
                    )
                    if l == 0:
                        nc.vector.tensor_reduce(
                            out=pack[:, 2:3], in_=ssum_e[:], op=OP.add, axis=AX.X
                        )
                        nc.vector.tensor_reduce(
                            out=pack[:, 3:4], in_=ssq_e[:], op=OP.add, axis=AX.X
                        )
                    nc.sync.dma_start(out=cc_st_in[l][:], in_=pack[:, :ncols])
                    nc.gpsimd.collective_compute(
                        "AllGather", OP.bypass, replica_groups=rg,
                        ins=[cc_st_in[l][:]], outs=[cc_st_out[l][:]],
                    )
                    gat = stp.tile([128, 4 * NC], F32, name=f"gat{l}_{ep}")
                    nc.sync.dma_start(
                        out=gat[:, : ncols * NC].rearrange("p (f c) -> p f c", c=NC),
                        in_=cc_st_out[l][:].rearrange("(c p) f -> p f c", p=128),
                    )
                    stt = stp.tile([128, 4], F32, name=f"stt{l}_{ep}")
                    nc.vector.tensor_reduce(
                        out=stt[:, :ncols].rearrange("p (f x) -> p f x", x=1),
                        in_=gat[:, : ncols * NC].rearrange("p (f c) -> p f c", c=NC),
                        op=OP.add, axis=AX.X,
                    )

                    # ---- bn coefficients
                    def bn_cols(sum_c, ssq_c, count, pref):
                        mean = stp.tile([128, 1], F32, name=f"{pref}mean{l}_{ep}")
                        nc.vector.tensor_scalar(
                            out=mean[:], in0=sum_c, scalar1=1.0 / count,
                            scalar2=None, op0=OP.mult,
                        )
                        msq = stp.tile([128, 1], F32, name=f"{pref}msq{l}_{ep}")
                        nc.vector.tensor_scalar(
                            out=msq[:], in0=ssq_c, scalar1=1.0 / count,
                            scalar2=None, op0=OP.mult,
                        )
                        m2 = stp.tile([128, 1], F32, name=f"{pref}m2{l}_{ep}")
                        nc.scalar.activation(out=m2[:], in_=mean[:], func=AF.Square)
                        var = stp.tile([128, 1], F32, name=f"{pref}var{l}_{ep}")
                        nc.vector.tensor_tensor(
                            out=var[:], in0=msq[:], in1=m2[:], op=OP.subtract
                        )
                        sd = stp.tile([128, 1], F32, name=f"{pref}sd{l}_{ep}")
                        nc.scalar.activation(
                            out=sd[:], in_=var[:], func=AF.Sqrt, bias=eps_col[:]
                        )
                        rs = stp.tile([128, 1], F32, name=f"{pref}rs{l}_{ep}")
                        nc.vector.reciprocal(out=rs[:], in_=sd[:])
                        bb = stp.tile([128, 1], F32, name=f"{pref}bb{l}_{ep}")
                        nc.vector.tensor_tensor(
                            out=bb[:], in0=mean[:], in1=rs[:], op=OP.mult
                        )
                        nc.vector.tensor_scalar(
                            out=bb[:], in0=bb[:], scalar1=-1.0, scalar2=None,
                            op0=OP.mult,
                        )
                        return rs, bb

                    rs_h, bb_h = bn_cols(stt[:, 0:1], stt[:, 1:2], N, "h")
                    if l == 0:
                        rs_e, bb_e = bn_cols(stt[:, 2:3], stt[:, 3:4], E, "e")

                    # ---- h update (wide)
                    for j in range(NW):
                        wj = min(512, (T - 4 * j) * 128)
                        r = sp.tile([128, 512], F32, tag="rh", bufs=2)
                        nc.scalar.activation(
                            out=r[:, :wj], in_=agg[j][:, :wj], func=AF.Relu,
                            bias=bb_h[:], scale=rs_h[:],
                        )
                        hfl = sp.tile([128, 512], F32, tag="hflw", bufs=2, name="hflu")
                        nc.sync.dma_start(
                            out=hfl[:, :wj],
                            in_=hfm_buf[l][:, j * 512 : j * 512 + wj],
                        )
                        hf2 = sp.tile([128, 512], F32, tag="hneww", bufs=2, name="hupd")
                        nc.vector.tensor_tensor(
                            out=hf2[:, :wj], in0=hfl[:, :wj], in1=r[:, :wj],
                            op=OP.add,
                        )
                        nc.sync.dma_start(
                            out=hfm_buf[l + 1][:, j * 512 : j * 512 + wj],
                            in_=hf2[:, :wj],
                        )

                    boundary(l + 1)

                    # ---- e update (pass 1, overlaps AllGather 1): w1 -> w_buf
                    if l == 0:
                        for gi, (c0, gsz) in enumerate(groups):
                            if True:
                                w = gsz * 128
                                zsb = sp.tile([128, 512], BF16, tag="zsb", bufs=2)
                                nc.sync.dma_start(
                                    out=zsb[:, :w],
                                    in_=z_buf[:, c0 * 128 : c0 * 128 + w],
                                )
                                r = sp.tile([128, 512], F32, tag="re", bufs=2)
                                nc.scalar.activation(
                                    out=r[:, :w], in_=zsb[:, :w], func=AF.Relu,
                                    bias=bb_e[:], scale=rs_e[:],
                                )
                                eold = sp.tile([128, 512], BF16, tag="esb", bufs=2)
                                nc.sync.dma_start(
                                    out=eold[:, :w],
                                    in_=e_buf[:, c0 * 128 : c0 * 128 + w],
                                )
                                enew = sp.tile([128, 512], F32, tag="enew", bufs=2)
                                nc.vector.tensor_tensor(
                                    out=enew[:, :w], in0=eold[:, :w], in1=r[:, :w],
                                    op=OP.add,
                                )
                                w1 = sp.tile([128, 512], BF16, tag="wsb", bufs=2)
                                nc.scalar.activation(
                                    out=w1[:, :w], in_=enew[:, :w], func=AF.Sigmoid
                                )
                                nc.sync.dma_start(
                                    out=w_buf[:, c0 * 128 : c0 * 128 + w],
                                    in_=w1[:, :w],
                                )
                        # ---- pass 2: layer-1 messages (needs AllGather 1)
                        for gi, (c0, gsz) in enumerate(groups):
                            if True:
                                w = gsz * 128
                                t, ci0, S = chunks[c0]
                                G = 128 // S
                                gv1 = group_gather_t(c0, gsz, 1)
                                w1sb = sp.tile([128, 512], BF16, tag="w1sb", bufs=2)
                                nc.sync.dma_start(
                                    out=w1sb[:, :w],
                                    in_=w_buf[:, c0 * 128 : c0 * 128 + w],
                                )
                                msgg = sp.tile([128, 512], BF16, tag="msgg", bufs=2)
                                nc.vector.tensor_tensor(
                                    out=msgg[:, :w], in0=gv1[:, :w],
                                    in1=w1sb[:, :w], op=OP.mult,
                                )
                                for k in range(gsz):
                                    ci = ci0 + k
                                    if ci == 0 and t % 4 == 0:
                                        agg[t // 4] = pp.tile(
                                            [128, 512], F32, tag=f"aggw{t // 4}",
                                            name=f"aggw_{t // 4}_1_{ep}",
                                        )
                                    ao = (t % 4) * 128
                                    nc.vector.tensor_reduce(
                                        out=agg[t // 4][
                                            :, ao + ci * G : ao + (ci + 1) * G
                                        ],
                                        in_=msgg[:, k * 128 : (k + 1) * 128].rearrange(
                                            "p (g s) -> p g s", s=S
                                        ),
                                        op=OP.max,
                                        axis=AX.X,
                                    )

            # ================= moy + base =================
                if lvl < 5:
                    ydummy = sp.tile([1, 4096], F32, tag="ydummy", bufs=1)
                    nc.gpsimd.memset(ydummy[:], 0.0)
                    for c0 in range(0, E_PAD, 4096):
                        w = min(4096, E_PAD - c0)
                        nc.sync.dma_start(
                            out=y_out[0:1, c0 : c0 + w], in_=ydummy[:, :w]
                        )
                else:
                    moysum = stp.tile([128, NW], F32, name=f"moysum{ep}")
                    for j in range(NW):
                        wj = min(512, (T - 4 * j) * 128)
                        nmr = sp.tile([1, 512], F32, tag="nmr", bufs=1)
                        nc.sync.dma_start(
                            out=nmr[:], in_=mpack_d[NG + j : NG + j + 1, :]
                        )
                        pnm = ps.tile([128, 512], F32, tag="pc")
                        nc.tensor.matmul(
                            out=pnm[:, :wj], lhsT=ones_row[:], rhs=nmr[:, :wj],
                            start=True, stop=True, skip_group_check=True,
                        )
                        hfl = sp.tile([128, 512], F32, tag="hflw", bufs=2, name="hflm")
                        nc.sync.dma_start(
                            out=hfl[:, :wj],
                            in_=hfm_buf[2][:, j * 512 : j * 512 + wj],
                        )
                        hm = sp.tile([128, 512], F32, tag="zhm", bufs=3)
                        nc.vector.tensor_tensor(
                            out=hm[:, :wj], in0=hfl[:, :wj], in1=pnm[:, :wj],
                            op=OP.mult,
                        )
                        nc.vector.tensor_reduce(
                            out=moysum[:, j : j + 1], in_=hm[:, :wj], op=OP.add,
                            axis=AX.X,
                        )
                    moyp = stp.tile([128, 1], F32, name=f"moyp{ep}")
                    nc.vector.tensor_reduce(
                        out=moyp[:], in_=moysum[:], op=OP.add, axis=AX.X
                    )
                    nc.sync.dma_start(out=cc_moy_in[:], in_=moyp[:])
                    nc.gpsimd.collective_compute(
                        "AllGather", OP.bypass, replica_groups=rg,
                        ins=[cc_moy_in[:]], outs=[cc_moy_out[:]],
                    )
                    gatm = stp.tile([128, NC], F32, name=f"gatm{ep}")
                    nc.sync.dma_start(
                        out=gatm[:].rearrange("p (f c) -> p f c", c=NC),
                        in_=cc_moy_out[:].rearrange("(c p) f -> p f c", p=128),
                    )
                    moyc = stp.tile([128, 1], F32, name=f"moyc{ep}")
                    nc.vector.tensor_reduce(
                        out=moyc[:].rearrange("p (f x) -> p f x", x=1),
                        in_=gatm[:].rearrange("p (f c) -> p f c", c=NC),
                        op=OP.add, axis=AX.X,
                    )
                    nc.vector.tensor_scalar(
                        out=moyc[:], in0=moyc[:], scalar1=1.0 / N, scalar2=None,
                        op0=OP.mult,
                    )
                    pbase = ps.tile([128, 128], F32, tag="pc")
                    nc.tensor.matmul(
                        out=pbase[:, 0:1], lhsT=W0a[:], rhs=moyc[:],
                        start=True, stop=True, skip_group_check=True,
                    )
                    base_col = stp.tile([128, 1], F32, name=f"base_col{ep}")
                    nc.vector.tensor_tensor(
                        out=base_col[:], in0=pbase[:, 0:1], in1=W0bc[:], op=OP.add
                    )

                    # ================= readout =================
                    for gi, (c0, gsz) in enumerate(groups):
                        if True:
                            w = gsz * 128
                            t, ci0, S = chunks[c0]
                            G = 128 // S
                            gvr = group_gather_t(c0, gsz, 2)
                            pm1 = ps.tile([128, 512], F32, tag="pa")
                            band = sp.tile(
                                [128, 512], BF16, tag="hbtb", bufs=2, name="bandb"
                            )
                            nc.sync.dma_start(
                                out=band[:G, : gsz * 128].rearrange(
                                    "p (k c) -> p k c", c=128
                                ),
                                in_=hlocal[
                                    t * 128 + ci0 * G : t * 128 + (ci0 + gsz) * G, :
                                ].rearrange("(k p) c -> p k c", p=G),
                            )
                            for k in range(gsz):
                                nc.tensor.matmul(
                                    out=pm1[:, k * 128 : (k + 1) * 128],
                                    lhsT=band[:G, k * 128 : (k + 1) * 128],
                                    rhs=kron_bb[kron_of[S]][:G, :],
                                    start=True, stop=True,
                                    skip_group_check=True,
                                )
                            zs = sp.tile([128, 512], F32, tag="zs", bufs=2)
                            nc.vector.tensor_tensor(
                                out=zs[:, :w], in0=pm1[:, :w],
                                in1=gvr[:, :w], op=OP.add,
                            )
                            t1 = sp.tile([128, 512], BF16, tag="t1", bufs=2)
                            nc.scalar.activation(
                                out=t1[:, :w], in_=zs[:, :w], func=AF.Relu,
                                bias=base_col[:],
                            )
                            pt2 = ps.tile([128, 512], F32, tag="pb")
                            nc.tensor.matmul(
                                out=pt2[:, :w], lhsT=Wk[0][:], rhs=t1[:, :w],
                                start=True, stop=True, skip_group_check=True,
                            )
                            t2 = sp.tile([128, 512], BF16, tag="t2", bufs=2)
                            if USE_TS:
                                nc.vector.tensor_scalar(
                                    out=t2[:, :w], in0=pt2[:, :w],
                                    scalar1=Wkb[0][:], scalar2=0.0,
                                    op0=OP.add, op1=OP.max,
                                )
                            else:
                                nc.scalar.activation(
                                    out=t2[:, :w], in_=pt2[:, :w], func=AF.Relu,
                                    bias=Wkb[0][:],
                                )
                            pt3 = ps.tile([128, 512], F32, tag="pc")
                            nc.tensor.matmul(
                                out=pt3[:, :w], lhsT=Wk[1][:], rhs=t2[:, :w],
                                start=True, stop=True, skip_group_check=True,
                            )
                            t3 = sp.tile([128, 512], BF16, tag="t3", bufs=2)
                            nc.scalar.activation(
                                out=t3[:, :w], in_=pt3[:, :w], func=AF.Relu,
                                bias=Wkb[1][:],
                            )
                            py = ps.tile([1, 512], F32, tag="pdy", bufs=2, name="py")
                            nc.tensor.matmul(
                                out=py[:, :w], lhsT=Wf[:], rhs=t3[:, :w],
                                start=True, stop=True, skip_group_check=True,
                            )
                            ysb = sp.tile([1, 512], F32, tag="ysb", bufs=2)
                            nc.scalar.activation(
                                out=ysb[:, :w], in_=py[:, :w], func=AF.Sigmoid,
                                bias=wfb[:],
                            )
                            nc.sync.dma_start(
                                out=y_out[0:1, c0 * 128 : c0 * 128 + w],
                                in_=ysb[:, :w],
                            )


            for _ep in range(epochs):
                _epoch(_ep)

    nc.compile()
    return nc


# ---------------------------------------------------------------------------
# top level
# ---------------------------------------------------------------------------


def _make_kron(S):
    G = 128 // S
    k = np.zeros((128, 128), np.float32)
    for p in range(128):
        g = p % G
        k[p, g * S : (g + 1) * S] = 1.0
    return k


def _prep(inputs):
    """plan + per-core input maps + origids (host-side prep)."""
    import ml_dtypes

    BF = ml_dtypes.bfloat16
    h = np.asarray(inputs["h"], np.float32)
    e = np.asarray(inputs["e"], np.float32)
    src = np.asarray(inputs["src"]).astype(np.int64)
    dst = np.asarray(inputs["dst"]).astype(np.int64)
    N = h.shape[0]

    plan = _plan(src, dst, N)
    plan["F_N"] = h.shape[1]
    plan["F_E"] = e.shape[1]

    U = np.asarray(inputs["U"], np.float32)
    V = np.asarray(inputs["V"], np.float32)
    A = np.asarray(inputs["A"], np.float32)
    B = np.asarray(inputs["B"], np.float32)
    C = np.asarray(inputs["C"], np.float32)
    W0_w = np.asarray(inputs["W0_w"], np.float32)
    Wk_w = np.asarray(inputs["Wk_w"], np.float32)
    Wk_b = np.asarray(inputs["Wk_b"], np.float32)
    Wf_w = np.asarray(inputs["Wf_w"], np.float32)
    Wf_b = np.asarray(inputs["Wf_b"], np.float32)

    S_vals = sorted(set(plan["S_list"]))
    krons = np.stack([_make_kron(s) for s in S_vals])
    fill2 = np.zeros((2, 256), np.float32)
    fill2[1, :] = -1e30

    WMAP, RW, BMAP, RB = _pack_layout(plan)
    wpack = np.zeros((RW, 384), np.float32)
    bpack = np.zeros((RB, 256), np.float32)

    def wput(name, arr):
        arr = np.atleast_2d(np.asarray(arr, np.float32))
        r0, rows = WMAP[name]
        assert arr.shape[0] == rows, (name, arr.shape)
        wpack[r0 : r0 + rows, : arr.shape[1]] = arr

    def bput(name, arr):
        arr = np.atleast_2d(np.asarray(arr, np.float32))
        r0, rows = BMAP[name]
        assert arr.shape[0] == rows, (name, arr.shape)
        bpack[r0 : r0 + rows, : arr.shape[1]] = arr

    wput("ones_row", np.ones((1, 128), np.float32))
    wput("emb_e_b", np.asarray(inputs["emb_e_b"], np.float32).reshape(1, 128))
    wput("emb_n_b", np.asarray(inputs["emb_n_b"], np.float32).reshape(1, 128))
    bput("emb_e_w", np.asarray(inputs["emb_e_w"], np.float32))
    bput("emb_n_w", np.asarray(inputs["emb_n_w"], np.float32))
    bput("A0", A[0])
    bput("A1", A[1])
    wput("VCB0", np.concatenate([V[0], C[0], B[0]], axis=1))
    wput("V1", V[1])
    wput("U0", U[0])
    wput("U1", U[1])
    wput("WBC", np.concatenate([W0_w[128:256], W0_w[256:384]], axis=1))
    wput("W0a", W0_w[:128])
    wput("W0b_col", np.asarray(inputs["W0_b"], np.float32).reshape(1, 128))
    wput("Wkb0", Wk_b[0].reshape(1, 128))
    wput("Wkb1", Wk_b[1].reshape(1, 128))
    wput("wfb", np.full((1, 1), float(Wf_b), np.float32))
    bput("fill2b", fill2)
    for i, s in enumerate(S_vals):
        bput(f"kronsb{i}", krons[i])
    bput("Wk0", Wk_w[0])
    bput("Wk1", Wk_w[1])
    bput("Wf", Wf_w.reshape(1, 128))

    shared = dict(wpack=wpack, bpack=bpack.astype(BF))

    in_maps = []
    origids = []
    for d in range(NC):
        pc = _per_core_arrays(plan, d, h, e)
        origids.append(pc.pop("origid"))
        m = dict(pc)
        m.update(shared)
        in_maps.append(m)
    return plan, in_maps, origids


def kernel(**inputs):
    import sys

    if "/opt/trn_rl_repo" not in sys.path:
        sys.path.insert(0, "/opt/trn_rl_repo")
    from concourse.bass_utils import run_bass_kernel_spmd

    plan, in_maps, origids = _prep(inputs)
    nc = _build_program(plan)
    res = run_bass_kernel_spmd(nc, in_maps, list(range(NC)))

    E = plan["E"]
    out = np.zeros(E, np.float32)
    for d in range(NC):
        y = np.asarray(res.results[d]["y"]).reshape(-1)
        oid = origids[d]
        valid = oid >= 0
        out[oid[valid]] = y[valid]
    return out



# revision 43
# speedup vs baseline: 2.0651x; 1.4566x over previous
"""GNN message-passing (gated GCN style) on 8 Trainium2 NeuronCores.

Strategy (edge-parallel, dst-sorted shards):
- Host sorts edges by dst and splits into 8 shards snapped to node-run
  boundaries, so each device owns a contiguous node range and its complete
  incoming-edge runs. segment_max is fully local.
- Per device, nodes are sorted by in-degree and each node's run is padded to
  a per-tile power-of-2 slot count S, so segment_max becomes a fixed-window
  reduce_max over contiguous columns (feat-major).
- Per layer, each device computes per-node tables for its own node slice and
  AllGathers them (layer 0: [h@V | h@C] fp32; layer 1 / readout: bf16).
  Per-edge src-side gathers run as batched 512-row dma_gather calls
  (single_packet=False): layer 0 edge-major + PE transposes accumulated in
  PSUM, layer 1 / readout transposing gathers (feat-major, no PE transpose).
  int16 gather indices only span 32K rows, so the 8-shard table is split in
  two 4-shard halves gathered separately and summed; a zeros row absorbs the
  other half, a -1e30 guard row keeps empty-slot max semantics.
- h@B (dst side, sorted) is expanded with a constant kron-pattern matmul.
- BatchNorm statistics are masked sums reduced on-chip and combined with a
  small AllGather + on-chip reduction per layer (AllReduce is ~213us here).
  h-side loops run 4 node-tiles wide; inputs are packed into 6 tensors to
  amortize per-argument dispatch cost.
- The readout MLP runs feat-major per <=512-edge group; h@W0b / h@W0c are
  pre-folded into the final AllGather payload / local table.
"""

import numpy as np

NC = 8
D = 128
MEGA = 4  # max chunks per dma_gather call (HW limit: 512 indices)

USE_STT = True  # scalar_tensor_tensor select in h-side
USE_TTR = False  # tensor_tensor_reduce fused stats — CRASHES HW, keep off
USE_TS = True  # tensor_scalar bias+relu in readout


# ---------------------------------------------------------------------------
# host-side planning
# ---------------------------------------------------------------------------


def _next_pow2(x):
    p = 1
    while p < x:
        p *= 2
    return p


def _pack_layout(plan):
    """Row layout for the packed fp32 [RW,384] and bf16 [RB,256] const
    tensors. Column-vector consts are stored as single rows and
    transposed by the load DMA."""
    S_vals = sorted(set(plan["S_list"]))
    F_N, F_E = plan["F_N"], plan["F_E"]
    w = {}
    r = 0

    def add(name, rows):
        nonlocal r
        w[name] = (r, rows)
        r += rows

    add("ones_row", 1)
    add("emb_e_b", 1)
    add("emb_n_b", 1)
    add("VCB0", 128)
    add("V1", 128)
    add("U0", 128)
    add("U1", 128)
    add("WBC", 128)
    add("W0a", 128)
    add("W0b_col", 1)
    add("Wkb0", 1)
    add("Wkb1", 1)
    add("wfb", 1)
    RW = r
    b = {}
    r = 0

    def addb(name, rows):
        nonlocal r
        b[name] = (r, rows)
        r += rows

    addb("fill2b", 2)
    addb("emb_e_w", F_E)
    addb("emb_n_w", F_N)
    addb("A0", 128)
    addb("A1", 128)
    for i in range(len(S_vals)):
        addb(f"kronsb{i}", 128)
    addb("Wk0", 128)
    addb("Wk1", 128)
    addb("Wf", 1)
    RB = r
    return w, RW, b, RB


def _plan(src, dst, N):
    E = src.shape[0]
    order = np.argsort(dst, kind="stable")
    dsts = dst[order]
    srcs = src[order]

    # shard boundaries snapped to run starts
    bounds = [0]
    for r in range(1, NC):
        t = (E * r) // NC
        b = int(np.searchsorted(dsts, dsts[t], side="left"))
        bounds.append(max(b, bounds[-1]))
    bounds.append(E)

    lo = np.zeros(NC, np.int64)
    for d in range(1, NC):
        lo[d] = int(dsts[bounds[d]]) if bounds[d] < E else N
    hi = np.empty(NC, np.int64)
    hi[:-1] = lo[1:]
    hi[-1] = N

    n_r = [int(hi[d] - lo[d]) for d in range(NC)]
    NODE_CAP = 128 * int(np.ceil((max(n_r) + 2) / 128))
    T = NODE_CAP // 128

    lo = np.asarray(lo)
    shards = []
    for d in range(NC):
        sl = slice(bounds[d], bounds[d + 1])
        dl = dsts[sl] - lo[d]
        cnt = np.bincount(dl, minlength=n_r[d]) if n_r[d] > 0 else np.zeros(0, int)
        starts = np.concatenate([[0], np.cumsum(cnt)])
        perm = np.argsort(-cnt, kind="stable") if n_r[d] > 0 else np.zeros(0, int)
        ipos = np.empty(n_r[d], np.int64)
        ipos[perm] = np.arange(n_r[d])
        shards.append(
            dict(sl=sl, dl=dl, cnt=cnt, starts=starts, perm=perm, ipos=ipos, d=d)
        )

    # shared per-tile slot counts
    S_list = []
    for t in range(T):
        mx = 1
        for sh in shards:
            p = sh["perm"][t * 128 : (t + 1) * 128]
            if len(p):
                c = sh["cnt"][p]
                if len(c):
                    mx = max(mx, int(c.max()))
        S_list.append(min(_next_pow2(mx), 128))

    E_PAD = 128 * int(np.sum(S_list))
    C_E = E_PAD // 128
    chunks = []  # (tile, ci, S)
    for t in range(T):
        for ci in range(S_list[t]):
            chunks.append((t, ci, S_list[t]))
    groups = []  # (c0, gsz) — tile-aligned: all chunks in a group share a tile
    c = 0
    for t in range(T):
        S = S_list[t]
        ci = 0
        while ci < S:
            g = min(4, S - ci)
            groups.append((c + ci, g))
            ci += g
        c += S
    # megas: runs of whole groups, <= MEGA chunks per run
    megas = []  # (cm, nch, [group idx])
    cur = None
    for gi, (c0, gsz) in enumerate(groups):
        if cur is None or cur[1] + gsz > MEGA:
            cur = [c0, 0, []]
            megas.append(cur)
        cur[1] += gsz
        cur[2].append(gi)

    return dict(
        E=E,
        N=N,
        order=order,
        srcs=srcs,
        bounds=bounds,
        lo=np.array(lo),
        hi=np.array(hi),
        n_r=n_r,
        NODE_CAP=NODE_CAP,
        T=T,
        S_list=S_list,
        E_PAD=E_PAD,
        C_E=C_E,
        chunks=chunks,
        groups=groups,
        megas=[tuple(m) for m in megas],
        shards=shards,
    )


def _per_core_arrays(plan, d, h, e):
    """Build padded per-core host arrays for shard d."""
    import ml_dtypes

    BF = ml_dtypes.bfloat16
    sh = plan["shards"][d]
    NODE_CAP, T = plan["NODE_CAP"], plan["T"]
    S_list = plan["S_list"]
    E_PAD, C_E = plan["E_PAD"], plan["C_E"]
    n_r = plan["n_r"][d]
    guard_row = d * NODE_CAP + (NODE_CAP - 1)

    e_sh = e[plan["order"]][sh["sl"]]  # [E_s, F_E]
    src_sh = plan["srcs"][sh["sl"]]
    orig_sh = np.arange(plan["E"])[plan["order"]][sh["sl"]]

    F_E = e.shape[1]
    e0_pad = np.zeros((E_PAD, F_E), np.float32)
    srcrow = np.full(E_PAD, guard_row, np.int64)
    maskf = np.zeros(E_PAD, np.float32)
    origid = np.full(E_PAD, -1, np.int64)

    base = 0
    perm = sh["perm"]
    cnt = sh["cnt"]
    starts = sh["starts"]
    rank_of = lambda g: np.clip(
        np.searchsorted(plan["lo"], g, side="right") - 1, 0, NC - 1
    )
    for t in range(T):
        S = S_list[t]
        pn = perm[t * 128 : (t + 1) * 128]
        # index matrix [128, S] of local edge positions, -1 = dummy
        im = np.full((128, S), -1, np.int64)
        for i, n in enumerate(pn):
            dg = int(cnt[n])
            k = min(dg, S)
            if k:
                im[i, :k] = np.arange(starts[n], starts[n] + k)
        flat = im.reshape(-1)
        real = flat >= 0
        fr = flat[real]
        blk = slice(base, base + 128 * S)
        e0_blk = np.zeros((128 * S, F_E), np.float32)
        e0_blk[real] = e_sh[fr]
        e0_pad[blk] = e0_blk
        sr = np.full(128 * S, guard_row, np.int64)
        g = src_sh[fr]
        r = rank_of(g)
        loc = g - plan["lo"][r]
        pp_ = np.empty(len(g), np.int64)
        for rr in np.unique(r):
            m = r == rr
            pp_[m] = plan["shards"][rr]["ipos"][loc[m]]
        sr[real] = r * NODE_CAP + pp_
        srcrow[blk] = sr
        mk = np.zeros(128 * S, np.float32)
        mk[real] = 1.0
        maskf[blk] = mk
        oi = np.full(128 * S, -1, np.int64)
        oi[real] = orig_sh[fr]
        origid[blk] = oi
        base += 128 * S

    # edge slot i (= c*128+p) -> srcrow; the slot order the e-side pipeline
    # uses IS this flat order.  dma_gather consumes indices 16-wrapped:
    # idx[q, j] = slot j*16+q, replicated over the 8 16-partition blocks.
    HALF = 4 * NODE_CAP
    memberA = srcrow < HALF
    idxA = np.where(memberA, srcrow, NODE_CAP - 2).astype(np.int16)
    idxB = np.where(~memberA, srcrow - HALF, NODE_CAP - 2).astype(np.int16)

    def wrap16(a):
        w = a.reshape(E_PAD // 16, 16).T  # [16, E_PAD/16]
        return np.ascontiguousarray(np.tile(w, (8, 1)))

    ipack = np.concatenate([wrap16(idxA), wrap16(idxB)], axis=1)

    # mpack: mask_e rows [NG, 512] then nodemask rows [NW, 512] (4 tiles/row)
    NG = len(plan["groups"])
    NW = (T + 3) // 4
    mpack = np.zeros((NG + NW, 512), np.float32)
    for gi, (c0, gsz) in enumerate(plan["groups"]):
        mpack[gi, : gsz * 128] = maskf[c0 * 128 : (c0 + gsz) * 128]
    nm = np.zeros(NW * 512, np.float32)
    nm[:n_r] = 1.0
    mpack[NG:, :] = nm.reshape(NW, 512)
    # h0T [F_N, NODE_CAP] permuted
    F_N = h.shape[1]
    h0p = np.zeros((NODE_CAP, F_N), np.float32)
    hl = h[plan["lo"][d] : plan["hi"][d]]
    h0p[: len(perm)] = hl[perm]
    h0T = np.ascontiguousarray(h0p.T)

    return dict(
        h0T=h0T.astype(BF),
        e0T=np.ascontiguousarray(e0_pad.T).astype(BF),
        ipack=ipack,
        mpack=mpack,
        origid=origid,
    )


# ---------------------------------------------------------------------------
# device program
# ---------------------------------------------------------------------------


def _build_program(plan, stop_after="full", epochs=1):
    import concourse.bass as bass
    import concourse.mybir as mybir
    import concourse.tile as tile
    from concourse import bacc

    F32 = mybir.dt.float32
    BF16 = mybir.dt.bfloat16
    I16 = mybir.dt.int16
    AF = mybir.ActivationFunctionType
    OP = mybir.AluOpType
    AX = mybir.AxisListType

    NODE_CAP, T = plan["NODE_CAP"], plan["T"]
    E_PAD, C_E = plan["E_PAD"], plan["C_E"]
    chunks, groups, megas = plan["chunks"], plan["groups"], plan["megas"]
    NG = len(groups)
    N, E = plan["N"], plan["E"]
    S_vals = sorted(set(plan["S_list"]))
    kron_of = {s: i for i, s in enumerate(S_vals)}
    F_N, F_E = plan["F_N"], plan["F_E"]
    EPS = 1e-5
    HALF = 4 * NODE_CAP
    NI16 = E_PAD // 16

    _phases = ["embed", "bound0", "epass0", "layer0", "layer1", "full"]
    lvl = _phases.index(stop_after)

    nc = bacc.Bacc(
        "TRN2", target_bir_lowering=False, debug=False, num_devices=NC
    )

    def din(name, shape, dt=F32):
        return nc.dram_tensor(name, shape, dt, kind="ExternalInput")

    # per-core inputs
    WMAP, RW, BMAP, RB = _pack_layout(plan)
    h0T = din("h0T", [F_N, NODE_CAP], BF16)
    e0T = din("e0T", [F_E, E_PAD], BF16)
    ipack_d = din("ipack", [128, 2 * NI16], I16)
    NW = (T + 3) // 4
    mpack_d = din("mpack", [NG + NW, 512])
    wpack_d = din("wpack", [RW, 384])
    bpack_d = din("bpack", [RB, 256], BF16)

    y_out = nc.dram_tensor("y", [1, E_PAD], F32, kind="ExternalOutput")

    rg = [list(range(NC))]

    with tile.TileContext(nc) as tc:
        with (
            tc.tile_pool(name="const", bufs=1) as cp,
            tc.tile_pool(name="pers", bufs=1) as pp,
            tc.tile_pool(name="st", bufs=1) as stp,
            tc.tile_pool(name="s", bufs=2) as sp,
            tc.tile_pool(name="ps", bufs=2, space="PSUM") as ps,
            tc.tile_pool(name="dram", bufs=1, space="DRAM") as dp,
        ):
            # ---- load constants from packs
            def wload(name, width, dt=F32, pack=None, pmap=None):
                pk = pack if pack is not None else wpack_d
                mp = pmap if pmap is not None else WMAP
                r0, rows = mp[name]
                t = cp.tile([rows, width], dt, name=f"{name}_sb")
                nc.sync.dma_start(out=t[:], in_=pk[r0 : r0 + rows, :width])
                return t

            def wload_col(name, dt=F32, pack=None, pmap=None):
                pk = pack if pack is not None else wpack_d
                mp = pmap if pmap is not None else WMAP
                r0, rows = mp[name]
                t = cp.tile([128, 1], dt, name=f"{name}_sb")
                nc.sync.dma_start(
                    out=t[:], in_=pk[r0 : r0 + 1, :128].rearrange("a p -> p a")
                )
                return t

            def bload(name, width, dt=BF16):
                return wload(name, width, dt, pack=bpack_d, pmap=BMAP)

            ones_row = wload("ones_row", 128)
            embe_w = bload("emb_e_w", 128)
            embn_w = bload("emb_n_w", 128)
            embe_b = wload_col("emb_e_b")
            embn_b = wload_col("emb_n_b")
            A_sb = [bload(f"A{l}", 128) for l in range(2)]
            VCB0 = wload("VCB0", 384)
            V1 = wload("V1", 128)
            U_sb = [wload(f"U{l}", 128) for l in range(2)]
            WBC = wload("WBC", 256)
            W0a = wload("W0a", 128)
            W0bc = wload_col("W0b_col")
            Wk = [bload(f"Wk{k}", 128) for k in range(2)]
            Wkb = [wload_col(f"Wkb{k}") for k in range(2)]
            Wfr0, _ = BMAP["Wf"]
            Wf = cp.tile([128, 1], BF16, name="Wf_sb")
            nc.sync.dma_start(
                out=Wf[:], in_=bpack_d[Wfr0 : Wfr0 + 1, :128].rearrange("a p -> p a")
            )
            wfb = wload("wfb", 1)
            kron_bb = [bload(f"kronsb{i}", 128) for i in range(len(S_vals))]
            ipack = cp.tile([128, 2 * NI16], I16, name="ipack_sb")
            nc.sync.dma_start(out=ipack[:], in_=ipack_d[:])
            eps_col = cp.tile([128, 1], F32, name="eps_col")
            nc.gpsimd.memset(eps_col[:], EPS)

            # ---- dram buffers
            e_buf = dp.tile([128, E_PAD], BF16, name="e_buf")
            z_buf = dp.tile([128, E_PAD], BF16, name="z_buf")
            w_buf = dp.tile([128, E_PAD], BF16, name="w_buf")
            hb_buf = dp.tile([NODE_CAP, 128], BF16, name="hb_buf")
            hfm_buf = [
                dp.tile([128, NODE_CAP], F32, name=f"hfm_buf{i}")
                for i in range(3)
            ]
            hlocal = dp.tile([NODE_CAP, 128], BF16, name="hlocal")
            # l=0: row-major [node, 256] for HBM gathers.
            # l=1,2: partition-major [128, NODE_CAP] so the AllGather output
            # loads into SBUF in the dma_gather token/rank-stripe layout.
            cc_hin = [
                dp.tile(
                    [NODE_CAP, 256] if l == 0 else [128, NODE_CAP],
                    BF16,
                    name=f"cc_hin{l}",
                )
                for l in range(3)
            ]
            cc_hout_ep = [
                [
                    dp.tile(
                        [NC * NODE_CAP, 256] if l == 0 else [NC * 128, NODE_CAP],
                        BF16,
                        name=f"cc_hout{l}_e{e_}",
                        addr_space="Shared",
                    )
                    for l in range(3)
                ]
                for e_ in range(epochs)
            ]
            cc_st_in = [
                dp.tile([128, 4 if l == 0 else 2], F32, name=f"cc_st_in{l}")
                for l in range(2)
            ]
            cc_st_out_ep = [
                [
                    dp.tile(
                        [NC * 128, 4 if l == 0 else 2],
                        F32,
                        name=f"cc_st_out{l}_e{e_}",
                        addr_space="Shared",
                    )
                    for l in range(2)
                ]
                for e_ in range(epochs)
            ]
            cc_moy_in = dp.tile([128, 1], F32, name="cc_moy_in")
            cc_moy_out_ep = [
                dp.tile(
                    [NC * 128, 1], F32, name=f"cc_moy_out_e{e_}",
                    addr_space="Shared",
                )
                for e_ in range(epochs)
            ]

            def _epoch(ep):
                cc_hout = cc_hout_ep[ep]
                cc_st_out = cc_st_out_ep[ep]
                cc_moy_out = cc_moy_out_ep[ep]
                # ---- persistent sbuf tiles
                agg = [None] * NW

                def load_tab(l, uniq):
                    """Load the partition-major AllGather output for layer l
                    into the shared SBUF table tile (token/rank layout)."""
                    tab = pp.tile(
                        [128, NC * NODE_CAP], BF16, tag="tab",
                        name=f"tab_{uniq}_{ep}",
                    )
                    for d in range(NC):
                        nc.sync.dma_start(
                            out=tab[:, d * NODE_CAP : (d + 1) * NODE_CAP],
                            in_=cc_hout[l][d * 128 : (d + 1) * 128, :],
                        )
                    return tab

                def group_gather_sb(c0, gsz, tab):
                    """Dual transposing SBUF-source gathers + merge:
                    gv[p, k*128 + q] = sum_half tab_half[idx[(c0+k)*128+q], p]."""
                    n = gsz * 128
                    ga = sp.tile([128, 1024], BF16, tag="gta", bufs=2)
                    gb = sp.tile([128, 1024], BF16, tag="gtb", bufs=2)
                    for g, ioff, e0 in ((ga, 0, 0), (gb, NI16, HALF)):
                        nc.gpsimd.dma_gather(
                            g[:, :n].rearrange("p (j i) -> p j i", i=n),
                            tab[:, e0 : e0 + HALF],
                            ipack[:, ioff + c0 * 8 : ioff + (c0 + gsz) * 8],
                            n,
                            n,
                            128,
                            transpose=True,
                            single_packet=False,
                            sbuf_tokens_per_rank=128,
                            sbuf_free_dim_per_rank=256,
                        )
                    nc.vector.tensor_tensor(
                        out=ga[:, :n], in0=ga[:, :n], in1=gb[:, :n], op=OP.add
                    )
                    return ga

                def group_gather_t2(c0, gsz):
                    """Dual transposing gathers (bf16 feat-major, 256-wide
                    table) + merge: gv[p, 0*n+i] = V-part feat p of slot i,
                    gv[p, 1*n+i] = C-part feat p of slot i."""
                    n = gsz * 128
                    ga = sp.tile([128, 1024], BF16, tag="gta", bufs=2)
                    gb = sp.tile([128, 1024], BF16, tag="gtb", bufs=2)
                    for g, ioff, r0 in ((ga, 0, 0), (gb, NI16, HALF)):
                        nc.gpsimd.dma_gather(
                            g[:, : 2 * n].rearrange("p (j i) -> p j i", i=n),
                            cc_hout[0][r0 : r0 + HALF, :],
                            ipack[:, ioff + c0 * 8 : ioff + (c0 + gsz) * 8],
                            n,
                            n,
                            256,
                            transpose=True,
                            single_packet=False,
                        )
                    nc.vector.tensor_tensor(
                        out=ga[:, : 2 * n], in0=ga[:, : 2 * n],
                        in1=gb[:, : 2 * n], op=OP.add,
                    )
                    return ga

                # ================= embed h =================
                for t in range(T):
                    h0sl = sp.tile([F_N, 128], BF16, tag="h0sl", bufs=2, name="h0sl")
                    nc.sync.dma_start(
                        out=h0sl[:], in_=h0T[:, t * 128 : (t + 1) * 128]
                    )
                    ph = ps.tile([128, 128], F32, tag="pc")
                    nc.tensor.matmul(
                        out=ph[:], lhsT=embn_w[:], rhs=h0sl[:],
                        start=True, stop=True,
                    )
                    hf = sp.tile([128, 128], F32, tag="hnew", bufs=4, name="hemb")
                    nc.scalar.activation(
                        out=hf[:], in_=ph[:], func=AF.Identity, bias=embn_b[:]
                    )
                    nc.sync.dma_start(
                        out=hfm_buf[0][:, t * 128 : (t + 1) * 128], in_=hf[:]
                    )

                # ================= boundary =================
                def boundary(l):
                    """Build tables for layer l (or readout if l==2) from hfm."""
                    if l == 0:
                        rhs, wdt = VCB0, 384
                    elif l == 1:
                        rhs, wdt = V1, 128
                    else:
                        rhs, wdt = WBC, 256
                    scat_w = 256 if l == 0 else 128
                    bdt = BF16
                    for t in range(T):
                        hfl = sp.tile([128, 128], F32, tag="hfl", bufs=4, name="hfl")
                        nc.sync.dma_start(
                            out=hfl[:], in_=hfm_buf[l][:, t * 128 : (t + 1) * 128]
                        )
                        pb = ps.tile([128, 512], F32, tag="pa")
                        nc.tensor.matmul(
                            out=pb[:, :wdt], lhsT=hfl[:], rhs=rhs[:],
                            start=True, stop=True,
                        )
                        bsb = sp.tile([128, 512], bdt, tag="bsbb", bufs=2)
                        nc.scalar.activation(
                            out=bsb[:, :wdt], in_=pb[:, :wdt], func=AF.Copy
                        )
                        if l == 0:
                            nc.sync.dma_start(
                                out=cc_hin[l][t * 128 : (t + 1) * 128, :scat_w],
                                in_=bsb[:, :scat_w],
                            )
                        else:
                            nc.sync.dma_start(
                                out=cc_hin[l][:, t * 128 : (t + 1) * 128],
                                in_=bsb[:, :128],
                            )
                        if l == 0:
                            nc.sync.dma_start(
                                out=hb_buf[t * 128 : (t + 1) * 128, :],
                                in_=bsb[:, 256:384],
                            )
                        if l == 2:
                            nc.sync.dma_start(
                                out=hlocal[t * 128 : (t + 1) * 128, :],
                                in_=bsb[:, 128:256],
                            )
                    # zero row (NODE_CAP-2) + guard row (NODE_CAP-1)
                    fr0, _ = BMAP["fill2b"]
                    if l == 0:
                        nc.sync.dma_start(
                            out=cc_hin[l][NODE_CAP - 2 : NODE_CAP, :scat_w],
                            in_=bpack_d[fr0 : fr0 + 2, :scat_w],
                        )
                    else:
                        nc.sync.dma_start(
                            out=cc_hin[l][126:128, NODE_CAP - 128 : NODE_CAP],
                            in_=bpack_d[fr0 : fr0 + 2, :128],
                        )
                    nc.gpsimd.collective_compute(
                        "AllGather",
                        OP.bypass,
                        replica_groups=rg,
                        ins=[cc_hin[l][:]],
                        outs=[cc_hout[l][:]],
                    )

                if lvl >= 1:
                    boundary(0)

                # ================= embed e =================
                for gi, (c0, gsz) in enumerate(groups):
                    w = gsz * 128
                    e0sl = sp.tile([F_E, 512], BF16, tag="e0sl")
                    nc.sync.dma_start(
                        out=e0sl[:, :w], in_=e0T[:, c0 * 128 : c0 * 128 + w]
                    )
                    pe = ps.tile([128, 512], F32, tag="pa")
                    nc.tensor.matmul(
                        out=pe[:, :w], lhsT=embe_w[:], rhs=e0sl[:, :w],
                        start=True, stop=True,
                    )
                    esb = sp.tile([128, 512], BF16, tag="esb", bufs=2)
                    nc.scalar.activation(
                        out=esb[:, :w], in_=pe[:, :w], func=AF.Identity,
                        bias=embe_b[:],
                    )
                    nc.sync.dma_start(
                        out=e_buf[:, c0 * 128 : c0 * 128 + w], in_=esb[:, :w]
                    )


                # ================= layers =================
                for l in range(2):
                    if l == 0 and lvl < 2:
                        break
                    if l == 1 and lvl < 4:
                        break
                    # stats accumulators
                    if l == 0:
                        ssum_e = stp.tile([128, NG], F32, name=f"ssum_e{ep}")
                        ssq_e = stp.tile([128, NG], F32, name=f"ssq_e{ep}")
                    hsum = stp.tile([128, NW], F32, name=f"hsum{l}_{ep}")
                    hssq = stp.tile([128, NW], F32, name=f"hssq{l}_{ep}")

                    # ---- e-pass (layer 1's is fused into the l==0 e-update)
                    for gi, (c0, gsz) in enumerate(groups if l == 0 else []):
                        if True:
                            w = gsz * 128
                            t, ci0, S = chunks[c0]
                            G = 128 // S
                            gv = group_gather_t2(c0, gsz)
                            esb = sp.tile([128, 512], BF16, tag="esb", bufs=2)
                            nc.sync.dma_start(
                                out=esb[:, :w],
                                in_=e_buf[:, c0 * 128 : c0 * 128 + w],
                            )
                            wsb = sp.tile([128, 512], BF16, tag="wsb", bufs=2)
                            nc.scalar.activation(
                                out=wsb[:, :w], in_=esb[:, :w], func=AF.Sigmoid
                            )
                            mrow = sp.tile([1, 512], F32, tag="mrow", bufs=1)
                            nc.sync.dma_start(out=mrow[:], in_=mpack_d[gi : gi + 1, :])
                            pm = ps.tile([128, 512], F32, tag="pb")
                            nc.tensor.matmul(
                                out=pm[:, :w], lhsT=ones_row[:], rhs=mrow[:, :w],
                                start=True, stop=True,
                            )
                            pz = ps.tile([128, 512], F32, tag="pa")
                            nc.tensor.matmul(
                                out=pz[:, :w], lhsT=A_sb[l][:], rhs=esb[:, :w],
                                start=True, stop=False, skip_group_check=True,
                            )
                            band = sp.tile([128, 512], BF16, tag="hbt", bufs=2, name="band")
                            nc.sync.dma_start(
                                out=band[:G, : gsz * 128].rearrange(
                                    "p (k c) -> p k c", c=128
                                ),
                                in_=hb_buf[
                                    t * 128 + ci0 * G : t * 128 + (ci0 + gsz) * G, :
                                ].rearrange("(k p) c -> p k c", p=G),
                            )
                            for k in range(gsz):
                                nc.tensor.matmul(
                                    out=pz[:, k * 128 : (k + 1) * 128],
                                    lhsT=band[:G, k * 128 : (k + 1) * 128],
                                    rhs=kron_bb[kron_of[S]][:G, :],
                                    start=False, stop=(k == gsz - 1),
                                    skip_group_check=True,
                                )
                            # message path: msg = (hV0)[src] * w, windowed max
                            msg = sp.tile([128, 512], BF16, tag="msg", bufs=2)
                            nc.vector.tensor_tensor(
                                out=msg[:, :w], in0=gv[:, :w],
                                in1=wsb[:, :w], op=OP.mult,
                            )
                            for k in range(gsz):
                                ci = ci0 + k
                                if ci == 0 and t % 4 == 0:
                                    agg[t // 4] = pp.tile(
                                        [128, 512], BF16, tag=f"aggw{t // 4}",
                                        name=f"aggw_{t // 4}_0_{ep}",
                                    )
                                ao = (t % 4) * 128
                                nc.vector.tensor_reduce(
                                    out=agg[t // 4][
                                        :, ao + ci * G : ao + (ci + 1) * G
                                    ],
                                    in_=msg[:, k * 128 : (k + 1) * 128].rearrange(
                                        "p (g s) -> p g s", s=S
                                    ),
                                    op=OP.max,
                                    axis=AX.X,
                                )
                            # z path: z = A e + (C h)[src] + kron(B h local)
                            zsum = sp.tile([128, 512], BF16, tag="zsum", bufs=2)
                            nc.vector.tensor_tensor(
                                out=zsum[:, :w], in0=pz[:, :w],
                                in1=gv[:, w : 2 * w], op=OP.add,
                            )
                            zm = sp.tile([128, 512], BF16, tag="zm", bufs=2)
                            if USE_TTR:
                                nc.vector.tensor_tensor_reduce(
                                    out=zm[:, :w], in0=zsum[:, :w], in1=pm[:, :w],
                                    scale=1.0, scalar=0.0, op0=OP.mult, op1=OP.add,
                                    accum_out=ssum_e[:, gi : gi + 1],
                                )
                                sq = sp.tile([128, 512], BF16, tag="sq", bufs=2)
                                nc.vector.tensor_tensor_reduce(
                                    out=sq[:, :w], in0=zm[:, :w], in1=zm[:, :w],
                                    scale=1.0, scalar=0.0, op0=OP.mult, op1=OP.add,
                                    accum_out=ssq_e[:, gi : gi + 1],
                                )
                            else:
                                nc.vector.tensor_tensor(
                                    out=zm[:, :w], in0=zsum[:, :w], in1=pm[:, :w],
                                    op=OP.mult,
                                )
                                nc.vector.tensor_reduce(
                                    out=ssum_e[:, gi : gi + 1], in_=zm[:, :w],
                                    op=OP.add, axis=AX.X,
                                )
                                sq = sp.tile([128, 512], F32, tag="sq", bufs=2)
                                nc.scalar.activation(
                                    out=sq[:, :w], in_=zm[:, :w], func=AF.Square
                                )
                                nc.vector.tensor_reduce(
                                    out=ssq_e[:, gi : gi + 1], in_=sq[:, :w],
                                    op=OP.add, axis=AX.X,
                                )
                            nc.sync.dma_start(
                                out=z_buf[:, c0 * 128 : c0 * 128 + w],
                                in_=zm[:, :w],
                            )

                    if l == 0 and lvl < 3:
                        break

                    # ---- h side: z_h = hU + select(agg); masked stats (wide)
                    for j in range(NW):
                        wj = min(512, (T - 4 * j) * 128)
                        if USE_STT:
                            nc.vector.scalar_tensor_tensor(
                                out=agg[j][:, :wj], in0=agg[j][:, :wj],
                                scalar=-1e20, in1=agg[j][:, :wj],
                                op0=OP.is_gt, op1=OP.mult,
                            )
                        else:
                            m01 = sp.tile([128, 512], F32, tag="zhm", bufs=3)
                            nc.vector.tensor_scalar(
                                out=m01[:, :wj], in0=agg[j][:, :wj],
                                scalar1=-1e20, scalar2=None, op0=OP.is_gt,
                            )
                            nc.vector.tensor_tensor(
                                out=agg[j][:, :wj], in0=agg[j][:, :wj],
                                in1=m01[:, :wj], op=OP.mult,
                            )
                        # recompute hU = (U h)^T for this window into PSUM
                        pu = ps.tile([128, 512], F32, tag="pc")
                        for q in range(min(4, T - 4 * j)):
                            t_ = 4 * j + q
                            hflq = sp.tile(
                                [128, 128], F32, tag="hfl", bufs=4, name="hflq"
                            )
                            nc.sync.dma_start(
                                out=hflq[:],
                                in_=hfm_buf[l][:, t_ * 128 : (t_ + 1) * 128],
                            )
                            nc.tensor.matmul(
                                out=pu[:, q * 128 : (q + 1) * 128],
                                lhsT=U_sb[l][:], rhs=hflq[:],
                                start=True, stop=True, skip_group_check=True,
                            )
                        nc.vector.tensor_tensor(
                            out=agg[j][:, :wj], in0=agg[j][:, :wj],
                            in1=pu[:, :wj], op=OP.add,
                        )
                        nmr = sp.tile([1, 512], F32, tag="nmr", bufs=1)
                        nc.sync.dma_start(
                            out=nmr[:], in_=mpack_d[NG + j : NG + j + 1, :]
                        )
                        pnm = ps.tile([128, 512], F32, tag="pc")
                        nc.tensor.matmul(
                            out=pnm[:, :wj], lhsT=ones_row[:], rhs=nmr[:, :wj],
                            start=True, stop=True, skip_group_check=True,
                        )
                        zhm = sp.tile([128, 512], F32, tag="zhm", bufs=3)
                        if USE_TTR:
                            nc.vector.tensor_tensor_reduce(
                                out=zhm[:, :wj], in0=agg[j][:, :wj],
                                in1=pnm[:, :wj], scale=1.0, scalar=0.0,
                                op0=OP.mult, op1=OP.add,
                                accum_out=hsum[:, j : j + 1],
                            )
                            sqh = sp.tile([128, 512], F32, tag="zhm", bufs=3)
                            nc.vector.tensor_tensor_reduce(
                                out=sqh[:, :wj], in0=zhm[:, :wj], in1=zhm[:, :wj],
                                scale=1.0, scalar=0.0, op0=OP.mult, op1=OP.add,
                                accum_out=hssq[:, j : j + 1],
                            )
                        else:
                            nc.vector.tensor_tensor(
                                out=zhm[:, :wj], in0=agg[j][:, :wj],
                                in1=pnm[:, :wj], op=OP.mult,
                            )
                            nc.vector.tensor_reduce(
                                out=hsum[:, j : j + 1], in_=zhm[:, :wj],
                                op=OP.add, axis=AX.X,
                            )
                            sqh = sp.tile([128, 512], F32, tag="zhm", bufs=3)
                            nc.scalar.activation(
                                out=sqh[:, :wj], in_=zhm[:, :wj], func=AF.Square
                            )
                            nc.vector.tensor_reduce(
                                out=hssq[:, j : j + 1], in_=sqh[:, :wj],
                                op=OP.add, axis=AX.X,
                            )

                    # ---- pack + allreduce stats
                    ncols = 4 if l == 0 else 2
                    pack = stp.tile([128, 4], F32, name=f"pack{l}_{ep}")
                    nc.vector.tensor_reduce(
                        out=pack[:, 0:1], in_=hsum[:], op=OP.add, axis=AX.X
                    )
                    nc.vector.tensor_reduce(
                        out=pack[:, 1:2], in_=hssq[:], op=OP.add, axis=AX.X
                    )
                    if l == 0:
                        nc.vector.tensor_reduce(
                            out=pack[:, 2:3], in_=ssum_e[:], op=OP.add, axis=AX.X
                        )
                        nc.vector.tensor_reduce(
                            out=pack[:, 3:4], in_=ssq_e[:], op=OP.add, axis=AX.X
                        )
                    nc.sync.dma_start(out=cc_st_in[l][:], in_=pack[:, :ncols])
                    nc.gpsimd.collective_compute(
                        "AllGather", OP.bypass, replica_groups=rg,
                        ins=[cc_st_in[l][:]], outs=[cc_st_out[l][:]],
                    )
                    gat = stp.tile([128, 4 * NC], F32, name=f"gat{l}_{ep}")
                    nc.sync.dma_start(
                        out=gat[:, : ncols * NC].rearrange("p (f c) -> p f c", c=NC),
                        in_=cc_st_out[l][:].rearrange("(c p) f -> p f c", p=128),
                    )
                    stt = stp.tile([128, 4], F32, name=f"stt{l}_{ep}")
                    nc.vector.tensor_reduce(
                        out=stt[:, :ncols].rearrange("p (f x) -> p f x", x=1),
                        in_=gat[:, : ncols * NC].rearrange("p (f c) -> p f c", c=NC),
                        op=OP.add, axis=AX.X,
                    )

                    # ---- bn coefficients
                    def bn_cols(sum_c, ssq_c, count, pref):
                        mean = stp.tile([128, 1], F32, name=f"{pref}mean{l}_{ep}")
                        nc.vector.tensor_scalar(
                            out=mean[:], in0=sum_c, scalar1=1.0 / count,
                            scalar2=None, op0=OP.mult,
                        )
                        msq = stp.tile([128, 1], F32, name=f"{pref}msq{l}_{ep}")
                        nc.vector.tensor_scalar(
                            out=msq[:], in0=ssq_c, scalar1=1.0 / count,
                            scalar2=None, op0=OP.mult,
                        )
                        m2 = stp.tile([128, 1], F32, name=f"{pref}m2{l}_{ep}")
                        nc.scalar.activation(out=m2[:], in_=mean[:], func=AF.Square)
                        var = stp.tile([128, 1], F32, name=f"{pref}var{l}_{ep}")
                        nc.vector.tensor_tensor(
                            out=var[:], in0=msq[:], in1=m2[:], op=OP.subtract
                        )
                        sd = stp.tile([128, 1], F32, name=f"{pref}sd{l}_{ep}")
                        nc.scalar.activation(
                            out=sd[:], in_=var[:], func=AF.Sqrt, bias=eps_col[:]
                        )
                        rs = stp.tile([128, 1], F32, name=f"{pref}rs{l}_{ep}")
                        nc.vector.reciprocal(out=rs[:], in_=sd[:])
                        bb = stp.tile([128, 1], F32, name=f"{pref}bb{l}_{ep}")
                        nc.vector.tensor_tensor(
                            out=bb[:], in0=mean[:], in1=rs[:], op=OP.mult
                        )
                        nc.vector.tensor_scalar(
                            out=bb[:], in0=bb[:], scalar1=-1.0, scalar2=None,
                            op0=OP.mult,
                        )
                        return rs, bb

                    rs_h, bb_h = bn_cols(stt[:, 0:1], stt[:, 1:2], N, "h")
                    if l == 0:
                        rs_e, bb_e = bn_cols(stt[:, 2:3], stt[:, 3:4], E, "e")

                    # ---- h update (wide)
                    for j in range(NW):
                        wj = min(512, (T - 4 * j) * 128)
                        r = sp.tile([128, 512], F32, tag="rh", bufs=2)
                        nc.scalar.activation(
                            out=r[:, :wj], in_=agg[j][:, :wj], func=AF.Relu,
                            bias=bb_h[:], scale=rs_h[:],
                        )
                        hfl = sp.tile([128, 512], F32, tag="hflw", bufs=2, name="hflu")
                        nc.sync.dma_start(
                            out=hfl[:, :wj],
                            in_=hfm_buf[l][:, j * 512 : j * 512 + wj],
                        )
                        hf2 = sp.tile([128, 512], F32, tag="hneww", bufs=2, name="hupd")
                        nc.vector.tensor_tensor(
                            out=hf2[:, :wj], in0=hfl[:, :wj], in1=r[:, :wj],
                            op=OP.add,
                        )
                        nc.sync.dma_start(
                            out=hfm_buf[l + 1][:, j * 512 : j * 512 + wj],
                            in_=hf2[:, :wj],
                        )

                    boundary(l + 1)

                    # ---- e update (pass 1, overlaps AllGather 1): w1 -> w_buf
                    if l == 0:
                        for gi, (c0, gsz) in enumerate(groups):
                            if True:
                                w = gsz * 128
                                zsb = sp.tile([128, 512], BF16, tag="zsb", bufs=2)
                                nc.sync.dma_start(
                                    out=zsb[:, :w],
                                    in_=z_buf[:, c0 * 128 : c0 * 128 + w],
                                )
                                r = sp.tile([128, 512], BF16, tag="msg", bufs=2)
                                nc.scalar.activation(
                                    out=r[:, :w], in_=zsb[:, :w], func=AF.Relu,
                                    bias=bb_e[:], scale=rs_e[:],
                                )
                                eold = sp.tile([128, 512], BF16, tag="esb", bufs=2)
                                nc.sync.dma_start(
                                    out=eold[:, :w],
                                    in_=e_buf[:, c0 * 128 : c0 * 128 + w],
                                )
                                enew = sp.tile([128, 512], BF16, tag="zm", bufs=2)
                                nc.vector.tensor_tensor(
                                    out=enew[:, :w], in0=eold[:, :w], in1=r[:, :w],
                                    op=OP.add,
                                )
                                w1 = sp.tile([128, 512], BF16, tag="wsb", bufs=2)
                                nc.scalar.activation(
                                    out=w1[:, :w], in_=enew[:, :w], func=AF.Sigmoid
                                )
                                nc.sync.dma_start(
                                    out=w_buf[:, c0 * 128 : c0 * 128 + w],
                                    in_=w1[:, :w],
                                )
                        # ---- pass 2: layer-1 messages (needs AllGather 1)
                        tab1 = load_tab(1, "l1")
                        for gi, (c0, gsz) in enumerate(groups):
                            if True:
                                w = gsz * 128
                                t, ci0, S = chunks[c0]
                                G = 128 // S
                                gv1 = group_gather_sb(c0, gsz, tab1)
                                w1sb = sp.tile([128, 512], BF16, tag="sq", bufs=2)
                                nc.sync.dma_start(
                                    out=w1sb[:, :w],
                                    in_=w_buf[:, c0 * 128 : c0 * 128 + w],
                                )
                                msgg = sp.tile([128, 512], BF16, tag="zsum", bufs=2)
                                nc.vector.tensor_tensor(
                                    out=msgg[:, :w], in0=gv1[:, :w],
                                    in1=w1sb[:, :w], op=OP.mult,
                                )
                                for k in range(gsz):
                                    ci = ci0 + k
                                    if ci == 0 and t % 4 == 0:
                                        agg[t // 4] = pp.tile(
                                            [128, 512], BF16, tag=f"aggw{t // 4}",
                                            name=f"aggw_{t // 4}_1_{ep}",
                                        )
                                    ao = (t % 4) * 128
                                    nc.vector.tensor_reduce(
                                        out=agg[t // 4][
                                            :, ao + ci * G : ao + (ci + 1) * G
                                        ],
                                        in_=msgg[:, k * 128 : (k + 1) * 128].rearrange(
                                            "p (g s) -> p g s", s=S
                                        ),
                                        op=OP.max,
                                        axis=AX.X,
                                    )

            # ================= moy + base =================
                if lvl < 5:
                    ydummy = sp.tile([1, 4096], F32, tag="ydummy", bufs=1)
                    nc.gpsimd.memset(ydummy[:], 0.0)
                    for c0 in range(0, E_PAD, 4096):
                        w = min(4096, E_PAD - c0)
                        nc.sync.dma_start(
                            out=y_out[0:1, c0 : c0 + w], in_=ydummy[:, :w]
                        )
                else:
                    moysum = stp.tile([128, NW], F32, name=f"moysum{ep}")
                    for j in range(NW):
                        wj = min(512, (T - 4 * j) * 128)
                        nmr = sp.tile([1, 512], F32, tag="nmr", bufs=1)
                        nc.sync.dma_start(
                            out=nmr[:], in_=mpack_d[NG + j : NG + j + 1, :]
                        )
                        pnm = ps.tile([128, 512], F32, tag="pc")
                        nc.tensor.matmul(
                            out=pnm[:, :wj], lhsT=ones_row[:], rhs=nmr[:, :wj],
                            start=True, stop=True, skip_group_check=True,
                        )
                        hfl = sp.tile([128, 512], F32, tag="hflw", bufs=2, name="hflm")
                        nc.sync.dma_start(
                            out=hfl[:, :wj],
                            in_=hfm_buf[2][:, j * 512 : j * 512 + wj],
                        )
                        hm = sp.tile([128, 512], F32, tag="zhm", bufs=3)
                        nc.vector.tensor_tensor(
                            out=hm[:, :wj], in0=hfl[:, :wj], in1=pnm[:, :wj],
                            op=OP.mult,
                        )
                        nc.vector.tensor_reduce(
                            out=moysum[:, j : j + 1], in_=hm[:, :wj], op=OP.add,
                            axis=AX.X,
                        )
                    moyp = stp.tile([128, 1], F32, name=f"moyp{ep}")
                    nc.vector.tensor_reduce(
                        out=moyp[:], in_=moysum[:], op=OP.add, axis=AX.X
                    )
                    nc.sync.dma_start(out=cc_moy_in[:], in_=moyp[:])
                    nc.gpsimd.collective_compute(
                        "AllGather", OP.bypass, replica_groups=rg,
                        ins=[cc_moy_in[:]], outs=[cc_moy_out[:]],
                    )
                    gatm = stp.tile([128, NC], F32, name=f"gatm{ep}")
                    nc.sync.dma_start(
                        out=gatm[:].rearrange("p (f c) -> p f c", c=NC),
                        in_=cc_moy_out[:].rearrange("(c p) f -> p f c", p=128),
                    )
                    moyc = stp.tile([128, 1], F32, name=f"moyc{ep}")
                    nc.vector.tensor_reduce(
                        out=moyc[:].rearrange("p (f x) -> p f x", x=1),
                        in_=gatm[:].rearrange("p (f c) -> p f c", c=NC),
                        op=OP.add, axis=AX.X,
                    )
                    nc.vector.tensor_scalar(
                        out=moyc[:], in0=moyc[:], scalar1=1.0 / N, scalar2=None,
                        op0=OP.mult,
                    )
                    pbase = ps.tile([128, 128], F32, tag="pc")
                    nc.tensor.matmul(
                        out=pbase[:, 0:1], lhsT=W0a[:], rhs=moyc[:],
                        start=True, stop=True, skip_group_check=True,
                    )
                    base_col = stp.tile([128, 1], F32, name=f"base_col{ep}")
                    nc.vector.tensor_tensor(
                        out=base_col[:], in0=pbase[:, 0:1], in1=W0bc[:], op=OP.add
                    )

                    # ================= readout =================
                    tab2 = load_tab(2, "ro")
                    for gi, (c0, gsz) in enumerate(groups):
                        if True:
                            w = gsz * 128
                            t, ci0, S = chunks[c0]
                            G = 128 // S
                            gvr = group_gather_sb(c0, gsz, tab2)
                            pm1 = ps.tile([128, 512], F32, tag="pa")
                            band = sp.tile(
                                [128, 512], BF16, tag="hbt", bufs=2, name="bandb"
                            )
                            nc.sync.dma_start(
                                out=band[:G, : gsz * 128].rearrange(
                                    "p (k c) -> p k c", c=128
                                ),
                                in_=hlocal[
                                    t * 128 + ci0 * G : t * 128 + (ci0 + gsz) * G, :
                                ].rearrange("(k p) c -> p k c", p=G),
                            )
                            for k in range(gsz):
                                nc.tensor.matmul(
                                    out=pm1[:, k * 128 : (k + 1) * 128],
                                    lhsT=band[:G, k * 128 : (k + 1) * 128],
                                    rhs=kron_bb[kron_of[S]][:G, :],
                                    start=True, stop=True,
                                    skip_group_check=True,
                                )
                            zs = sp.tile([128, 512], F32, tag="zsum", bufs=2)
                            nc.vector.tensor_tensor(
                                out=zs[:, :w], in0=pm1[:, :w],
                                in1=gvr[:, :w], op=OP.add,
                            )
                            t1 = sp.tile([128, 512], BF16, tag="wsb", bufs=2)
                            nc.scalar.activation(
                                out=t1[:, :w], in_=zs[:, :w], func=AF.Relu,
                                bias=base_col[:],
                            )
                            pt2 = ps.tile([128, 512], F32, tag="pb")
                            nc.tensor.matmul(
                                out=pt2[:, :w], lhsT=Wk[0][:], rhs=t1[:, :w],
                                start=True, stop=True, skip_group_check=True,
                            )
                            t2 = sp.tile([128, 512], BF16, tag="zm", bufs=2)
                            if USE_TS:
                                nc.vector.tensor_scalar(
                                    out=t2[:, :w], in0=pt2[:, :w],
                                    scalar1=Wkb[0][:], scalar2=0.0,
                                    op0=OP.add, op1=OP.max,
                                )
                            else:
                                nc.scalar.activation(
                                    out=t2[:, :w], in_=pt2[:, :w], func=AF.Relu,
                                    bias=Wkb[0][:],
                                )
                            pt3 = ps.tile([128, 512], F32, tag="pc")
                            nc.tensor.matmul(
                                out=pt3[:, :w], lhsT=Wk[1][:], rhs=t2[:, :w],
                                start=True, stop=True, skip_group_check=True,
                            )
                            t3 = sp.tile([128, 512], BF16, tag="sq", bufs=2)
                            nc.scalar.activation(
                                out=t3[:, :w], in_=pt3[:, :w], func=AF.Relu,
                                bias=Wkb[1][:],
                            )
                            py = ps.tile([1, 512], F32, tag="pdy", bufs=2, name="py")
                            nc.tensor.matmul(
                                out=py[:, :w], lhsT=Wf[:], rhs=t3[:, :w],
                                start=True, stop=True, skip_group_check=True,
                            )
                            ysb = sp.tile([1, 512], F32, tag="ysb", bufs=2)
                            nc.scalar.activation(
                                out=ysb[:, :w], in_=py[:, :w], func=AF.Sigmoid,
                                bias=wfb[:],
                            )
                            nc.sync.dma_start(
                                out=y_out[0:1, c0 * 128 : c0 * 128 + w],
                                in_=ysb[:, :w],
                            )


            for _ep in range(epochs):
                _epoch(_ep)

    nc.compile()
    return nc


# ---------------------------------------------------------------------------
# top level
# ---------------------------------------------------------------------------


def _make_kron(S):
    G = 128 // S
    k = np.zeros((128, 128), np.float32)
    for p in range(128):
        g = p % G
        k[p, g * S : (g + 1) * S] = 1.0
    return k


def _prep(inputs):
    """plan + per-core input maps + origids (host-side prep)."""
    import ml_dtypes

    BF = ml_dtypes.bfloat16
    h = np.asarray(inputs["h"], np.float32)
    e = np.asarray(inputs["e"], np.float32)
    src = np.asarray(inputs["src"]).astype(np.int64)
    dst = np.asarray(inputs["dst"]).astype(np.int64)
    N = h.shape[0]

    plan = _plan(src, dst, N)
    plan["F_N"] = h.shape[1]
    plan["F_E"] = e.shape[1]

    U = np.asarray(inputs["U"], np.float32)
    V = np.asarray(inputs["V"], np.float32)
    A = np.asarray(inputs["A"], np.float32)
    B = np.asarray(inputs["B"], np.float32)
    C = np.asarray(inputs["C"], np.float32)
    W0_w = np.asarray(inputs["W0_w"], np.float32)
    Wk_w = np.asarray(inputs["Wk_w"], np.float32)
    Wk_b = np.asarray(inputs["Wk_b"], np.float32)
    Wf_w = np.asarray(inputs["Wf_w"], np.float32)
    Wf_b = np.asarray(inputs["Wf_b"], np.float32)

    S_vals = sorted(set(plan["S_list"]))
    krons = np.stack([_make_kron(s) for s in S_vals])
    fill2 = np.zeros((2, 256), np.float32)
    fill2[1, :] = -1e30

    WMAP, RW, BMAP, RB = _pack_layout(plan)
    wpack = np.zeros((RW, 384), np.float32)
    bpack = np.zeros((RB, 256), np.float32)

    def wput(name, arr):
        arr = np.atleast_2d(np.asarray(arr, np.float32))
        r0, rows = WMAP[name]
        assert arr.shape[0] == rows, (name, arr.shape)
        wpack[r0 : r0 + rows, : arr.shape[1]] = arr

    def bput(name, arr):
        arr = np.atleast_2d(np.asarray(arr, np.float32))
        r0, rows = BMAP[name]
        assert arr.shape[0] == rows, (name, arr.shape)
        bpack[r0 : r0 + rows, : arr.shape[1]] = arr

    wput("ones_row", np.ones((1, 128), np.float32))
    wput("emb_e_b", np.asarray(inputs["emb_e_b"], np.float32).reshape(1, 128))
    wput("emb_n_b", np.asarray(inputs["emb_n_b"], np.float32).reshape(1, 128))
    bput("emb_e_w", np.asarray(inputs["emb_e_w"], np.float32))
    bput("emb_n_w", np.asarray(inputs["emb_n_w"], np.float32))
    bput("A0", A[0])
    bput("A1", A[1])
    wput("VCB0", np.concatenate([V[0], C[0], B[0]], axis=1))
    wput("V1", V[1])
    wput("U0", U[0])
    wput("U1", U[1])
    wput("WBC", np.concatenate([W0_w[128:256], W0_w[256:384]], axis=1))
    wput("W0a", W0_w[:128])
    wput("W0b_col", np.asarray(inputs["W0_b"], np.float32).reshape(1, 128))
    wput("Wkb0", Wk_b[0].reshape(1, 128))
    wput("Wkb1", Wk_b[1].reshape(1, 128))
    wput("wfb", np.full((1, 1), float(Wf_b), np.float32))
    bput("fill2b", fill2)
    for i, s in enumerate(S_vals):
        bput(f"kronsb{i}", krons[i])
    bput("Wk0", Wk_w[0])
    bput("Wk1", Wk_w[1])
    bput("Wf", Wf_w.reshape(1, 128))

    shared = dict(wpack=wpack, bpack=bpack.astype(BF))

    in_maps = []
    origids = []
    for d in range(NC):
        pc = _per_core_arrays(plan, d, h, e)
        origids.append(pc.pop("origid"))
        m = dict(pc)
        m.update(shared)
        in_maps.append(m)
    return plan, in_maps, origids


def kernel(**inputs):
    import sys

    if "/opt/trn_rl_repo" not in sys.path:
        sys.path.insert(0, "/opt/trn_rl_repo")
    from concourse.bass_utils import run_bass_kernel_spmd

    plan, in_maps, origids = _prep(inputs)
    nc = _build_program(plan)
    res = run_bass_kernel_spmd(nc, in_maps, list(range(NC)))

    E = plan["E"]
    out = np.zeros(E, np.float32)
    for d in range(NC):
        y = np.asarray(res.results[d]["y"]).reshape(-1)
        oid = origids[d]
        valid = oid >= 0
        out[oid[valid]] = y[valid]
    return out

